# revision 46
# baseline (speedup 1.0000x reference)
"""TRN2 Bass kernel for nn_Block_6476810682806 (dense_cnn).

Bottleneck block: 1x1 kerv -> BN -> 3x3 kerv -> BN -> 1x1 kerv -> BN -> +residual,
where kerv(x) = (conv(x) + 1)^2 and BN is inference-mode (frozen stats).

Distribution: data-parallel over batch (128 -> 16 per core) across 8 cores,
weights replicated. Each core computes its shard fully independently.

Device strategy (per core):
  - everything that crosses DMA is bf16 (halves HBM traffic; norm_rel ~5.6e-3
    vs the 2e-2 gate). Host pre-transposes x/out to channel-major so bf16
    descriptor runs stay >=512B (smaller runs pay a 2x DMA latency penalty).
  - activations channel-major: [C partitions, batch*spatial free]
  - convs as PE matmuls in bf16 (1 cyc/row at any N), f32 PSUM accumulate
  - 3x3 conv: 9 shifted matmuls over a zero-padded per-image 16x16 SBUF plane
  - BN scale folded into the kervolution square on ACT:
        s*(y+1)^2 = (sqrt(s)*y + sqrt(s))^2  (requires s > 0)
    shifts (t = b - m*s) are zero for this problem's fills; generic paths
    emit an extra per-channel add / affine when they are not.
  - residual add on DVE, straight from the resident x tiles (all-bf16 SBUF
    operands hit the DVE 2x/4x fast modes)
  - pass plan 4+4+4+2+2 images: small final passes shrink the tail drain
    (ACT pointwise + DVE residual + store DMA after the last matmul)
"""

import numpy as np

import concourse.bacc as bacc
import concourse.mybir as mybir
import concourse.tile as tile

F32 = mybir.dt.float32
BF = mybir.dt.bfloat16
EPS = 1e-5

B = 16          # images per core
C_IN = 1024
C_MID = 256
HW = 14
S = HW * HW     # 196
N = 2 * S       # matmul moving size per image pair = 392
PAD = 16        # padded plane side
PS = PAD * PAD  # 256 padded plane size
K1 = C_IN // 128          # 8
K2 = C_MID // 128         # 2
M1 = C_MID // 128         # 2
M3 = C_IN // 128          # 8
MAXBP = 4                 # max images per pass

# layer modes
FAST_T0 = 0   # all s>0, all t==0: ACT-only pointwise
FAST_T = 1    # all s>0, some t!=0: ACT + per-channel add
SLOW = 2      # some s<=0: plain square on ACT + DVE affine

# packed scale/bias column offsets in scb [128, 24]
SC1, BI1, SC2, BI2, SC3, BI3 = 0, 2, 4, 6, 8, 16
# packed shift column offsets in shb [128, 12]
SH1, SH2, SH3 = 0, 2, 4


def _build(modes, reps=None):
    mode1, mode2, mode3 = modes
    nc = bacc.Bacc("TRN2", target_bir_lowering=False, debug=False)

    x_d = nc.dram_tensor("x", [K1, 128, B, S], BF, kind="ExternalInput").ap()
    w1_d = nc.dram_tensor("w1t", [C_IN, C_MID], BF, kind="ExternalInput").ap()
    w2_d = nc.dram_tensor("w2t", [K2, 9, 128, C_MID], BF, kind="ExternalInput").ap()
    w3_d = nc.dram_tensor("w3t", [C_MID, C_IN], BF, kind="ExternalInput").ap()
    scb_d = nc.dram_tensor("scb", [128, 24], F32, kind="ExternalInput").ap()
    shb_d = nc.dram_tensor("shb", [128, 12], F32, kind="ExternalInput").ap()
    out_d = nc.dram_tensor("out", [M3, 128, B, S], BF, kind="ExternalOutput").ap()

    x_cm = x_d.rearrange("k p n s -> p k n s")     # [128, 8, 16, 196]
    out_cm = out_d.rearrange("m p n s -> p m n s")  # [128, 8, 16, 196]

    Sq = mybir.ActivationFunctionType.Square
    Alu = mybir.AluOpType

    with tile.TileContext(nc) as tc:
        with (
            tc.tile_pool(name="wpool", bufs=1) as wpool,
            tc.tile_pool(name="xpool", bufs=4) as xpool,
            tc.tile_pool(name="h1pool", bufs=2) as h1pool,
            tc.tile_pool(name="h2pool", bufs=2) as h2pool,
            tc.tile_pool(name="tpool", bufs=2) as tpool,
            tc.tile_pool(name="opool", bufs=4) as opool,
            tc.tile_pool(name="psp", bufs=4, space="PSUM") as pspool,
        ):
            # every PSUM tile is 2 banks; accumulation groups go to the
            # bank-aligned halves [0:N] and [HB:HB+N], drained by ONE
            # strided ACT op (halves the per-op init overhead share)
            HB = 512
            def xcol(xh, k, j):
                # [128, 2, S] rhs slice for k-tile k, image pair j
                v = xh[j][:].rearrange("p (k n s) -> p k n s", k=K1, n=2)
                return v[:, k, :, :]

            # ---- startup: one serialized DMA stream (SP queue) in first-use
            # order: xj0, scale vec, w1, w2, xj1, w3 ----
            def load_xj(pair, j):
                # pair: global image-pair index 0..7; j: slot parity in pass
                t = xpool.tile([128, K1 * 2 * S], BF, tag=f"x{j}",
                               name=f"xt_q{pair}")
                nc.sync.dma_start(
                    t[:].rearrange("p (k n s) -> p k n s", k=K1, n=2),
                    x_cm[:, :, 2 * pair:2 * pair + 2, :])
                return t

            # first x pair split into k-halves so conv1's first matmuls can
            # start ~1us sooner (w1 slots between the halves)
            xj0 = xpool.tile([128, K1 * 2 * S], BF, tag="x0", name="xt_q0")
            xj0v = xj0[:].rearrange("p (k n s) -> p k n s", k=K1, n=2)
            nc.sync.dma_start(xj0v[:, 0:K1 // 2], x_cm[:, 0:K1 // 2, 0:2, :])
            w1view = w1_d.rearrange("(k p) o -> p k o", p=128)
            w1s = wpool.tile([128, K1 * C_MID], BF, tag="w1s")
            w1v = w1s[:].rearrange("p (k o) -> p k o", k=K1)
            nc.sync.dma_start(w1v[:], w1view[:])
            nc.sync.dma_start(xj0v[:, K1 // 2:], x_cm[:, K1 // 2:, 0:2, :])
            scb = wpool.tile([128, 24], F32, tag="scb")
            nc.sync.dma_start(scb[:], scb_d)
            if any(mo[0] != FAST_T0 for mo in modes):
                shb = wpool.tile([128, 12], F32, tag="shb")
                nc.sync.dma_start(shb[:], shb_d)
            else:
                shb = None
            xj1 = load_xj(1, 1)
            xt0 = [xj0, xj1]
            w2view = w2_d.rearrange("k t p o -> p (k t) o")
            w2s = wpool.tile([128, 18 * C_MID], BF, tag="w2s")
            w2v = w2s[:].rearrange("p (kt o) -> p kt o", kt=18)
            nc.sync.dma_start(w2v[:], w2view[:])
            w3s = wpool.tile([128, K2 * C_IN], BF, tag="w3s")
            nc.sync.dma_start(
                w3s[:].rearrange("p (k o) -> p k o", k=K2),
                w3_d.rearrange("(k p) o -> p k o", p=128))

            def w1ap(k, m):
                return w1s[:, k * C_MID + m * 128: k * C_MID + (m + 1) * 128]

            def w2ap(kt, m):
                return w2s[:, kt * C_MID + m * 128: kt * C_MID + (m + 1) * 128]

            def w3ap(k, m):
                return w3s[:, k * C_IN + m * 128: k * C_IN + (m + 1) * 128]

            def pointwise(lmode, src_ap, out_ap, sc_off, sh_off, m):
                """out = s*(src+1)^2 + t, written to out_ap.

                lmode is (mode, const): const is sqrt(s) as a python float
                when s is channel-uniform (allows m-paired drains), else
                None (per-channel scb column; src must be single-m)."""
                mode, const = lmode
                if mode == SLOW:
                    nc.scalar.activation(out_ap, src_ap, Sq, bias=1.0, scale=1.0)
                    nc.vector.tensor_scalar(
                        out_ap, out_ap, scb[:, sc_off + m:sc_off + m + 1],
                        shb[:, sh_off + m:sh_off + m + 1], Alu.mult, Alu.add)
                else:
                    # for m-paired drains (const flag set) the scale is
                    # channel-uniform, so the first m's column is valid for
                    # the whole pair
                    nc.scalar.activation(
                        out_ap, src_ap, Sq,
                        bias=scb[:, sc_off + (M1 if sc_off < SC3 else M3) + m:
                                 sc_off + (M1 if sc_off < SC3 else M3) + m + 1],
                        scale=scb[:, sc_off + m:sc_off + m + 1])
                    if mode == FAST_T:
                        nc.vector.tensor_scalar(
                            out_ap, out_ap, shb[:, sh_off + m:sh_off + m + 1],
                            None, Alu.add)

            def pointwise_dve(src_ap, dst_ap, sc_off, m, nelem):
                """conv1 pointwise on DVE (FAST_T0 only): frees the ACT
                queue for conv3 drains at pass boundaries.
                t = sqrt(s)*y + sqrt(s); dst = t*t."""
                tq = tpool.tile([128, 2 * N], BF, tag="tq")
                tv = tq[:, 0:nelem]
                if nelem > N:
                    tv = tv.rearrange("c (j b) -> c j b", b=N)
                nc.vector.tensor_scalar(
                    tv, src_ap, scb[:, sc_off + m:sc_off + m + 1],
                    scb[:, sc_off + m:sc_off + m + 1], Alu.mult, Alu.add)
                tsq = (tq[:, 0:nelem]
                       .rearrange("c (n a b) -> c n a b", a=HW, b=HW))
                nc.vector.tensor_tensor(dst_ap, tsq, tsq, Alu.mult)

            # ---- PE warmup: dummy matmuls on scratch data keep the PE
            # clock ramping while the startup DMAs land; the early dummy
            # activation pulls the act-table load off the critical path ----
            wu = wpool.tile([128, 128], BF, tag="wu")
            nc.vector.memset(wu[:].bitcast(F32), 0.0)
            wusc = wpool.tile([128, 4], F32, tag="wusc")
            # act-table preload reads SBUF (reading the warmup PSUM tile
            # would WAR-serialize every warmup matmul behind the 1.3us
            # LoadActFuncSet)
            nc.scalar.activation(wusc[:], wu[:].bitcast(F32)[:, 0:4], Sq,
                                 bias=1.0, scale=1.0)
            wups = pspool.tile([128, 2 * HB], F32, tag="psp", name="wups")
            NWU = 120
            for i in range(NWU):
                nc.tensor.matmul(wups[:, 0:64], wu[:], wu[:, 0:64],
                                 start=(i == 0), stop=(i == NWU - 1))

            # ---- main passes: (first image pair index, n pairs) ----
            def alloc_h1(npairs):
                h1 = []
                for k in range(K2):
                    t = h1pool.tile([128, MAXBP * PS], BF, tag=f"h1_{k}",
                                    name=f"h1t{k}")
                    nc.gpsimd.memset(t[:, 0:2 * npairs * PS].bitcast(F32),
                                     0.0)
                    h1.append(t)
                return h1

            def conv1_chunks(pi, npairs, xt, h1):
                """Returns a list of emitter callables (2 chunks) for this
                pass's conv1; each chunk is one PSUM tile's worth."""
                def emit_pair(m):
                    # pair (j0,j1) per m: PSUM halves, one drain per m
                    ps = pspool.tile([128, 2 * HB], F32, tag="psp",
                                     name=f"c1ps{m}")
                    for j in range(2):
                        for k in range(K1):
                            nc.tensor.matmul(
                                ps[:, j * HB:j * HB + N],
                                w1ap(k, m), xcol(xt, k, j),
                                start=(k == 0), stop=(k == K1 - 1))
                    src = ps[:].rearrange("c (j b) -> c j b", j=2)[:, :, 0:N]
                    dst = (h1[m][:]
                           .rearrange("c (n a b) -> c n a b", a=PAD, b=PAD)
                           [:, 0:4, 1:1 + HW, 1:1 + HW])
                    if mode1[0] == FAST_T0:
                        pointwise_dve(src, dst, SC1, m, 2 * N)
                    else:
                        pointwise(mode1, src, dst, SC1, SH1, m)

                def emit_single(j, m):
                    ps = pspool.tile([128, 2 * HB], F32, tag="psp",
                                     name=f"c1ps{j}_{m}")
                    for k in range(K1):
                        nc.tensor.matmul(
                            ps[:, 0:N], w1ap(k, m), xcol(xt, k, j),
                            start=(k == 0), stop=(k == K1 - 1))
                    dst = (h1[m][:]
                           .rearrange("c (n a b) -> c n a b", a=PAD, b=PAD)
                           [:, 2 * j:2 * j + 2, 1:1 + HW, 1:1 + HW])
                    if mode1[0] == FAST_T0:
                        pointwise_dve(ps[:, 0:N], dst, SC1, m, N)
                    else:
                        pointwise(mode1, ps[:, 0:N], dst, SC1, SH1, m)

                def emit_j0_khalves():
                    # startup: both m groups in one tile, k-halves
                    # interleaved, so matmuls start on the first half-x DMA
                    ps = pspool.tile([128, 2 * HB], F32, tag="psp",
                                     name="c1ps_j0")
                    for khalf in range(2):
                        for m in range(M1):
                            for k in range(4 * khalf, 4 * khalf + 4):
                                nc.tensor.matmul(
                                    ps[:, m * HB:m * HB + N],
                                    w1ap(k, m), xcol(xt, k, 0),
                                    start=(k == 0), stop=(k == K1 - 1),
                                    skip_group_check=True)
                    for m in range(M1):
                        dst = (h1[m][:]
                               .rearrange("c (n a b) -> c n a b",
                                          a=PAD, b=PAD)
                               [:, 0:2, 1:1 + HW, 1:1 + HW])
                        if mode1[0] == FAST_T0:
                            pointwise_dve(ps[:, m * HB:m * HB + N],
                                          dst, SC1, m, N)
                        else:
                            pointwise(mode1, ps[:, m * HB:m * HB + N],
                                      dst, SC1, SH1, m)

                if npairs == 2 and pi == 0:
                    # j-outer so conv1(j0) never waits on the xj1 DMA (a
                    # long stall would also reset the PE p-state clock)
                    return [emit_j0_khalves,
                            lambda: [emit_single(1, m) for m in range(M1)]]
                if npairs == 2:
                    return [lambda m=m: emit_pair(m) for m in range(M1)]
                return [lambda m=m: emit_single(0, m) for m in range(M1)]

            def emit_passes():
              plan = [(0, 2), (2, 2), (4, 2), (6, 1), (7, 1)]
              # prefetch: emit pass p+1's x loads at the START of pass p so
              # they sit ahead of pass p's store DMAs in SP queue order
              xt_next = xt0
              h1_next = None
              for pi, (q0, npairs) in enumerate(plan):
                xt = xt_next
                if pi + 1 < len(plan):
                    nq0, nnp = plan[pi + 1]
                    xt_next = [load_xj(nq0 + j, j) for j in range(nnp)]

                if pi == 0:
                    h1 = alloc_h1(npairs)
                    for c in conv1_chunks(pi, npairs, xt, h1):
                        c()
                else:
                    h1 = h1_next  # conv1 already emitted inside pass pi-1

                # h2 is one k-major tile so a paired (m0,m1) ACT drain has a
                # single dst AP
                h2a = h2pool.tile([128, K2 * MAXBP * S], BF, tag="h2")

                def h2ap(k, j, nj=1):
                    return h2a[:, k * MAXBP * S + j * N:
                               k * MAXBP * S + (j + nj) * N]

                # conv1 for pass pi+1 is emitted interleaved into this
                # pass's conv3 (software pipelining: PE fills ACT's drain
                # lag with conv1 matmuls whose pointwise runs on DVE)
                if pi + 1 < len(plan):
                    h1_next = alloc_h1(plan[pi + 1][1])
                    next_chunks = conv1_chunks(pi + 1, plan[pi + 1][1],
                                               xt_next, h1_next)
                else:
                    next_chunks = []

                # conv2: 3x3 pad 1, C_MID -> C_MID
                # (m0,m1) paired per j when the scale is channel-uniform
                h1v = [t[:].rearrange("c (n a b) -> c n a b", a=PAD, b=PAD)
                       for t in h1]
                pair2 = mode2[1] is not None
                for j in range(npairs):
                    ps = pspool.tile([128, 2 * HB], F32, tag="psp")
                    # k-phase-major: all k0 taps (both m groups) before any
                    # k1 tap, so the k1 taps' h1-plane dependency (conv1's
                    # second ACT drain) resolves while k0 taps execute
                    for k in range(K2):
                        for m in range(M1):
                            for tp in range(9):
                                kh, kw = tp // 3, tp % 3
                                rhs = h1v[k][:, 2 * j:2 * j + 2,
                                             kh:kh + HW, kw:kw + HW]
                                nc.tensor.matmul(
                                    ps[:, m * HB:m * HB + N],
                                    w2ap(k * 9 + tp, m), rhs,
                                    start=(k == 0 and tp == 0),
                                    stop=(k == K2 - 1 and tp == 8),
                                    skip_group_check=True)
                    if pair2 and npairs == 2:
                        src = (ps[:].rearrange("c (m b) -> c m b", m=2)
                               [:, :, 0:N])
                        dst = (h2a[:].rearrange("c (k t) -> c k t", k=K2)
                               [:, :, j * N:(j + 1) * N])
                        pointwise(mode2, src, dst, SC2, SH2, 0)
                    else:
                        # small pass: unpaired so the k0 plane (m0 drain)
                        # lands while m1's taps still execute
                        for m in range(M1):
                            pointwise(mode2, ps[:, m * HB:m * HB + N],
                                      h2ap(m, j), SC2, SH2, m)

                # conv3: 1x1, C_MID -> C_IN, (2mp, 2mp+1) paired per j when
                # uniform; + residual, store per (m-pair, j)
                pair3 = mode3[1] is not None
                # interleave next-pass conv1 chunks where ring/drain waits
                # would otherwise stall PE: before group 1 (covers the
                # conv2-j1/c2 drain latency) and mid-phase
                if npairs == 2:
                    c1_at = {1: 0, 4: 1}
                else:
                    c1_at = {0: 0, 2: 1}
                gidx = 0
                for mp in range(M3 // 2):
                    for j in range(npairs):
                        if gidx in c1_at and c1_at[gidx] < len(next_chunks):
                            next_chunks[c1_at[gidx]]()
                        gidx += 1
                        ps = pspool.tile([128, 2 * HB], F32, tag="psp")
                        for mi in range(2):
                            m = 2 * mp + mi
                            for k in range(K2):
                                nc.tensor.matmul(
                                    ps[:, mi * HB:mi * HB + N],
                                    w3ap(k, m), h2ap(k, j),
                                    start=(k == 0), stop=(k == K2 - 1))
                        zt = opool.tile([128, 2 * N], BF, tag="z")
                        last_pass = pi == len(plan) - 1
                        if (last_pass and pair3 and mode3[0] == FAST_T0
                                and mp == 1):
                            # tail relief: one group's pointwise on DVE so
                            # the final ACT drain train is shorter
                            tq3 = tpool.tile([128, 2 * N], BF, tag="tq")
                            tv3 = tq3[:].rearrange("c (m b) -> c m b", b=N)
                            src = (ps[:].rearrange("c (m b) -> c m b", m=2)
                                   [:, :, 0:N])
                            nc.vector.tensor_scalar(
                                tv3, src,
                                scb[:, SC3 + 2 * mp:SC3 + 2 * mp + 1],
                                scb[:, SC3 + 2 * mp:SC3 + 2 * mp + 1],
                                Alu.mult, Alu.add)
                            nc.vector.tensor_tensor(zt[:], tq3[:], tq3[:],
                                                    Alu.mult)
                        elif last_pass and mp == M3 // 2 - 1:
                            # final group: single-m drains + residuals (the
                            # chain after the very last matmul halves), but
                            # ONE paired store (two stores would serialize
                            # their HWDGE generations + DGE delays)
                            for mi in range(2):
                                pointwise(mode3, ps[:, mi * HB:mi * HB + N],
                                          zt[:, mi * N:(mi + 1) * N],
                                          SC3, SH3, 2 * mp + mi)
                                zvi = (zt[:, mi * N:(mi + 1) * N]
                                       .rearrange("c (n s) -> c n s", n=2))
                                xvi = (xt[j][:]
                                       .rearrange("p (k n s) -> p k n s",
                                                  k=K1, n=2)
                                       [:, 2 * mp + mi, :, :])
                                nc.vector.tensor_tensor(zvi, zvi, xvi,
                                                        Alu.add)
                            zv = zt[:].rearrange("c (m n s) -> c m n s",
                                                 m=2, n=2)
                            dst = out_cm[:, 2 * mp:2 * mp + 2,
                                         2 * (q0 + j):2 * (q0 + j) + 2, :]
                            nc.sync.dma_start(dst, zv)
                            continue
                        elif pair3:
                            src = (ps[:].rearrange("c (m b) -> c m b", m=2)
                                   [:, :, 0:N])
                            pointwise(mode3, src, zt[:], SC3, SH3, 2 * mp)
                        else:
                            for mi in range(2):
                                pointwise(mode3, ps[:, mi * HB:mi * HB + N],
                                          zt[:, mi * N:(mi + 1) * N],
                                          SC3, SH3, 2 * mp + mi)
                        zv = zt[:].rearrange("c (m n s) -> c m n s",
                                             m=2, n=2)
                        xv = (xt[j][:].rearrange("p (k n s) -> p k n s",
                                                 k=K1, n=2)
                              [:, 2 * mp:2 * mp + 2, :, :])
                        nc.vector.tensor_tensor(zv, zv, xv, Alu.add)
                        dst = out_cm[:, 2 * mp:2 * mp + 2,
                                     2 * (q0 + j):2 * (q0 + j) + 2, :]
                        nc.sync.dma_start(dst, zv)

            if reps is None:
                emit_passes()
            else:
                with tc.For_i(0, reps, 1):
                    emit_passes()

    nc.compile()
    return nc


# ---------------- host side ----------------

_CACHE = {}


def _get_runner(modes):
    if modes in _CACHE:
        return _CACHE[modes]
    import jax
    from jax.experimental.shard_map import shard_map
    from jax.sharding import Mesh, PartitionSpec
    from concourse.bass2jax import (_bass_exec_p, install_neuronx_cc_hook,
                                    partition_id_tensor)

    nc = _build(modes)
    install_neuronx_cc_hook()
    partition_name = nc.partition_id_tensor.name if nc.partition_id_tensor else None
    in_names, out_names, out_avals = [], [], []
    for alloc in nc.m.functions[0].allocations:
        if not isinstance(alloc, mybir.MemoryLocationSet):
            continue
        name = alloc.memorylocations[0].name
        if alloc.kind == "ExternalInput":
            if name != partition_name:
                in_names.append(name)
        elif alloc.kind == "ExternalOutput":
            out_names.append(name)
            out_avals.append(jax.core.ShapedArray(
                tuple(alloc.tensor_shape), mybir.dt.np(alloc.dtype)))
    n_params, n_outs = len(in_names), len(out_avals)
    all_in_names = list(in_names) + list(out_names)
    if partition_name is not None:
        all_in_names.append(partition_name)

    def _body(*args):
        operands = list(args)
        if partition_name is not None:
            operands.append(partition_id_tensor())
        outs = _bass_exec_p.bind(
            *operands,
            out_avals=tuple(out_avals),
            in_names=tuple(all_in_names),
            out_names=tuple(out_names),
            lowering_input_output_aliases=(),
            sim_require_finite=True,
            sim_require_nnan=True,
            nc=nc,
        )
        return tuple(outs)

    devices = jax.devices()[:8]
    mesh = Mesh(np.asarray(devices), ("core",))
    sharded = jax.jit(
        shard_map(_body, mesh=mesh,
                  in_specs=(PartitionSpec("core"),) * (n_params + n_outs),
                  out_specs=(PartitionSpec("core"),) * n_outs,
                  check_rep=False),
        donate_argnums=tuple(range(n_params, n_params + n_outs)),
        keep_unused=True,
    )
    sharding = jax.sharding.NamedSharding(mesh, PartitionSpec("core"))
    runner = dict(nc=nc, sharded=sharded, sharding=sharding, jax=jax,
                  in_names=in_names, out_names=out_names, out_avals=out_avals)
    _CACHE[modes] = runner
    return runner


def _vec_tile(v, m_tiles):
    """[C] -> [128, m_tiles] column-per-m-tile layout."""
    return np.ascontiguousarray(np.asarray(v).reshape(m_tiles, 128).T
                                .astype(np.float32))


def _bf16(a):
    import ml_dtypes
    return np.asarray(a, dtype=np.float32).astype(ml_dtypes.bfloat16)


def prepare(w1, w2, w3, g1, b1, m1, v1, g2, b2, m2, v2, g3, b3, m3, v3):
    """Host prep: returns (modes, shared_input_dict_without_x)."""
    s1 = g1 / np.sqrt(v1 + EPS)
    t1 = b1 - m1 * s1
    s2 = g2 / np.sqrt(v2 + EPS)
    t2 = b2 - m2 * s2
    s3 = g3 / np.sqrt(v3 + EPS)
    t3 = b3 - m3 * s3

    def mode_of(s, t):
        """Returns (mode, const): const = sqrt(s) as a float when s is
        exactly channel-uniform and shifts are zero (enables m-paired
        PSUM drains on ACT), else None."""
        if np.all(s > 0):
            if not np.any(t):
                r = np.sqrt(s)
                const = float(r[0]) if np.all(s == s[0]) else None
                return (FAST_T0, const)
            return (FAST_T, None)
        return (SLOW, None)

    modes = (mode_of(s1, t1), mode_of(s2, t2), mode_of(s3, t3))

    def sc_bi(lmode, s, m_tiles):
        mode = lmode[0]
        if mode == SLOW:
            return _vec_tile(s, m_tiles), np.ones((128, m_tiles), np.float32)
        r = np.sqrt(s)
        return _vec_tile(r, m_tiles), _vec_tile(r, m_tiles)

    sc1, bi1 = sc_bi(modes[0], s1, M1)
    sc2, bi2 = sc_bi(modes[1], s2, M1)
    sc3, bi3 = sc_bi(modes[2], s3, M3)
    scb = np.concatenate([sc1, bi1, sc2, bi2, sc3, bi3], axis=1)
    shb = np.concatenate([_vec_tile(t1, M1), _vec_tile(t2, M1),
                          _vec_tile(t3, M3)], axis=1)

    w1t = _bf16(np.ascontiguousarray(w1[:, :, 0, 0].T))          # [1024,256]
    # w2: [o, i, kh, kw] -> [k, tap, i_local, o]
    w2t = _bf16(np.ascontiguousarray(
        w2.transpose(1, 2, 3, 0)                  # [i, kh, kw, o]
          .reshape(K2, 128, 9, C_MID)             # [k, i_local, tap, o]
          .transpose(0, 2, 1, 3)))                # [k, tap, i_local, o]
    w3t = _bf16(np.ascontiguousarray(w3[:, :, 0, 0].T))          # [256,1024]

    shared = dict(w1t=w1t, w2t=w2t, w3t=w3t, scb=scb, shb=shb)
    return modes, shared


def kernel(**inputs):
    inputs = {k: np.asarray(v) for k, v in inputs.items()}
    x = inputs.pop("x").astype(np.float32)
    modes, shared = prepare(**inputs)
    r = _get_runner(modes)
    jax = r["jax"]

    n_cores = 8
    # x: [128, 1024, 14, 14] -> [core(8) x k(8), 128, 16, 196] bf16 channel-major
    x_cm = _bf16(x.reshape(8, B, K1, 128, S)
                 .transpose(0, 2, 3, 1, 4)
                 .reshape(n_cores * K1, 128, B, S))
    dev_in = []
    for name in r["in_names"]:
        if name == "x":
            cat = x_cm
        else:
            a = shared[name]
            cat = np.concatenate([a] * n_cores, axis=0)
        dev_in.append(jax.device_put(cat, r["sharding"]))
    zero_outs = [
        jax.device_put(np.zeros((n_cores * av.shape[0], *av.shape[1:]), av.dtype),
                       r["sharding"])
        for av in r["out_avals"]
    ]
    outs = r["sharded"](*dev_in, *zero_outs)
    jax.block_until_ready(outs)
    out = np.asarray(outs[r["out_names"].index("out")])
    # [core x m(8), 128, 16, 196] bf16 -> [128, 1024, 14, 14] f32
    return np.ascontiguousarray(
        out.reshape(n_cores, M3, 128, B, S)
           .transpose(0, 3, 1, 2, 4)
           .reshape(128, C_IN, HW, HW)).astype(np.float32)


# revision 62
# speedup vs baseline: 1.0686x; 1.0686x over previous
"""TRN2 Bass kernel for nn_Block_6476810682806 (dense_cnn).

Bottleneck block: 1x1 kerv -> BN -> 3x3 kerv -> BN -> 1x1 kerv -> BN -> +residual,
where kerv(x) = (conv(x) + 1)^2 and BN is inference-mode (frozen stats).

Distribution: data-parallel over batch (128 -> 16 per core) across 8 cores,
weights replicated. Each core computes its shard fully independently.

Device strategy (per core):
  - everything that crosses DMA is bf16 (halves HBM traffic; norm_rel ~5.6e-3
    vs the 2e-2 gate). Host pre-transposes x/out to channel-major so bf16
    descriptor runs stay >=512B (smaller runs pay a 2x DMA latency penalty).
  - activations channel-major: [C partitions, batch*spatial free]
  - convs as PE matmuls in bf16 (1 cyc/row at any N), f32 PSUM accumulate
  - 3x3 conv: 9 shifted matmuls over a zero-padded per-image 16x16 SBUF plane
  - BN scale folded into the kervolution square on ACT:
        s*(y+1)^2 = (sqrt(s)*y + sqrt(s))^2  (requires s > 0)
    shifts (t = b - m*s) are zero for this problem's fills; generic paths
    emit an extra per-channel add / affine when they are not.
  - residual add on DVE, straight from the resident x tiles (all-bf16 SBUF
    operands hit the DVE 2x/4x fast modes)
  - pass plan 4+4+4+2+2 images: small final passes shrink the tail drain
    (ACT pointwise + DVE residual + store DMA after the last matmul)
"""

import numpy as np

import concourse.bacc as bacc
import concourse.mybir as mybir
import concourse.tile as tile

F32 = mybir.dt.float32
BF = mybir.dt.bfloat16
EPS = 1e-5

B = 16          # images per core
C_IN = 1024
C_MID = 256
HW = 14
S = HW * HW     # 196
N = 2 * S       # matmul moving size per image pair = 392
PAD = 16        # padded plane side
PS = PAD * PAD  # 256 padded plane size
K1 = C_IN // 128          # 8
K2 = C_MID // 128         # 2
M1 = C_MID // 128         # 2
M3 = C_IN // 128          # 8
MAXBP = 4                 # max images per pass

# layer modes
FAST_T0 = 0   # all s>0, all t==0: ACT-only pointwise
FAST_T = 1    # all s>0, some t!=0: ACT + per-channel add
SLOW = 2      # some s<=0: plain square on ACT + DVE affine

# packed scale/bias column offsets in scb [128, 24]
SC1, BI1, SC2, BI2, SC3, BI3 = 0, 2, 4, 6, 8, 16
# packed shift column offsets in shb [128, 12]
SH1, SH2, SH3 = 0, 2, 4


def _build(modes, reps=None):
    mode1, mode2, mode3 = modes
    nc = bacc.Bacc("TRN2", target_bir_lowering=False, debug=False)

    x_d = nc.dram_tensor("x", [K1, 128, B, S], BF, kind="ExternalInput").ap()
    w1_d = nc.dram_tensor("w1t", [C_IN, C_MID], BF, kind="ExternalInput").ap()
    w2_d = nc.dram_tensor("w2t", [K2, 3, 4, 128, C_MID], BF,
                          kind="ExternalInput").ap()
    w3_d = nc.dram_tensor("w3t", [C_MID, C_IN], BF, kind="ExternalInput").ap()
    scb_d = nc.dram_tensor("scb", [128, 24], F32, kind="ExternalInput").ap()
    shb_d = nc.dram_tensor("shb", [128, 12], F32, kind="ExternalInput").ap()
    out_d = nc.dram_tensor("out", [M3, 128, B, S], BF, kind="ExternalOutput").ap()

    x_cm = x_d.rearrange("k p n s -> p k n s")     # [128, 8, 16, 196]
    out_cm = out_d.rearrange("m p n s -> p m n s")  # [128, 8, 16, 196]

    Sq = mybir.ActivationFunctionType.Square
    Alu = mybir.AluOpType

    with tile.TileContext(nc) as tc:
        with (
            tc.tile_pool(name="wpool", bufs=1) as wpool,
            tc.tile_pool(name="xpool", bufs=4) as xpool,
            tc.tile_pool(name="h1pool", bufs=2) as h1pool,
            tc.tile_pool(name="h2pool", bufs=2) as h2pool,
            tc.tile_pool(name="tpool", bufs=2) as tpool,
            tc.tile_pool(name="vpool", bufs=2) as vpool,
            tc.tile_pool(name="opool", bufs=4) as opool,
            tc.tile_pool(name="psp", bufs=4, space="PSUM") as pspool,
        ):
            # every PSUM tile is 2 banks; accumulation groups go to the
            # bank-aligned halves [0:N] and [HB:HB+N], drained by ONE
            # strided ACT op (halves the per-op init overhead share)
            HB = 512
            def xcol(xh, k, j):
                # [128, 2, S] rhs slice for k-tile k, image pair j
                v = xh[j][:].rearrange("p (k n s) -> p k n s", k=K1, n=2)
                return v[:, k, :, :]

            # ---- startup: one serialized DMA stream (SP queue) in first-use
            # order: xj0, scale vec, w1, w2, xj1, w3 ----
            def load_xj(pair, j):
                # pair: global image-pair index 0..7; j: slot parity in pass
                t = xpool.tile([128, K1 * 2 * S], BF, tag=f"x{j}",
                               name=f"xt_q{pair}")
                nc.sync.dma_start(
                    t[:].rearrange("p (k n s) -> p k n s", k=K1, n=2),
                    x_cm[:, :, 2 * pair:2 * pair + 2, :])
                return t

            # first x pair split into k-halves so conv1's first matmuls can
            # start ~1us sooner (w1 slots between the halves)
            xj0 = xpool.tile([128, K1 * 2 * S], BF, tag="x0", name="xt_q0")
            xj0v = xj0[:].rearrange("p (k n s) -> p k n s", k=K1, n=2)
            nc.sync.dma_start(xj0v[:, 0:K1 // 2], x_cm[:, 0:K1 // 2, 0:2, :])
            w1view = w1_d.rearrange("(k p) o -> p k o", p=128)
            w1s = wpool.tile([128, K1 * C_MID], BF, tag="w1s")
            w1v = w1s[:].rearrange("p (k o) -> p k o", k=K1)
            nc.sync.dma_start(w1v[:], w1view[:])
            nc.sync.dma_start(xj0v[:, K1 // 2:], x_cm[:, K1 // 2:, 0:2, :])
            scb = wpool.tile([128, 24], F32, tag="scb")
            nc.sync.dma_start(scb[:], scb_d)
            if any(mo[0] != FAST_T0 for mo in modes):
                shb = wpool.tile([128, 12], F32, tag="shb")
                nc.sync.dma_start(shb[:], shb_d)
            else:
                shb = None
            xj1 = load_xj(1, 1)
            xt0 = [xj0, xj1]
            w2view = w2_d.rearrange("k h j p o -> p (k h j) o")
            w2s = wpool.tile([128, 24 * C_MID], BF, tag="w2s")
            w2v = w2s[:].rearrange("p (kt o) -> p kt o", kt=24)
            nc.sync.dma_start(w2v[:], w2view[:])
            w3s = wpool.tile([128, K2 * C_IN], BF, tag="w3s")
            nc.sync.dma_start(
                w3s[:].rearrange("p (k o) -> p k o", k=K2),
                w3_d.rearrange("(k p) o -> p k o", p=128))

            def w1ap(k, m):
                return w1s[:, k * C_MID + m * 128: k * C_MID + (m + 1) * 128]

            def w2ap(kt, kh, j4, m):
                base = ((kt * 3 + kh) * 4 + j4) * C_MID + m * 128
                return w2s[:, base: base + 128]

            def w3ap(k, m):
                return w3s[:, k * C_IN + m * 128: k * C_IN + (m + 1) * 128]

            def pointwise(lmode, src_ap, out_ap, sc_off, sh_off, m):
                """out = s*(src+1)^2 + t, written to out_ap.

                lmode is (mode, const): const is sqrt(s) as a python float
                when s is channel-uniform (allows m-paired drains), else
                None (per-channel scb column; src must be single-m)."""
                mode, const = lmode
                if mode == SLOW:
                    nc.scalar.activation(out_ap, src_ap, Sq, bias=1.0, scale=1.0)
                    nc.vector.tensor_scalar(
                        out_ap, out_ap, scb[:, sc_off + m:sc_off + m + 1],
                        shb[:, sh_off + m:sh_off + m + 1], Alu.mult, Alu.add)
                else:
                    # for m-paired drains (const flag set) the scale is
                    # channel-uniform, so the first m's column is valid for
                    # the whole pair
                    nc.scalar.activation(
                        out_ap, src_ap, Sq,
                        bias=scb[:, sc_off + (M1 if sc_off < SC3 else M3) + m:
                                 sc_off + (M1 if sc_off < SC3 else M3) + m + 1],
                        scale=scb[:, sc_off + m:sc_off + m + 1])
                    if mode == FAST_T:
                        nc.vector.tensor_scalar(
                            out_ap, out_ap, shb[:, sh_off + m:sh_off + m + 1],
                            None, Alu.add)

            def pointwise_dve(src_ap, dst_ap, sc_off, m, nelem):
                """conv1 pointwise on DVE (FAST_T0 only): frees the ACT
                queue for conv3 drains at pass boundaries.
                t = sqrt(s)*y + sqrt(s); dst = t*t."""
                tq = tpool.tile([128, 2 * N], BF, tag="tq")
                tv = tq[:, 0:nelem]
                if nelem > N:
                    tv = tv.rearrange("c (j b) -> c j b", b=N)
                nc.vector.tensor_scalar(
                    tv, src_ap, scb[:, sc_off + m:sc_off + m + 1],
                    scb[:, sc_off + m:sc_off + m + 1], Alu.mult, Alu.add)
                tsq = (tq[:, 0:nelem]
                       .rearrange("c (n a b) -> c n a b", a=HW, b=HW))
                nc.vector.tensor_tensor(dst_ap, tsq, tsq, Alu.mult)

            # ---- PE warmup: dummy matmuls on scratch data keep the PE
            # clock ramping while the startup DMAs land; the early dummy
            # activation pulls the act-table load off the critical path ----
            wu = wpool.tile([128, 128], BF, tag="wu")
            nc.vector.memset(wu[:].bitcast(F32), 0.0)
            wusc = wpool.tile([128, 4], F32, tag="wusc")
            # act-table preload reads SBUF (reading the warmup PSUM tile
            # would WAR-serialize every warmup matmul behind the 1.3us
            # LoadActFuncSet)
            nc.scalar.activation(wusc[:], wu[:].bitcast(F32)[:, 0:4], Sq,
                                 bias=1.0, scale=1.0)
            wups = pspool.tile([128, 2 * HB], F32, tag="psp", name="wups")
            NWU = 120
            for i in range(NWU):
                nc.tensor.matmul(wups[:, 0:64], wu[:], wu[:, 0:64],
                                 start=(i == 0), stop=(i == NWU - 1))

            # ---- main passes: (first image pair index, n pairs) ----
            def alloc_h1(npairs):
                h1, vt = [], []
                for k in range(K2):
                    t = h1pool.tile([128, MAXBP * PS], BF, tag=f"h1_{k}",
                                    name=f"h1t{k}")
                    nc.gpsimd.memset(t[:, 0:2 * npairs * PS].bitcast(F32),
                                     0.0)
                    h1.append(t)
                    v = vpool.tile([128, MAXBP * 16 * 4 * 7], BF,
                                   tag=f"v_{k}", name=f"vt{k}")
                    vt.append(v)
                return h1, vt

            def emit_vtf(k, h1t, vtt, npairs):
                """1D-winograd input transform for one k-plane on GPSIMD:
                V0=d0-d2 V1=d1+d2 V2=d2-d1 V3=d1-d3 over width pairs."""
                nn = 2 * npairs
                hv = (h1t[:, 0:nn * PS]
                      .rearrange("c (n a b) -> c n a b", a=PAD, b=PAD))
                vv = (vtt[:, 0:nn * 16 * 4 * 7]
                      .rearrange("c (n r j t) -> c n r j t", n=nn, r=16, j=4))

                def dsel(c0, par):
                    # pad cols c0+2t+par for t=0..6 (stride-2 column pick)
                    return (hv[:, :, :, c0:c0 + 14]
                            .rearrange("c n r (t two) -> c n r t two", two=2)
                            [:, :, :, :, par])

                d0 = dsel(0, 0)
                d1 = dsel(0, 1)
                d2 = dsel(2, 0)
                d3 = dsel(2, 1)
                eng = nc.gpsimd
                eng.tensor_tensor(vv[:, :, :, 0, :], d0, d2, Alu.subtract)
                eng.tensor_tensor(vv[:, :, :, 1, :], d1, d2, Alu.add)
                eng.tensor_tensor(vv[:, :, :, 2, :], d2, d1, Alu.subtract)
                eng.tensor_tensor(vv[:, :, :, 3, :], d1, d3, Alu.subtract)

            def emit_vtf_dve_jmajor(h1, vt, npairs):
                """Pass-1 variant: transforms on DVE, freq-major across both
                k planes, so conv2's first freq GEMM unblocks after 2 ops."""
                nn = 2 * npairs
                for j4 in range(4):
                    for k in range(K2):
                        hv = (h1[k][:, 0:nn * PS]
                              .rearrange("c (n a b) -> c n a b",
                                         a=PAD, b=PAD))
                        vv = (vt[k][:, 0:nn * 16 * 4 * 7]
                              .rearrange("c (n r j t) -> c n r j t",
                                         n=nn, r=16, j=4))

                        def dsel(c0, par):
                            return (hv[:, :, :, c0:c0 + 14]
                                    .rearrange("c n r (t two) -> c n r t two",
                                               two=2)[:, :, :, :, par])

                        pairs = {0: (dsel(0, 0), dsel(2, 0), Alu.subtract),
                                 1: (dsel(0, 1), dsel(2, 0), Alu.add),
                                 2: (dsel(2, 0), dsel(0, 1), Alu.subtract),
                                 3: (dsel(0, 1), dsel(2, 1), Alu.subtract)}
                        a, b, op = pairs[j4]
                        nc.vector.tensor_tensor(vv[:, :, :, j4, :], a, b, op)

            def conv1_chunks(pi, npairs, xt, h1, vt):
                """Returns a list of emitter callables (2 chunks) for this
                pass's conv1; each chunk is one PSUM tile's worth."""
                def emit_pair(m):
                    # pair (j0,j1) per m: PSUM halves, one drain per m
                    ps = pspool.tile([128, 2 * HB], F32, tag="psp",
                                     name=f"c1ps{m}")
                    for j in range(2):
                        for k in range(K1):
                            nc.tensor.matmul(
                                ps[:, j * HB:j * HB + N],
                                w1ap(k, m), xcol(xt, k, j),
                                start=(k == 0), stop=(k == K1 - 1))
                    src = ps[:].rearrange("c (j b) -> c j b", j=2)[:, :, 0:N]
                    dst = (h1[m][:]
                           .rearrange("c (n a b) -> c n a b", a=PAD, b=PAD)
                           [:, 0:4, 1:1 + HW, 1:1 + HW])
                    pointwise(mode1, src, dst, SC1, SH1, m)

                def emit_single(j, m):
                    ps = pspool.tile([128, 2 * HB], F32, tag="psp",
                                     name=f"c1ps{j}_{m}")
                    for k in range(K1):
                        nc.tensor.matmul(
                            ps[:, 0:N], w1ap(k, m), xcol(xt, k, j),
                            start=(k == 0), stop=(k == K1 - 1))
                    dst = (h1[m][:]
                           .rearrange("c (n a b) -> c n a b", a=PAD, b=PAD)
                           [:, 2 * j:2 * j + 2, 1:1 + HW, 1:1 + HW])
                    pointwise(mode1, ps[:, 0:N], dst, SC1, SH1, m)

                def emit_j0_khalves():
                    # startup: both m groups in one tile, k-halves
                    # interleaved, so matmuls start on the first half-x DMA
                    ps = pspool.tile([128, 2 * HB], F32, tag="psp",
                                     name="c1ps_j0")
                    for khalf in range(2):
                        for m in range(M1):
                            for k in range(4 * khalf, 4 * khalf + 4):
                                nc.tensor.matmul(
                                    ps[:, m * HB:m * HB + N],
                                    w1ap(k, m), xcol(xt, k, 0),
                                    start=(k == 0), stop=(k == K1 - 1),
                                    skip_group_check=True)
                    for m in range(M1):
                        dst = (h1[m][:]
                               .rearrange("c (n a b) -> c n a b",
                                          a=PAD, b=PAD)
                               [:, 0:2, 1:1 + HW, 1:1 + HW])
                        pointwise(mode1, ps[:, m * HB:m * HB + N],
                                  dst, SC1, SH1, m)

                if npairs == 2 and pi == 0:
                    # j-outer so conv1(j0) never waits on the xj1 DMA (a
                    # long stall would also reset the PE p-state clock)
                    def chunk1():
                        for m in range(M1):
                            emit_single(1, m)
                        emit_vtf_dve_jmajor(h1, vt, npairs)
                    return [emit_j0_khalves, chunk1]
                if npairs == 2:
                    def mk(m):
                        def c():
                            emit_pair(m)
                            emit_vtf(m, h1[m], vt[m], npairs)
                        return c
                    return [mk(m) for m in range(M1)]

                def mk1(m):
                    def c():
                        emit_single(0, m)
                        emit_vtf(m, h1[m], vt[m], npairs)
                    return c
                return [mk1(m) for m in range(M1)]

            def emit_passes():
              plan = [(0, 2), (2, 2), (4, 2), (6, 1), (7, 1)]
              # prefetch: emit pass p+1's x loads at the START of pass p so
              # they sit ahead of pass p's store DMAs in SP queue order
              xt_next = xt0
              h1_next = None
              for pi, (q0, npairs) in enumerate(plan):
                xt = xt_next
                if pi + 1 < len(plan):
                    nq0, nnp = plan[pi + 1]
                    xt_next = [load_xj(nq0 + j, j) for j in range(nnp)]

                if pi == 0:
                    h1, vt = alloc_h1(npairs)
                    for c in conv1_chunks(pi, npairs, xt, h1, vt):
                        c()
                else:
                    h1, vt = h1_next  # conv1 emitted inside pass pi-1

                # h2 per k-plane (conv3's k0 matmuls then don't wait on the
                # k1 plane's drain chain)
                h2l = [h2pool.tile([128, MAXBP * S], BF, tag=f"h2_{k}",
                                   name=f"h2t{k}")
                       for k in range(K2)]

                def h2ap(k, j, nj=1):
                    return h2l[k][:, j * N:(j + nj) * N]

                # conv1 for pass pi+1 is emitted interleaved into this
                # pass's conv3 (software pipelining: PE fills ACT's drain
                # lag with conv1 matmuls whose pointwise runs on DVE)
                if pi + 1 < len(plan):
                    h1_next = alloc_h1(plan[pi + 1][1])
                    next_chunks = conv1_chunks(pi + 1, plan[pi + 1][1],
                                               xt_next, *h1_next)
                else:
                    next_chunks = []

                # conv2: 3x3 pad 1 via 1D-winograd F(2,3) along width:
                # per (pair, m): 4 freq GEMMs (N=196) accumulating over
                # (kh, kt), then the A^T output transform on DVE and the
                # kervolution square on ACT
                nn = 2 * npairs
                vv = [vt[k][:, 0:nn * 16 * 4 * 7]
                      .rearrange("c (n r j t) -> c n r j t", n=nn, r=16, j=4)
                      for k in range(K2)]
                for jp in range(npairs):
                    for m in range(M1):
                        ps = pspool.tile([128, 2 * HB], F32, tag="psp")
                        for j4 in range(4):
                            off = (j4 // 2) * HB + (j4 % 2) * S
                            first = True
                            for kh in range(3):
                                for kt in range(K2):
                                    rhs = vv[kt][:, 2 * jp:2 * jp + 2,
                                                 kh:kh + HW, j4, :]
                                    nc.tensor.matmul(
                                        ps[:, off:off + S],
                                        w2ap(kt, kh, j4, m), rhs,
                                        start=first,
                                        stop=(kh == 2 and kt == K2 - 1))
                                    first = False
                        # output transform: even = M0+M1+M2, odd = M1-M2-M3
                        yv = (h2ap(m, jp)
                              .rearrange("c (n a b) -> c n a b", a=HW, b=HW))

                        def ysel(par):
                            return (yv.rearrange(
                                "c n a (t two) -> c n a t two", two=2)
                                [:, :, :, :, par])

                        def msel(j4):
                            off = (j4 // 2) * HB + (j4 % 2) * S
                            return (ps[:, off:off + S]
                                    .rearrange("c (n a t) -> c n a t",
                                               n=2, a=HW))

                        ye, yo = ysel(0), ysel(1)
                        # stage M1/M2 to SBUF first: a TensorTensor may read
                        # at most one PSUM operand
                        s12 = tpool.tile([128, 2 * S], BF, tag="tq",
                                         name="s12")
                        sv = s12[:].rearrange("c (g n a t) -> c g n a t",
                                              g=2, n=2, a=HW)
                        nc.vector.tensor_scalar(sv[:, 0], msel(1), 1.0,
                                                None, Alu.mult)
                        nc.vector.tensor_scalar(sv[:, 1], msel(2), 1.0,
                                                None, Alu.mult)
                        nc.vector.tensor_tensor(ye, msel(0), sv[:, 0],
                                                Alu.add)
                        nc.vector.tensor_tensor(ye, ye, sv[:, 1], Alu.add)
                        nc.vector.tensor_tensor(yo, sv[:, 0], sv[:, 1],
                                                Alu.subtract)
                        nc.vector.tensor_tensor(yo, yo, msel(3),
                                                Alu.subtract)
                        # kervolution square, in place on the h2 slice
                        ph = h2ap(m, jp)
                        if mode2[0] == SLOW:
                            nc.scalar.activation(ph, ph, Sq, bias=1.0,
                                                 scale=1.0)
                            nc.vector.tensor_scalar(
                                ph, ph, scb[:, SC2 + m:SC2 + m + 1],
                                shb[:, SH2 + m:SH2 + m + 1],
                                Alu.mult, Alu.add)
                        else:
                            nc.scalar.activation(
                                ph, ph, Sq,
                                bias=scb[:, SC2 + M1 + m:SC2 + M1 + m + 1],
                                scale=scb[:, SC2 + m:SC2 + m + 1])
                            if mode2[0] == FAST_T:
                                nc.vector.tensor_scalar(
                                    ph, ph, shb[:, SH2 + m:SH2 + m + 1],
                                    None, Alu.add)

                # conv3: 1x1, C_MID -> C_IN, (2mp, 2mp+1) paired per j when
                # uniform; + residual, store per (m-pair, j)
                pair3 = mode3[1] is not None
                # interleave next-pass conv1 chunks where ring/drain waits
                # would otherwise stall PE: before group 1 (covers the
                # conv2-j1/c2 drain latency) and mid-phase
                if npairs == 2:
                    c1_at = {0: 0, 4: 1}
                else:
                    c1_at = {0: 0, 2: 1}
                gidx = 0
                for mp in range(M3 // 2):
                    for j in range(npairs):
                        if gidx in c1_at and c1_at[gidx] < len(next_chunks):
                            next_chunks[c1_at[gidx]]()
                        gidx += 1
                        ps = pspool.tile([128, 2 * HB], F32, tag="psp")
                        for mi in range(2):
                            m = 2 * mp + mi
                            for k in range(K2):
                                nc.tensor.matmul(
                                    ps[:, mi * HB:mi * HB + N],
                                    w3ap(k, m), h2ap(k, j),
                                    start=(k == 0), stop=(k == K2 - 1))
                        zt = opool.tile([128, 2 * N], BF, tag="z")
                        last_pass = pi == len(plan) - 1
                        if (last_pass and pair3 and mode3[0] == FAST_T0
                                and mp == 1):
                            # tail relief: one group's pointwise on DVE so
                            # the final ACT drain train is shorter
                            tq3 = tpool.tile([128, 2 * N], BF, tag="tq")
                            tv3 = tq3[:].rearrange("c (m b) -> c m b", b=N)
                            src = (ps[:].rearrange("c (m b) -> c m b", m=2)
                                   [:, :, 0:N])
                            nc.vector.tensor_scalar(
                                tv3, src,
                                scb[:, SC3 + 2 * mp:SC3 + 2 * mp + 1],
                                scb[:, SC3 + 2 * mp:SC3 + 2 * mp + 1],
                                Alu.mult, Alu.add)
                            nc.vector.tensor_tensor(zt[:], tq3[:], tq3[:],
                                                    Alu.mult)
                        elif last_pass and mp == M3 // 2 - 1:
                            # final group: single-m drains + residuals (the
                            # chain after the very last matmul halves), but
                            # ONE paired store (two stores would serialize
                            # their HWDGE generations + DGE delays)
                            for mi in range(2):
                                pointwise(mode3, ps[:, mi * HB:mi * HB + N],
                                          zt[:, mi * N:(mi + 1) * N],
                                          SC3, SH3, 2 * mp + mi)
                                zvi = (zt[:, mi * N:(mi + 1) * N]
                                       .rearrange("c (n s) -> c n s", n=2))
                                xvi = (xt[j][:]
                                       .rearrange("p (k n s) -> p k n s",
                                                  k=K1, n=2)
                                       [:, 2 * mp + mi, :, :])
                                nc.vector.tensor_tensor(zvi, zvi, xvi,
                                                        Alu.add)
                            zv = zt[:].rearrange("c (m n s) -> c m n s",
                                                 m=2, n=2)
                            dst = out_cm[:, 2 * mp:2 * mp + 2,
                                         2 * (q0 + j):2 * (q0 + j) + 2, :]
                            nc.sync.dma_start(dst, zv)
                            continue
                        elif pair3:
                            src = (ps[:].rearrange("c (m b) -> c m b", m=2)
                                   [:, :, 0:N])
                            pointwise(mode3, src, zt[:], SC3, SH3, 2 * mp)
                        else:
                            for mi in range(2):
                                pointwise(mode3, ps[:, mi * HB:mi * HB + N],
                                          zt[:, mi * N:(mi + 1) * N],
                                          SC3, SH3, 2 * mp + mi)
                        zv = zt[:].rearrange("c (m n s) -> c m n s",
                                             m=2, n=2)
                        xv = (xt[j][:].rearrange("p (k n s) -> p k n s",
                                                 k=K1, n=2)
                              [:, 2 * mp:2 * mp + 2, :, :])
                        nc.vector.tensor_tensor(zv, zv, xv, Alu.add)
                        dst = out_cm[:, 2 * mp:2 * mp + 2,
                                     2 * (q0 + j):2 * (q0 + j) + 2, :]
                        nc.sync.dma_start(dst, zv)

            if reps is None:
                emit_passes()
            else:
                with tc.For_i(0, reps, 1):
                    emit_passes()

    nc.compile()
    return nc


# ---------------- host side ----------------

_CACHE = {}


def _get_runner(modes):
    if modes in _CACHE:
        return _CACHE[modes]
    import jax
    from jax.experimental.shard_map import shard_map
    from jax.sharding import Mesh, PartitionSpec
    from concourse.bass2jax import (_bass_exec_p, install_neuronx_cc_hook,
                                    partition_id_tensor)

    nc = _build(modes)
    install_neuronx_cc_hook()
    partition_name = nc.partition_id_tensor.name if nc.partition_id_tensor else None
    in_names, out_names, out_avals = [], [], []
    for alloc in nc.m.functions[0].allocations:
        if not isinstance(alloc, mybir.MemoryLocationSet):
            continue
        name = alloc.memorylocations[0].name
        if alloc.kind == "ExternalInput":
            if name != partition_name:
                in_names.append(name)
        elif alloc.kind == "ExternalOutput":
            out_names.append(name)
            out_avals.append(jax.core.ShapedArray(
                tuple(alloc.tensor_shape), mybir.dt.np(alloc.dtype)))
    n_params, n_outs = len(in_names), len(out_avals)
    all_in_names = list(in_names) + list(out_names)
    if partition_name is not None:
        all_in_names.append(partition_name)

    def _body(*args):
        operands = list(args)
        if partition_name is not None:
            operands.append(partition_id_tensor())
        outs = _bass_exec_p.bind(
            *operands,
            out_avals=tuple(out_avals),
            in_names=tuple(all_in_names),
            out_names=tuple(out_names),
            lowering_input_output_aliases=(),
            sim_require_finite=True,
            sim_require_nnan=True,
            nc=nc,
        )
        return tuple(outs)

    devices = jax.devices()[:8]
    mesh = Mesh(np.asarray(devices), ("core",))
    sharded = jax.jit(
        shard_map(_body, mesh=mesh,
                  in_specs=(PartitionSpec("core"),) * (n_params + n_outs),
                  out_specs=(PartitionSpec("core"),) * n_outs,
                  check_rep=False),
        donate_argnums=tuple(range(n_params, n_params + n_outs)),
        keep_unused=True,
    )
    sharding = jax.sharding.NamedSharding(mesh, PartitionSpec("core"))
    runner = dict(nc=nc, sharded=sharded, sharding=sharding, jax=jax,
                  in_names=in_names, out_names=out_names, out_avals=out_avals)
    _CACHE[modes] = runner
    return runner


def _vec_tile(v, m_tiles):
    """[C] -> [128, m_tiles] column-per-m-tile layout."""
    return np.ascontiguousarray(np.asarray(v).reshape(m_tiles, 128).T
                                .astype(np.float32))


def _bf16(a):
    import ml_dtypes
    return np.asarray(a, dtype=np.float32).astype(ml_dtypes.bfloat16)


def prepare(w1, w2, w3, g1, b1, m1, v1, g2, b2, m2, v2, g3, b3, m3, v3):
    """Host prep: returns (modes, shared_input_dict_without_x)."""
    s1 = g1 / np.sqrt(v1 + EPS)
    t1 = b1 - m1 * s1
    s2 = g2 / np.sqrt(v2 + EPS)
    t2 = b2 - m2 * s2
    s3 = g3 / np.sqrt(v3 + EPS)
    t3 = b3 - m3 * s3

    def mode_of(s, t):
        """Returns (mode, const): const = sqrt(s) as a float when s is
        exactly channel-uniform and shifts are zero (enables m-paired
        PSUM drains on ACT), else None."""
        if np.all(s > 0):
            if not np.any(t):
                r = np.sqrt(s)
                const = float(r[0]) if np.all(s == s[0]) else None
                return (FAST_T0, const)
            return (FAST_T, None)
        return (SLOW, None)

    modes = (mode_of(s1, t1), mode_of(s2, t2), mode_of(s3, t3))

    def sc_bi(lmode, s, m_tiles):
        mode = lmode[0]
        if mode == SLOW:
            return _vec_tile(s, m_tiles), np.ones((128, m_tiles), np.float32)
        r = np.sqrt(s)
        return _vec_tile(r, m_tiles), _vec_tile(r, m_tiles)

    sc1, bi1 = sc_bi(modes[0], s1, M1)
    sc2, bi2 = sc_bi(modes[1], s2, M1)
    sc3, bi3 = sc_bi(modes[2], s3, M3)
    scb = np.concatenate([sc1, bi1, sc2, bi2, sc3, bi3], axis=1)
    shb = np.concatenate([_vec_tile(t1, M1), _vec_tile(t2, M1),
                          _vec_tile(t3, M3)], axis=1)

    w1t = _bf16(np.ascontiguousarray(w1[:, :, 0, 0].T))          # [1024,256]
    # w2: [o, i, kh, kw] -> 1D-winograd F(2,3) along kw: U_j = G @ w[kw]
    G = np.array([[1, 0, 0], [.5, .5, .5], [.5, -.5, .5], [0, 0, 1]],
                 np.float64)
    U = np.einsum('jw,oihw->oihj', G, w2.astype(np.float64))  # [o,i,kh,j]
    w2t = _bf16(np.ascontiguousarray(
        U.transpose(1, 2, 3, 0)                   # [i, kh, j, o]
          .reshape(K2, 128, 3, 4, C_MID)          # [kt, i128, kh, j, o]
          .transpose(0, 2, 3, 1, 4)))             # [kt, kh, j, i128, o]
    w3t = _bf16(np.ascontiguousarray(w3[:, :, 0, 0].T))          # [256,1024]

    shared = dict(w1t=w1t, w2t=w2t, w3t=w3t, scb=scb, shb=shb)
    return modes, shared


def kernel(**inputs):
    inputs = {k: np.asarray(v) for k, v in inputs.items()}
    x = inputs.pop("x").astype(np.float32)
    modes, shared = prepare(**inputs)
    r = _get_runner(modes)
    jax = r["jax"]

    n_cores = 8
    # x: [128, 1024, 14, 14] -> [core(8) x k(8), 128, 16, 196] bf16 channel-major
    x_cm = _bf16(x.reshape(8, B, K1, 128, S)
                 .transpose(0, 2, 3, 1, 4)
                 .reshape(n_cores * K1, 128, B, S))
    dev_in = []
    for name in r["in_names"]:
        if name == "x":
            cat = x_cm
        else:
            a = shared[name]
            cat = np.concatenate([a] * n_cores, axis=0)
        dev_in.append(jax.device_put(cat, r["sharding"]))
    zero_outs = [
        jax.device_put(np.zeros((n_cores * av.shape[0], *av.shape[1:]), av.dtype),
                       r["sharding"])
        for av in r["out_avals"]
    ]
    outs = r["sharded"](*dev_in, *zero_outs)
    jax.block_until_ready(outs)
    out = np.asarray(outs[r["out_names"].index("out")])
    # [core x m(8), 128, 16, 196] bf16 -> [128, 1024, 14, 14] f32
    return np.ascontiguousarray(
        out.reshape(n_cores, M3, 128, B, S)
           .transpose(0, 3, 1, 2, 4)
           .reshape(128, C_IN, HW, HW)).astype(np.float32)


# revision 66
# speedup vs baseline: 1.0741x; 1.0051x over previous
"""TRN2 Bass kernel for nn_Block_6476810682806 (dense_cnn).

Bottleneck block: 1x1 kerv -> BN -> 3x3 kerv -> BN -> 1x1 kerv -> BN -> +residual,
where kerv(x) = (conv(x) + 1)^2 and BN is inference-mode (frozen stats).

Distribution: data-parallel over batch (128 -> 16 per core) across 8 cores,
weights replicated. Each core computes its shard fully independently.

Device strategy (per core):
  - everything that crosses DMA is bf16 (halves HBM traffic; norm_rel ~5.6e-3
    vs the 2e-2 gate). Host pre-transposes x/out to channel-major so bf16
    descriptor runs stay >=512B (smaller runs pay a 2x DMA latency penalty).
  - activations channel-major: [C partitions, batch*spatial free]
  - convs as PE matmuls in bf16 (1 cyc/row at any N), f32 PSUM accumulate
  - 3x3 conv: 9 shifted matmuls over a zero-padded per-image 16x16 SBUF plane
  - BN scale folded into the kervolution square on ACT:
        s*(y+1)^2 = (sqrt(s)*y + sqrt(s))^2  (requires s > 0)
    shifts (t = b - m*s) are zero for this problem's fills; generic paths
    emit an extra per-channel add / affine when they are not.
  - residual add on DVE, straight from the resident x tiles (all-bf16 SBUF
    operands hit the DVE 2x/4x fast modes)
  - pass plan 4+4+4+2+2 images: small final passes shrink the tail drain
    (ACT pointwise + DVE residual + store DMA after the last matmul)
"""

import numpy as np

import concourse.bacc as bacc
import concourse.mybir as mybir
import concourse.tile as tile

F32 = mybir.dt.float32
BF = mybir.dt.bfloat16
EPS = 1e-5

B = 16          # images per core
C_IN = 1024
C_MID = 256
HW = 14
S = HW * HW     # 196
N = 2 * S       # matmul moving size per image pair = 392
PAD = 16        # padded plane side
PS = PAD * PAD  # 256 padded plane size
K1 = C_IN // 128          # 8
K2 = C_MID // 128         # 2
M1 = C_MID // 128         # 2
M3 = C_IN // 128          # 8
MAXBP = 4                 # max images per pass

# layer modes
FAST_T0 = 0   # all s>0, all t==0: ACT-only pointwise
FAST_T = 1    # all s>0, some t!=0: ACT + per-channel add
SLOW = 2      # some s<=0: plain square on ACT + DVE affine

# packed scale/bias column offsets in scb [128, 24]
SC1, BI1, SC2, BI2, SC3, BI3 = 0, 2, 4, 6, 8, 16
# packed shift column offsets in shb [128, 12]
SH1, SH2, SH3 = 0, 2, 4


def _build(modes, reps=None):
    mode1, mode2, mode3 = modes
    nc = bacc.Bacc("TRN2", target_bir_lowering=False, debug=False)

    x_d = nc.dram_tensor("x", [K1, 128, B, S], BF, kind="ExternalInput").ap()
    w1_d = nc.dram_tensor("w1t", [C_IN, C_MID], BF, kind="ExternalInput").ap()
    w2_d = nc.dram_tensor("w2t", [K2, 3, 4, 128, C_MID], BF,
                          kind="ExternalInput").ap()
    w3_d = nc.dram_tensor("w3t", [C_MID, C_IN], BF, kind="ExternalInput").ap()
    scb_d = nc.dram_tensor("scb", [128, 24], F32, kind="ExternalInput").ap()
    shb_d = nc.dram_tensor("shb", [128, 12], F32, kind="ExternalInput").ap()
    out_d = nc.dram_tensor("out", [M3, 128, B, S], BF, kind="ExternalOutput").ap()

    x_cm = x_d.rearrange("k p n s -> p k n s")     # [128, 8, 16, 196]
    out_cm = out_d.rearrange("m p n s -> p m n s")  # [128, 8, 16, 196]

    Sq = mybir.ActivationFunctionType.Square
    Alu = mybir.AluOpType

    with tile.TileContext(nc) as tc:
        with (
            tc.tile_pool(name="wpool", bufs=1) as wpool,
            tc.tile_pool(name="xpool", bufs=4) as xpool,
            tc.tile_pool(name="h1pool", bufs=2) as h1pool,
            tc.tile_pool(name="h2pool", bufs=2) as h2pool,
            tc.tile_pool(name="tpool", bufs=2) as tpool,
            tc.tile_pool(name="vpool", bufs=2) as vpool,
            tc.tile_pool(name="opool", bufs=4) as opool,
            tc.tile_pool(name="psp", bufs=4, space="PSUM") as pspool,
        ):
            # every PSUM tile is 2 banks; accumulation groups go to the
            # bank-aligned halves [0:N] and [HB:HB+N], drained by ONE
            # strided ACT op (halves the per-op init overhead share)
            HB = 512
            def xcol(xh, k, j):
                # [128, 2, S] rhs slice for k-tile k, image pair j
                v = xh[j][:].rearrange("p (k n s) -> p k n s", k=K1, n=2)
                return v[:, k, :, :]

            # ---- startup: one serialized DMA stream (SP queue) in first-use
            # order: xj0, scale vec, w1, w2, xj1, w3 ----
            def load_xj(pair, j):
                # pair: global image-pair index 0..7; j: slot parity in pass
                t = xpool.tile([128, K1 * 2 * S], BF, tag=f"x{j}",
                               name=f"xt_q{pair}")
                nc.sync.dma_start(
                    t[:].rearrange("p (k n s) -> p k n s", k=K1, n=2),
                    x_cm[:, :, 2 * pair:2 * pair + 2, :])
                return t

            # first x pair split into k-halves so conv1's first matmuls can
            # start ~1us sooner (w1 slots between the halves)
            xj0 = xpool.tile([128, K1 * 2 * S], BF, tag="x0", name="xt_q0")
            xj0v = xj0[:].rearrange("p (k n s) -> p k n s", k=K1, n=2)
            nc.sync.dma_start(xj0v[:, 0:K1 // 2], x_cm[:, 0:K1 // 2, 0:2, :])
            w1view = w1_d.rearrange("(k p) o -> p k o", p=128)
            w1s = wpool.tile([128, K1 * C_MID], BF, tag="w1s")
            w1v = w1s[:].rearrange("p (k o) -> p k o", k=K1)
            nc.sync.dma_start(w1v[:, 0:K1 // 2], w1view[:, 0:K1 // 2])
            nc.sync.dma_start(xj0v[:, K1 // 2:], x_cm[:, K1 // 2:, 0:2, :])
            nc.sync.dma_start(w1v[:, K1 // 2:], w1view[:, K1 // 2:])
            scb = wpool.tile([128, 24], F32, tag="scb")
            nc.sync.dma_start(scb[:], scb_d)
            if any(mo[0] != FAST_T0 for mo in modes):
                shb = wpool.tile([128, 12], F32, tag="shb")
                nc.sync.dma_start(shb[:], shb_d)
            else:
                shb = None
            xj1 = load_xj(1, 1)
            xt0 = [xj0, xj1]
            w2view = w2_d.rearrange("k h j p o -> p (k h j) o")
            w2s = wpool.tile([128, 24 * C_MID], BF, tag="w2s")
            w2v = w2s[:].rearrange("p (kt o) -> p kt o", kt=24)
            nc.sync.dma_start(w2v[:], w2view[:])
            w3s = wpool.tile([128, K2 * C_IN], BF, tag="w3s")
            nc.sync.dma_start(
                w3s[:].rearrange("p (k o) -> p k o", k=K2),
                w3_d.rearrange("(k p) o -> p k o", p=128))

            def w1ap(k, m):
                return w1s[:, k * C_MID + m * 128: k * C_MID + (m + 1) * 128]

            def w2ap(kt, kh, j4, m):
                base = ((kt * 3 + kh) * 4 + j4) * C_MID + m * 128
                return w2s[:, base: base + 128]

            def w3ap(k, m):
                return w3s[:, k * C_IN + m * 128: k * C_IN + (m + 1) * 128]

            def pointwise(lmode, src_ap, out_ap, sc_off, sh_off, m):
                """out = s*(src+1)^2 + t, written to out_ap.

                lmode is (mode, const): const is sqrt(s) as a python float
                when s is channel-uniform (allows m-paired drains), else
                None (per-channel scb column; src must be single-m)."""
                mode, const = lmode
                if mode == SLOW:
                    nc.scalar.activation(out_ap, src_ap, Sq, bias=1.0, scale=1.0)
                    nc.vector.tensor_scalar(
                        out_ap, out_ap, scb[:, sc_off + m:sc_off + m + 1],
                        shb[:, sh_off + m:sh_off + m + 1], Alu.mult, Alu.add)
                else:
                    # for m-paired drains (const flag set) the scale is
                    # channel-uniform, so the first m's column is valid for
                    # the whole pair
                    nc.scalar.activation(
                        out_ap, src_ap, Sq,
                        bias=scb[:, sc_off + (M1 if sc_off < SC3 else M3) + m:
                                 sc_off + (M1 if sc_off < SC3 else M3) + m + 1],
                        scale=scb[:, sc_off + m:sc_off + m + 1])
                    if mode == FAST_T:
                        nc.vector.tensor_scalar(
                            out_ap, out_ap, shb[:, sh_off + m:sh_off + m + 1],
                            None, Alu.add)

            def pointwise_dve(src_ap, dst_ap, sc_off, m, nelem):
                """conv1 pointwise on DVE (FAST_T0 only): frees the ACT
                queue for conv3 drains at pass boundaries.
                t = sqrt(s)*y + sqrt(s); dst = t*t."""
                tq = tpool.tile([128, 2 * N], BF, tag="tq")
                tv = tq[:, 0:nelem]
                if nelem > N:
                    tv = tv.rearrange("c (j b) -> c j b", b=N)
                nc.vector.tensor_scalar(
                    tv, src_ap, scb[:, sc_off + m:sc_off + m + 1],
                    scb[:, sc_off + m:sc_off + m + 1], Alu.mult, Alu.add)
                tsq = (tq[:, 0:nelem]
                       .rearrange("c (n a b) -> c n a b", a=HW, b=HW))
                nc.vector.tensor_tensor(dst_ap, tsq, tsq, Alu.mult)

            # ---- PE warmup: dummy matmuls on scratch data keep the PE
            # clock ramping while the startup DMAs land; the early dummy
            # activation pulls the act-table load off the critical path ----
            wu = wpool.tile([128, 128], BF, tag="wu")
            nc.vector.memset(wu[:].bitcast(F32), 0.0)
            wusc = wpool.tile([128, 4], F32, tag="wusc")
            # act-table preload reads SBUF (reading the warmup PSUM tile
            # would WAR-serialize every warmup matmul behind the 1.3us
            # LoadActFuncSet)
            nc.scalar.activation(wusc[:], wu[:].bitcast(F32)[:, 0:4], Sq,
                                 bias=1.0, scale=1.0)
            wups = pspool.tile([128, 2 * HB], F32, tag="psp", name="wups")
            NWU = 120
            for i in range(NWU):
                nc.tensor.matmul(wups[:, 0:64], wu[:], wu[:, 0:64],
                                 start=(i == 0), stop=(i == NWU - 1))

            # ---- main passes: (first image pair index, n pairs) ----
            def alloc_h1(npairs):
                h1, vt = [], []
                for k in range(K2):
                    t = h1pool.tile([128, MAXBP * PS], BF, tag=f"h1_{k}",
                                    name=f"h1t{k}")
                    nc.gpsimd.memset(t[:, 0:2 * npairs * PS].bitcast(F32),
                                     0.0)
                    h1.append(t)
                    v = vpool.tile([128, MAXBP * 16 * 4 * 7], BF,
                                   tag=f"v_{k}", name=f"vt{k}")
                    vt.append(v)
                return h1, vt

            def emit_vtf(k, h1t, vtt, npairs):
                """1D-winograd input transform for one k-plane on GPSIMD:
                V0=d0-d2 V1=d1+d2 V2=d2-d1 V3=d1-d3 over width pairs."""
                nn = 2 * npairs
                hv = (h1t[:, 0:nn * PS]
                      .rearrange("c (n a b) -> c n a b", a=PAD, b=PAD))
                vv = (vtt[:, 0:nn * 16 * 4 * 7]
                      .rearrange("c (n r j t) -> c n r j t", n=nn, r=16, j=4))

                def dsel(c0, par):
                    # pad cols c0+2t+par for t=0..6 (stride-2 column pick)
                    return (hv[:, :, :, c0:c0 + 14]
                            .rearrange("c n r (t two) -> c n r t two", two=2)
                            [:, :, :, :, par])

                d0 = dsel(0, 0)
                d1 = dsel(0, 1)
                d2 = dsel(2, 0)
                d3 = dsel(2, 1)
                eng = nc.gpsimd
                eng.tensor_tensor(vv[:, :, :, 0, :], d0, d2, Alu.subtract)
                eng.tensor_tensor(vv[:, :, :, 1, :], d1, d2, Alu.add)
                eng.tensor_tensor(vv[:, :, :, 2, :], d2, d1, Alu.subtract)
                eng.tensor_tensor(vv[:, :, :, 3, :], d1, d3, Alu.subtract)

            def emit_vtf_dve_jmajor(h1, vt, npairs):
                """Pass-1 variant: transforms on DVE, freq-major across both
                k planes, so conv2's first freq GEMM unblocks after 2 ops."""
                nn = 2 * npairs
                for j4 in range(4):
                    for k in range(K2):
                        hv = (h1[k][:, 0:nn * PS]
                              .rearrange("c (n a b) -> c n a b",
                                         a=PAD, b=PAD))
                        vv = (vt[k][:, 0:nn * 16 * 4 * 7]
                              .rearrange("c (n r j t) -> c n r j t",
                                         n=nn, r=16, j=4))

                        def dsel(c0, par):
                            return (hv[:, :, :, c0:c0 + 14]
                                    .rearrange("c n r (t two) -> c n r t two",
                                               two=2)[:, :, :, :, par])

                        pairs = {0: (dsel(0, 0), dsel(2, 0), Alu.subtract),
                                 1: (dsel(0, 1), dsel(2, 0), Alu.add),
                                 2: (dsel(2, 0), dsel(0, 1), Alu.subtract),
                                 3: (dsel(0, 1), dsel(2, 1), Alu.subtract)}
                        a, b, op = pairs[j4]
                        nc.vector.tensor_tensor(vv[:, :, :, j4, :], a, b, op)

            def conv1_chunks(pi, npairs, xt, h1, vt):
                """Returns a list of emitter callables (2 chunks) for this
                pass's conv1; each chunk is one PSUM tile's worth."""
                def emit_pair(m):
                    # pair (j0,j1) per m: PSUM halves, one drain per m
                    ps = pspool.tile([128, 2 * HB], F32, tag="psp",
                                     name=f"c1ps{m}")
                    for j in range(2):
                        for k in range(K1):
                            nc.tensor.matmul(
                                ps[:, j * HB:j * HB + N],
                                w1ap(k, m), xcol(xt, k, j),
                                start=(k == 0), stop=(k == K1 - 1))
                    src = ps[:].rearrange("c (j b) -> c j b", j=2)[:, :, 0:N]
                    dst = (h1[m][:]
                           .rearrange("c (n a b) -> c n a b", a=PAD, b=PAD)
                           [:, 0:4, 1:1 + HW, 1:1 + HW])
                    pointwise(mode1, src, dst, SC1, SH1, m)

                def emit_single(j, m):
                    ps = pspool.tile([128, 2 * HB], F32, tag="psp",
                                     name=f"c1ps{j}_{m}")
                    for k in range(K1):
                        nc.tensor.matmul(
                            ps[:, 0:N], w1ap(k, m), xcol(xt, k, j),
                            start=(k == 0), stop=(k == K1 - 1))
                    dst = (h1[m][:]
                           .rearrange("c (n a b) -> c n a b", a=PAD, b=PAD)
                           [:, 2 * j:2 * j + 2, 1:1 + HW, 1:1 + HW])
                    pointwise(mode1, ps[:, 0:N], dst, SC1, SH1, m)

                def emit_j0_khalves():
                    # startup: both m groups in one tile, k-halves
                    # interleaved, so matmuls start on the first half-x DMA
                    ps = pspool.tile([128, 2 * HB], F32, tag="psp",
                                     name="c1ps_j0")
                    for khalf in range(2):
                        for m in range(M1):
                            for k in range(4 * khalf, 4 * khalf + 4):
                                nc.tensor.matmul(
                                    ps[:, m * HB:m * HB + N],
                                    w1ap(k, m), xcol(xt, k, 0),
                                    start=(k == 0), stop=(k == K1 - 1),
                                    skip_group_check=True)
                    for m in range(M1):
                        dst = (h1[m][:]
                               .rearrange("c (n a b) -> c n a b",
                                          a=PAD, b=PAD)
                               [:, 0:2, 1:1 + HW, 1:1 + HW])
                        pointwise(mode1, ps[:, m * HB:m * HB + N],
                                  dst, SC1, SH1, m)

                if npairs == 2 and pi == 0:
                    # j-outer so conv1(j0) never waits on the xj1 DMA (a
                    # long stall would also reset the PE p-state clock)
                    def chunk1():
                        for m in range(M1):
                            emit_single(1, m)
                        emit_vtf_dve_jmajor(h1, vt, npairs)
                    return [emit_j0_khalves, chunk1]
                if npairs == 2:
                    def mk(m):
                        def c():
                            emit_pair(m)
                            emit_vtf(m, h1[m], vt[m], npairs)
                        return c
                    return [mk(m) for m in range(M1)]

                def mk1(m):
                    def c():
                        emit_single(0, m)
                        emit_vtf(m, h1[m], vt[m], npairs)
                    return c
                return [mk1(m) for m in range(M1)]

            def emit_passes():
              plan = [(0, 2), (2, 2), (4, 2), (6, 1), (7, 1)]
              # prefetch: emit pass p+1's x loads at the START of pass p so
              # they sit ahead of pass p's store DMAs in SP queue order
              xt_next = xt0
              h1_next = None
              for pi, (q0, npairs) in enumerate(plan):
                xt = xt_next
                if pi + 1 < len(plan):
                    nq0, nnp = plan[pi + 1]
                    xt_next = [load_xj(nq0 + j, j) for j in range(nnp)]

                if pi == 0:
                    h1, vt = alloc_h1(npairs)
                    for c in conv1_chunks(pi, npairs, xt, h1, vt):
                        c()
                else:
                    h1, vt = h1_next  # conv1 emitted inside pass pi-1

                # h2 per k-plane (conv3's k0 matmuls then don't wait on the
                # k1 plane's drain chain)
                h2l = [h2pool.tile([128, MAXBP * S], BF, tag=f"h2_{k}",
                                   name=f"h2t{k}")
                       for k in range(K2)]

                def h2ap(k, j, nj=1):
                    return h2l[k][:, j * N:(j + nj) * N]

                # conv1 for pass pi+1 is emitted interleaved into this
                # pass's conv3 (software pipelining: PE fills ACT's drain
                # lag with conv1 matmuls whose pointwise runs on DVE)
                if pi + 1 < len(plan):
                    h1_next = alloc_h1(plan[pi + 1][1])
                    next_chunks = conv1_chunks(pi + 1, plan[pi + 1][1],
                                               xt_next, *h1_next)
                else:
                    next_chunks = []

                # conv2: 3x3 pad 1 via 1D-winograd F(2,3) along width:
                # per (pair, m): 4 freq GEMMs (N=196) accumulating over
                # (kh, kt), then the A^T output transform on DVE and the
                # kervolution square on ACT
                nn = 2 * npairs
                vv = [vt[k][:, 0:nn * 16 * 4 * 7]
                      .rearrange("c (n r j t) -> c n r j t", n=nn, r=16, j=4)
                      for k in range(K2)]
                for jp in range(npairs):
                    for m in range(M1):
                        ps = pspool.tile([128, 2 * HB], F32, tag="psp")
                        for j4 in range(4):
                            off = (j4 // 2) * HB + (j4 % 2) * S
                            first = True
                            for kh in range(3):
                                for kt in range(K2):
                                    rhs = vv[kt][:, 2 * jp:2 * jp + 2,
                                                 kh:kh + HW, j4, :]
                                    nc.tensor.matmul(
                                        ps[:, off:off + S],
                                        w2ap(kt, kh, j4, m), rhs,
                                        start=first,
                                        stop=(kh == 2 and kt == K2 - 1))
                                    first = False
                        # output transform: even = M0+M1+M2, odd = M1-M2-M3
                        yv = (h2ap(m, jp)
                              .rearrange("c (n a b) -> c n a b", a=HW, b=HW))

                        def ysel(par):
                            return (yv.rearrange(
                                "c n a (t two) -> c n a t two", two=2)
                                [:, :, :, :, par])

                        def msel(j4):
                            off = (j4 // 2) * HB + (j4 % 2) * S
                            return (ps[:, off:off + S]
                                    .rearrange("c (n a t) -> c n a t",
                                               n=2, a=HW))

                        ye, yo = ysel(0), ysel(1)
                        # stage M1 to SBUF via ACT Copy, M2 via DVE (a
                        # TensorTensor may read at most one PSUM operand;
                        # splitting the staging balances the two engines —
                        # copy+square share every act table, so no reload)
                        s12 = tpool.tile([128, 2 * S], BF, tag="tq",
                                         name="s12")
                        sv = s12[:].rearrange("c (g n a t) -> c g n a t",
                                              g=2, n=2, a=HW)
                        Cp = mybir.ActivationFunctionType.Copy
                        nc.scalar.activation(sv[:, 0], msel(1), Cp)
                        nc.vector.tensor_scalar(sv[:, 1], msel(2), 1.0,
                                                None, Alu.mult)
                        nc.vector.tensor_tensor(ye, msel(0), sv[:, 0],
                                                Alu.add)
                        nc.vector.tensor_tensor(yo, sv[:, 0], sv[:, 1],
                                                Alu.subtract)
                        nc.vector.tensor_tensor(yo, yo, msel(3),
                                                Alu.subtract)
                        nc.vector.tensor_tensor(ye, ye, sv[:, 1], Alu.add)
                        # kervolution square, in place on the h2 slice
                        ph = h2ap(m, jp)
                        if mode2[0] == SLOW:
                            nc.scalar.activation(ph, ph, Sq, bias=1.0,
                                                 scale=1.0)
                            nc.vector.tensor_scalar(
                                ph, ph, scb[:, SC2 + m:SC2 + m + 1],
                                shb[:, SH2 + m:SH2 + m + 1],
                                Alu.mult, Alu.add)
                        else:
                            nc.scalar.activation(
                                ph, ph, Sq,
                                bias=scb[:, SC2 + M1 + m:SC2 + M1 + m + 1],
                                scale=scb[:, SC2 + m:SC2 + m + 1])
                            if mode2[0] == FAST_T:
                                nc.vector.tensor_scalar(
                                    ph, ph, shb[:, SH2 + m:SH2 + m + 1],
                                    None, Alu.add)

                # conv3: 1x1, C_MID -> C_IN, (2mp, 2mp+1) paired per j when
                # uniform; + residual, store per (m-pair, j)
                pair3 = mode3[1] is not None
                # interleave next-pass conv1 chunks where ring/drain waits
                # would otherwise stall PE: before group 1 (covers the
                # conv2-j1/c2 drain latency) and mid-phase
                if npairs == 2:
                    c1_at = {0: 0, 4: 1}
                else:
                    c1_at = {0: 0, 2: 1}
                gidx = 0
                for mp in range(M3 // 2):
                    for j in range(npairs):
                        if gidx in c1_at and c1_at[gidx] < len(next_chunks):
                            next_chunks[c1_at[gidx]]()
                        gidx += 1
                        ps = pspool.tile([128, 2 * HB], F32, tag="psp")
                        for mi in range(2):
                            m = 2 * mp + mi
                            for k in range(K2):
                                nc.tensor.matmul(
                                    ps[:, mi * HB:mi * HB + N],
                                    w3ap(k, m), h2ap(k, j),
                                    start=(k == 0), stop=(k == K2 - 1))
                        zt = opool.tile([128, 2 * N], BF, tag="z")
                        last_pass = pi == len(plan) - 1
                        if last_pass and mp == M3 // 2 - 1:
                            # final group: single-m drains + residuals (the
                            # chain after the very last matmul halves), but
                            # ONE paired store (two stores would serialize
                            # their HWDGE generations + DGE delays)
                            for mi in range(2):
                                pointwise(mode3, ps[:, mi * HB:mi * HB + N],
                                          zt[:, mi * N:(mi + 1) * N],
                                          SC3, SH3, 2 * mp + mi)
                                zvi = (zt[:, mi * N:(mi + 1) * N]
                                       .rearrange("c (n s) -> c n s", n=2))
                                xvi = (xt[j][:]
                                       .rearrange("p (k n s) -> p k n s",
                                                  k=K1, n=2)
                                       [:, 2 * mp + mi, :, :])
                                nc.vector.tensor_tensor(zvi, zvi, xvi,
                                                        Alu.add)
                            zv = zt[:].rearrange("c (m n s) -> c m n s",
                                                 m=2, n=2)
                            dst = out_cm[:, 2 * mp:2 * mp + 2,
                                         2 * (q0 + j):2 * (q0 + j) + 2, :]
                            nc.sync.dma_start(dst, zv)
                            continue
                        elif pair3:
                            src = (ps[:].rearrange("c (m b) -> c m b", m=2)
                                   [:, :, 0:N])
                            pointwise(mode3, src, zt[:], SC3, SH3, 2 * mp)
                        else:
                            for mi in range(2):
                                pointwise(mode3, ps[:, mi * HB:mi * HB + N],
                                          zt[:, mi * N:(mi + 1) * N],
                                          SC3, SH3, 2 * mp + mi)
                        zv = zt[:].rearrange("c (m n s) -> c m n s",
                                             m=2, n=2)
                        xv = (xt[j][:].rearrange("p (k n s) -> p k n s",
                                                 k=K1, n=2)
                              [:, 2 * mp:2 * mp + 2, :, :])
                        nc.vector.tensor_tensor(zv, zv, xv, Alu.add)
                        dst = out_cm[:, 2 * mp:2 * mp + 2,
                                     2 * (q0 + j):2 * (q0 + j) + 2, :]
                        nc.sync.dma_start(dst, zv)

            if reps is None:
                emit_passes()
            else:
                with tc.For_i(0, reps, 1):
                    emit_passes()

    nc.compile()
    return nc


# ---------------- host side ----------------

_CACHE = {}


def _get_runner(modes):
    if modes in _CACHE:
        return _CACHE[modes]
    import jax
    from jax.experimental.shard_map import shard_map
    from jax.sharding import Mesh, PartitionSpec
    from concourse.bass2jax import (_bass_exec_p, install_neuronx_cc_hook,
                                    partition_id_tensor)

    nc = _build(modes)
    install_neuronx_cc_hook()
    partition_name = nc.partition_id_tensor.name if nc.partition_id_tensor else None
    in_names, out_names, out_avals = [], [], []
    for alloc in nc.m.functions[0].allocations:
        if not isinstance(alloc, mybir.MemoryLocationSet):
            continue
        name = alloc.memorylocations[0].name
        if alloc.kind == "ExternalInput":
            if name != partition_name:
                in_names.append(name)
        elif alloc.kind == "ExternalOutput":
            out_names.append(name)
            out_avals.append(jax.core.ShapedArray(
                tuple(alloc.tensor_shape), mybir.dt.np(alloc.dtype)))
    n_params, n_outs = len(in_names), len(out_avals)
    all_in_names = list(in_names) + list(out_names)
    if partition_name is not None:
        all_in_names.append(partition_name)

    def _body(*args):
        operands = list(args)
        if partition_name is not None:
            operands.append(partition_id_tensor())
        outs = _bass_exec_p.bind(
            *operands,
            out_avals=tuple(out_avals),
            in_names=tuple(all_in_names),
            out_names=tuple(out_names),
            lowering_input_output_aliases=(),
            sim_require_finite=True,
            sim_require_nnan=True,
            nc=nc,
        )
        return tuple(outs)

    devices = jax.devices()[:8]
    mesh = Mesh(np.asarray(devices), ("core",))
    sharded = jax.jit(
        shard_map(_body, mesh=mesh,
                  in_specs=(PartitionSpec("core"),) * (n_params + n_outs),
                  out_specs=(PartitionSpec("core"),) * n_outs,
                  check_rep=False),
        donate_argnums=tuple(range(n_params, n_params + n_outs)),
        keep_unused=True,
    )
    sharding = jax.sharding.NamedSharding(mesh, PartitionSpec("core"))
    runner = dict(nc=nc, sharded=sharded, sharding=sharding, jax=jax,
                  in_names=in_names, out_names=out_names, out_avals=out_avals)
    _CACHE[modes] = runner
    return runner


def _vec_tile(v, m_tiles):
    """[C] -> [128, m_tiles] column-per-m-tile layout."""
    return np.ascontiguousarray(np.asarray(v).reshape(m_tiles, 128).T
                                .astype(np.float32))


def _bf16(a):
    import ml_dtypes
    return np.asarray(a, dtype=np.float32).astype(ml_dtypes.bfloat16)


def prepare(w1, w2, w3, g1, b1, m1, v1, g2, b2, m2, v2, g3, b3, m3, v3):
    """Host prep: returns (modes, shared_input_dict_without_x)."""
    s1 = g1 / np.sqrt(v1 + EPS)
    t1 = b1 - m1 * s1
    s2 = g2 / np.sqrt(v2 + EPS)
    t2 = b2 - m2 * s2
    s3 = g3 / np.sqrt(v3 + EPS)
    t3 = b3 - m3 * s3

    def mode_of(s, t):
        """Returns (mode, const): const = sqrt(s) as a float when s is
        exactly channel-uniform and shifts are zero (enables m-paired
        PSUM drains on ACT), else None."""
        if np.all(s > 0):
            if not np.any(t):
                r = np.sqrt(s)
                const = float(r[0]) if np.all(s == s[0]) else None
                return (FAST_T0, const)
            return (FAST_T, None)
        return (SLOW, None)

    modes = (mode_of(s1, t1), mode_of(s2, t2), mode_of(s3, t3))

    def sc_bi(lmode, s, m_tiles):
        mode = lmode[0]
        if mode == SLOW:
            return _vec_tile(s, m_tiles), np.ones((128, m_tiles), np.float32)
        r = np.sqrt(s)
        return _vec_tile(r, m_tiles), _vec_tile(r, m_tiles)

    sc1, bi1 = sc_bi(modes[0], s1, M1)
    sc2, bi2 = sc_bi(modes[1], s2, M1)
    sc3, bi3 = sc_bi(modes[2], s3, M3)
    scb = np.concatenate([sc1, bi1, sc2, bi2, sc3, bi3], axis=1)
    shb = np.concatenate([_vec_tile(t1, M1), _vec_tile(t2, M1),
                          _vec_tile(t3, M3)], axis=1)

    w1t = _bf16(np.ascontiguousarray(w1[:, :, 0, 0].T))          # [1024,256]
    # w2: [o, i, kh, kw] -> 1D-winograd F(2,3) along kw: U_j = G @ w[kw]
    G = np.array([[1, 0, 0], [.5, .5, .5], [.5, -.5, .5], [0, 0, 1]],
                 np.float64)
    U = np.einsum('jw,oihw->oihj', G, w2.astype(np.float64))  # [o,i,kh,j]
    w2t = _bf16(np.ascontiguousarray(
        U.transpose(1, 2, 3, 0)                   # [i, kh, j, o]
          .reshape(K2, 128, 3, 4, C_MID)          # [kt, i128, kh, j, o]
          .transpose(0, 2, 3, 1, 4)))             # [kt, kh, j, i128, o]
    w3t = _bf16(np.ascontiguousarray(w3[:, :, 0, 0].T))          # [256,1024]

    shared = dict(w1t=w1t, w2t=w2t, w3t=w3t, scb=scb, shb=shb)
    return modes, shared


def kernel(**inputs):
    inputs = {k: np.asarray(v) for k, v in inputs.items()}
    x = inputs.pop("x").astype(np.float32)
    modes, shared = prepare(**inputs)
    r = _get_runner(modes)
    jax = r["jax"]

    n_cores = 8
    # x: [128, 1024, 14, 14] -> [core(8) x k(8), 128, 16, 196] bf16 channel-major
    x_cm = _bf16(x.reshape(8, B, K1, 128, S)
                 .transpose(0, 2, 3, 1, 4)
                 .reshape(n_cores * K1, 128, B, S))
    dev_in = []
    for name in r["in_names"]:
        if name == "x":
            cat = x_cm
        else:
            a = shared[name]
            cat = np.concatenate([a] * n_cores, axis=0)
        dev_in.append(jax.device_put(cat, r["sharding"]))
    zero_outs = [
        jax.device_put(np.zeros((n_cores * av.shape[0], *av.shape[1:]), av.dtype),
                       r["sharding"])
        for av in r["out_avals"]
    ]
    outs = r["sharded"](*dev_in, *zero_outs)
    jax.block_until_ready(outs)
    out = np.asarray(outs[r["out_names"].index("out")])
    # [core x m(8), 128, 16, 196] bf16 -> [128, 1024, 14, 14] f32
    return np.ascontiguousarray(
        out.reshape(n_cores, M3, 128, B, S)
           .transpose(0, 3, 1, 2, 4)
           .reshape(128, C_IN, HW, HW)).astype(np.float32)


# revision 72
# speedup vs baseline: 1.1001x; 1.0242x over previous
"""TRN2 Bass kernel for nn_Block_6476810682806 (dense_cnn).

Bottleneck block: 1x1 kerv -> BN -> 3x3 kerv -> BN -> 1x1 kerv -> BN -> +residual,
where kerv(x) = (conv(x) + 1)^2 and BN is inference-mode (frozen stats).

Distribution: data-parallel over batch (128 -> 16 per core) across 8 cores,
weights replicated. Each core computes its shard fully independently.

Device strategy (per core):
  - everything that crosses DMA is bf16 (halves HBM traffic; norm_rel ~5.6e-3
    vs the 2e-2 gate). Host pre-transposes x/out to channel-major so bf16
    descriptor runs stay >=512B (smaller runs pay a 2x DMA latency penalty).
  - activations channel-major: [C partitions, batch*spatial free]
  - convs as PE matmuls in bf16 (1 cyc/row at any N), f32 PSUM accumulate
  - 3x3 conv: 9 shifted matmuls over a zero-padded per-image 16x16 SBUF plane
  - BN scale folded into the kervolution square on ACT:
        s*(y+1)^2 = (sqrt(s)*y + sqrt(s))^2  (requires s > 0)
    shifts (t = b - m*s) are zero for this problem's fills; generic paths
    emit an extra per-channel add / affine when they are not.
  - residual add on DVE, straight from the resident x tiles (all-bf16 SBUF
    operands hit the DVE 2x/4x fast modes)
  - pass plan 4+4+4+2+2 images: small final passes shrink the tail drain
    (ACT pointwise + DVE residual + store DMA after the last matmul)
"""

import numpy as np

import concourse.bacc as bacc
import concourse.mybir as mybir
import concourse.tile as tile

F32 = mybir.dt.float32
BF = mybir.dt.bfloat16
EPS = 1e-5

B = 16          # images per core
C_IN = 1024
C_MID = 256
HW = 14
S = HW * HW     # 196
N = 2 * S       # matmul moving size per image pair = 392
PAD = 16        # padded plane side
PS = PAD * PAD  # 256 padded plane size
K1 = C_IN // 128          # 8
K2 = C_MID // 128         # 2
M1 = C_MID // 128         # 2
M3 = C_IN // 128          # 8
MAXBP = 4                 # max images per pass

# layer modes
FAST_T0 = 0   # all s>0, all t==0: ACT-only pointwise
FAST_T = 1    # all s>0, some t!=0: ACT + per-channel add
SLOW = 2      # some s<=0: plain square on ACT + DVE affine

# packed scale/bias column offsets in scb [128, 24]
SC1, BI1, SC2, BI2, SC3, BI3 = 0, 2, 4, 6, 8, 16
# packed shift column offsets in shb [128, 12]
SH1, SH2, SH3 = 0, 2, 4


def _build(modes, reps=None):
    mode1, mode2, mode3 = modes
    nc = bacc.Bacc("TRN2", target_bir_lowering=False, debug=False)

    x_d = nc.dram_tensor("x", [K1, 128, B, S], BF, kind="ExternalInput").ap()
    w1_d = nc.dram_tensor("w1t", [C_IN, C_MID], BF, kind="ExternalInput").ap()
    w2_d = nc.dram_tensor("w2t", [K2, 3, 4, 128, C_MID], BF,
                          kind="ExternalInput").ap()
    w3_d = nc.dram_tensor("w3t", [C_MID, C_IN], BF, kind="ExternalInput").ap()
    scb_d = nc.dram_tensor("scb", [128, 24], F32, kind="ExternalInput").ap()
    shb_d = nc.dram_tensor("shb", [128, 12], F32, kind="ExternalInput").ap()
    out_d = nc.dram_tensor("out", [M3, 128, B, S], BF, kind="ExternalOutput").ap()

    x_cm = x_d.rearrange("k p n s -> p k n s")     # [128, 8, 16, 196]
    out_cm = out_d.rearrange("m p n s -> p m n s")  # [128, 8, 16, 196]

    Sq = mybir.ActivationFunctionType.Square
    Alu = mybir.AluOpType

    with tile.TileContext(nc) as tc:
        with (
            tc.tile_pool(name="wpool", bufs=1) as wpool,
            tc.tile_pool(name="xpool", bufs=4) as xpool,
            tc.tile_pool(name="h1pool", bufs=2) as h1pool,
            tc.tile_pool(name="h2pool", bufs=2) as h2pool,
            tc.tile_pool(name="tpool", bufs=2) as tpool,
            tc.tile_pool(name="vpool", bufs=2) as vpool,
            tc.tile_pool(name="opool", bufs=4) as opool,
            tc.tile_pool(name="psp", bufs=4, space="PSUM") as pspool,
        ):
            # every PSUM tile is 2 banks; accumulation groups go to the
            # bank-aligned halves [0:N] and [HB:HB+N], drained by ONE
            # strided ACT op (halves the per-op init overhead share)
            HB = 512
            def xcol(xh, k, j):
                # [128, 2, S] rhs slice for k-tile k, image pair j
                v = xh[j][:].rearrange("p (k n s) -> p k n s", k=K1, n=2)
                return v[:, k, :, :]

            # ---- startup: one serialized DMA stream (SP queue) in first-use
            # order: xj0, scale vec, w1, w2, xj1, w3 ----
            def load_xj(pair, j):
                # pair: global image-pair index 0..7; j: slot parity in pass
                t = xpool.tile([128, K1 * 2 * S], BF, tag=f"x{j}",
                               name=f"xt_q{pair}")
                nc.sync.dma_start(
                    t[:].rearrange("p (k n s) -> p k n s", k=K1, n=2),
                    x_cm[:, :, 2 * pair:2 * pair + 2, :])
                return t

            # first x pair split into k-halves so conv1's first matmuls can
            # start ~1us sooner (w1 slots between the halves)
            xj0 = xpool.tile([128, K1 * 2 * S], BF, tag="x0", name="xt_q0")
            xj0v = xj0[:].rearrange("p (k n s) -> p k n s", k=K1, n=2)
            nc.sync.dma_start(xj0v[:, 0:K1 // 2], x_cm[:, 0:K1 // 2, 0:2, :])
            w1view = w1_d.rearrange("(k p) o -> p k o", p=128)
            w1s = wpool.tile([128, K1 * C_MID], BF, tag="w1s")
            w1v = w1s[:].rearrange("p (k o) -> p k o", k=K1)
            nc.sync.dma_start(w1v[:, 0:K1 // 2], w1view[:, 0:K1 // 2])
            nc.sync.dma_start(xj0v[:, K1 // 2:], x_cm[:, K1 // 2:, 0:2, :])
            nc.sync.dma_start(w1v[:, K1 // 2:], w1view[:, K1 // 2:])
            scb = wpool.tile([128, 24], F32, tag="scb")
            nc.sync.dma_start(scb[:], scb_d)
            if any(mo[0] != FAST_T0 for mo in modes):
                shb = wpool.tile([128, 12], F32, tag="shb")
                nc.sync.dma_start(shb[:], shb_d)
            else:
                shb = None
            xj1 = load_xj(1, 1)
            xt0 = [xj0, xj1]
            w2view = w2_d.rearrange("k h j p o -> p (k h j) o")
            w2s = wpool.tile([128, 24 * C_MID], BF, tag="w2s")
            w2v = w2s[:].rearrange("p (kt o) -> p kt o", kt=24)
            nc.sync.dma_start(w2v[:], w2view[:])
            w3s = wpool.tile([128, K2 * C_IN], BF, tag="w3s")
            nc.sync.dma_start(
                w3s[:].rearrange("p (k o) -> p k o", k=K2),
                w3_d.rearrange("(k p) o -> p k o", p=128))

            def w1ap(k, m):
                return w1s[:, k * C_MID + m * 128: k * C_MID + (m + 1) * 128]

            def w2ap(kt, kh, j4, m):
                base = ((kt * 3 + kh) * 4 + j4) * C_MID + m * 128
                return w2s[:, base: base + 128]

            def w3ap(k, m):
                return w3s[:, k * C_IN + m * 128: k * C_IN + (m + 1) * 128]

            def pointwise(lmode, src_ap, out_ap, sc_off, sh_off, m):
                """out = s*(src+1)^2 + t, written to out_ap.

                lmode is (mode, const): const is sqrt(s) as a python float
                when s is channel-uniform (allows m-paired drains), else
                None (per-channel scb column; src must be single-m)."""
                mode, const = lmode
                if mode == SLOW:
                    nc.scalar.activation(out_ap, src_ap, Sq, bias=1.0, scale=1.0)
                    nc.vector.tensor_scalar(
                        out_ap, out_ap, scb[:, sc_off + m:sc_off + m + 1],
                        shb[:, sh_off + m:sh_off + m + 1], Alu.mult, Alu.add)
                else:
                    # for m-paired drains (const flag set) the scale is
                    # channel-uniform, so the first m's column is valid for
                    # the whole pair
                    nc.scalar.activation(
                        out_ap, src_ap, Sq,
                        bias=scb[:, sc_off + (M1 if sc_off < SC3 else M3) + m:
                                 sc_off + (M1 if sc_off < SC3 else M3) + m + 1],
                        scale=scb[:, sc_off + m:sc_off + m + 1])
                    if mode == FAST_T:
                        nc.vector.tensor_scalar(
                            out_ap, out_ap, shb[:, sh_off + m:sh_off + m + 1],
                            None, Alu.add)

            def pointwise_dve(src_ap, dst_ap, sc_off, m, nelem):
                """conv1 pointwise on DVE (FAST_T0 only): frees the ACT
                queue for conv3 drains at pass boundaries.
                t = sqrt(s)*y + sqrt(s); dst = t*t."""
                tq = tpool.tile([128, 2 * N], BF, tag="tq")
                tv = tq[:, 0:nelem]
                if nelem > N:
                    tv = tv.rearrange("c (j b) -> c j b", b=N)
                nc.vector.tensor_scalar(
                    tv, src_ap, scb[:, sc_off + m:sc_off + m + 1],
                    scb[:, sc_off + m:sc_off + m + 1], Alu.mult, Alu.add)
                tsq = (tq[:, 0:nelem]
                       .rearrange("c (n a b) -> c n a b", a=HW, b=HW))
                nc.vector.tensor_tensor(dst_ap, tsq, tsq, Alu.mult)

            # ---- PE warmup: dummy matmuls on scratch data keep the PE
            # clock ramping while the startup DMAs land; the early dummy
            # activation pulls the act-table load off the critical path ----
            wu = wpool.tile([128, 128], BF, tag="wu")
            nc.vector.memset(wu[:].bitcast(F32), 0.0)
            wusc = wpool.tile([128, 4], F32, tag="wusc")
            # act-table preload reads SBUF (reading the warmup PSUM tile
            # would WAR-serialize every warmup matmul behind the 1.3us
            # LoadActFuncSet)
            nc.scalar.activation(wusc[:], wu[:].bitcast(F32)[:, 0:4], Sq,
                                 bias=1.0, scale=1.0)
            wups = pspool.tile([128, 2 * HB], F32, tag="psp", name="wups")
            NWU = 120
            for i in range(NWU):
                nc.tensor.matmul(wups[:, 0:64], wu[:], wu[:, 0:64],
                                 start=(i == 0), stop=(i == NWU - 1))

            # ---- main passes: (first image pair index, n pairs) ----
            def alloc_h1(npairs):
                h1, vt = [], []
                for k in range(K2):
                    t = h1pool.tile([128, MAXBP * PS], BF, tag=f"h1_{k}",
                                    name=f"h1t{k}")
                    nc.gpsimd.memset(t[:, 0:2 * npairs * PS].bitcast(F32),
                                     0.0)
                    h1.append(t)
                    v = vpool.tile([128, MAXBP * 16 * 4 * 7], BF,
                                   tag=f"v_{k}", name=f"vt{k}")
                    vt.append(v)
                return h1, vt

            def emit_vtf(k, h1t, vtt, npairs):
                """1D-winograd input transform for one k-plane on GPSIMD:
                V0=d0-d2 V1=d1+d2 V2=d2-d1 V3=d1-d3 over width pairs."""
                nn = 2 * npairs
                hv = (h1t[:, 0:nn * PS]
                      .rearrange("c (n a b) -> c n a b", a=PAD, b=PAD))
                vv = (vtt[:, 0:nn * 16 * 4 * 7]
                      .rearrange("c (n r j t) -> c n r j t", n=nn, r=16, j=4))

                def dsel(c0, par):
                    # pad cols c0+2t+par for t=0..6 (stride-2 column pick)
                    return (hv[:, :, :, c0:c0 + 14]
                            .rearrange("c n r (t two) -> c n r t two", two=2)
                            [:, :, :, :, par])

                d0 = dsel(0, 0)
                d1 = dsel(0, 1)
                d2 = dsel(2, 0)
                d3 = dsel(2, 1)
                eng = nc.gpsimd
                eng.tensor_tensor(vv[:, :, :, 0, :], d0, d2, Alu.subtract)
                eng.tensor_tensor(vv[:, :, :, 1, :], d1, d2, Alu.add)
                eng.tensor_tensor(vv[:, :, :, 2, :], d2, d1, Alu.subtract)
                eng.tensor_tensor(vv[:, :, :, 3, :], d1, d3, Alu.subtract)

            def emit_vtf_dve_jmajor(h1, vt, npairs):
                """Pass-1 variant: transforms on DVE, freq-major across both
                k planes, so conv2's first freq GEMM unblocks after 2 ops."""
                nn = 2 * npairs
                for j4 in range(4):
                    for k in range(K2):
                        hv = (h1[k][:, 0:nn * PS]
                              .rearrange("c (n a b) -> c n a b",
                                         a=PAD, b=PAD))
                        vv = (vt[k][:, 0:nn * 16 * 4 * 7]
                              .rearrange("c (n r j t) -> c n r j t",
                                         n=nn, r=16, j=4))

                        def dsel(c0, par):
                            return (hv[:, :, :, c0:c0 + 14]
                                    .rearrange("c n r (t two) -> c n r t two",
                                               two=2)[:, :, :, :, par])

                        pairs = {0: (dsel(0, 0), dsel(2, 0), Alu.subtract),
                                 1: (dsel(0, 1), dsel(2, 0), Alu.add),
                                 2: (dsel(2, 0), dsel(0, 1), Alu.subtract),
                                 3: (dsel(0, 1), dsel(2, 1), Alu.subtract)}
                        a, b, op = pairs[j4]
                        nc.vector.tensor_tensor(vv[:, :, :, j4, :], a, b, op)

            def conv1_chunks(pi, npairs, xt, h1, vt):
                """Returns a list of emitter callables (2 chunks) for this
                pass's conv1; each chunk is one PSUM tile's worth."""
                def emit_pair(m):
                    # pair (j0,j1) per m: PSUM halves, one drain per m
                    ps = pspool.tile([128, 2 * HB], F32, tag="psp",
                                     name=f"c1ps{m}")
                    for j in range(2):
                        for k in range(K1):
                            nc.tensor.matmul(
                                ps[:, j * HB:j * HB + N],
                                w1ap(k, m), xcol(xt, k, j),
                                start=(k == 0), stop=(k == K1 - 1))
                    src = ps[:].rearrange("c (j b) -> c j b", j=2)[:, :, 0:N]
                    dst = (h1[m][:]
                           .rearrange("c (n a b) -> c n a b", a=PAD, b=PAD)
                           [:, 0:4, 1:1 + HW, 1:1 + HW])
                    pointwise(mode1, src, dst, SC1, SH1, m)

                def emit_single(j, m):
                    ps = pspool.tile([128, 2 * HB], F32, tag="psp",
                                     name=f"c1ps{j}_{m}")
                    for k in range(K1):
                        nc.tensor.matmul(
                            ps[:, 0:N], w1ap(k, m), xcol(xt, k, j),
                            start=(k == 0), stop=(k == K1 - 1))
                    dst = (h1[m][:]
                           .rearrange("c (n a b) -> c n a b", a=PAD, b=PAD)
                           [:, 2 * j:2 * j + 2, 1:1 + HW, 1:1 + HW])
                    pointwise(mode1, ps[:, 0:N], dst, SC1, SH1, m)

                def emit_j0_khalves():
                    # startup: both m groups in one tile, k-halves
                    # interleaved, so matmuls start on the first half-x DMA
                    ps = pspool.tile([128, 2 * HB], F32, tag="psp",
                                     name="c1ps_j0")
                    for khalf in range(2):
                        for m in range(M1):
                            for k in range(4 * khalf, 4 * khalf + 4):
                                nc.tensor.matmul(
                                    ps[:, m * HB:m * HB + N],
                                    w1ap(k, m), xcol(xt, k, 0),
                                    start=(k == 0), stop=(k == K1 - 1),
                                    skip_group_check=True)
                    for m in range(M1):
                        dst = (h1[m][:]
                               .rearrange("c (n a b) -> c n a b",
                                          a=PAD, b=PAD)
                               [:, 0:2, 1:1 + HW, 1:1 + HW])
                        pointwise(mode1, ps[:, m * HB:m * HB + N],
                                  dst, SC1, SH1, m)

                if npairs == 2 and pi == 0:
                    # j-outer so conv1(j0) never waits on the xj1 DMA (a
                    # long stall would also reset the PE p-state clock)
                    def chunk1():
                        for m in range(M1):
                            emit_single(1, m)
                        emit_vtf_dve_jmajor(h1, vt, npairs)
                    return [emit_j0_khalves, chunk1]
                if npairs == 2:
                    def mk(m):
                        def c():
                            emit_pair(m)
                            emit_vtf(m, h1[m], vt[m], npairs)
                        return c
                    return [mk(m) for m in range(M1)]

                def mk1(m):
                    def c():
                        emit_single(0, m)
                        emit_vtf(m, h1[m], vt[m], npairs)
                    return c
                return [mk1(m) for m in range(M1)]

            def emit_passes():
              plan = [(0, 2), (2, 2), (4, 2), (6, 2)]
              # prefetch: emit pass p+1's x loads at the START of pass p so
              # they sit ahead of pass p's store DMAs in SP queue order
              xt_next = xt0
              h1_next = None
              for pi, (q0, npairs) in enumerate(plan):
                xt = xt_next
                if pi + 1 < len(plan):
                    nq0, nnp = plan[pi + 1]
                    xt_next = [load_xj(nq0 + j, j) for j in range(nnp)]

                if pi == 0:
                    h1, vt = alloc_h1(npairs)
                    for c in conv1_chunks(pi, npairs, xt, h1, vt):
                        c()
                else:
                    h1, vt = h1_next  # conv1 emitted inside pass pi-1

                # h2 per k-plane (conv3's k0 matmuls then don't wait on the
                # k1 plane's drain chain)
                h2l = [h2pool.tile([128, MAXBP * S], BF, tag=f"h2_{k}",
                                   name=f"h2t{k}")
                       for k in range(K2)]

                def h2ap(k, j, nj=1):
                    return h2l[k][:, j * N:(j + nj) * N]

                # conv1 for pass pi+1 is emitted interleaved into this
                # pass's conv3 (software pipelining: PE fills ACT's drain
                # lag with conv1 matmuls whose pointwise runs on DVE)
                if pi + 1 < len(plan):
                    h1_next = alloc_h1(plan[pi + 1][1])
                    next_chunks = conv1_chunks(pi + 1, plan[pi + 1][1],
                                               xt_next, *h1_next)
                else:
                    next_chunks = []

                # conv2: 3x3 pad 1 via 1D-winograd F(2,3) along width:
                # per (pair, m): 4 freq GEMMs (N=196) accumulating over
                # (kh, kt), then the A^T output transform on DVE and the
                # kervolution square on ACT
                nn = 2 * npairs
                vv = [vt[k][:, 0:nn * 16 * 4 * 7]
                      .rearrange("c (n r j t) -> c n r j t", n=nn, r=16, j=4)
                      for k in range(K2)]
                for jp in range(npairs):
                    for m in range(M1):
                        ps = pspool.tile([128, 2 * HB], F32, tag="psp")
                        for j4 in range(4):
                            off = (j4 // 2) * HB + (j4 % 2) * S
                            first = True
                            for kh in range(3):
                                for kt in range(K2):
                                    rhs = vv[kt][:, 2 * jp:2 * jp + 2,
                                                 kh:kh + HW, j4, :]
                                    nc.tensor.matmul(
                                        ps[:, off:off + S],
                                        w2ap(kt, kh, j4, m), rhs,
                                        start=first,
                                        stop=(kh == 2 and kt == K2 - 1))
                                    first = False
                        # output transform: even = M0+M1+M2, odd = M1-M2-M3
                        yv = (h2ap(m, jp)
                              .rearrange("c (n a b) -> c n a b", a=HW, b=HW))

                        def ysel(par):
                            return (yv.rearrange(
                                "c n a (t two) -> c n a t two", two=2)
                                [:, :, :, :, par])

                        def msel(j4):
                            off = (j4 // 2) * HB + (j4 % 2) * S
                            return (ps[:, off:off + S]
                                    .rearrange("c (n a t) -> c n a t",
                                               n=2, a=HW))

                        ye, yo = ysel(0), ysel(1)
                        # stage M1 to SBUF via ACT Copy, M2 via DVE (a
                        # TensorTensor may read at most one PSUM operand;
                        # splitting the staging balances the two engines —
                        # copy+square share every act table, so no reload)
                        s12 = tpool.tile([128, 2 * S], BF, tag="tq",
                                         name="s12")
                        sv = s12[:].rearrange("c (g n a t) -> c g n a t",
                                              g=2, n=2, a=HW)
                        Cp = mybir.ActivationFunctionType.Copy
                        nc.scalar.activation(sv[:, 0], msel(1), Cp)
                        nc.vector.tensor_scalar(sv[:, 1], msel(2), 1.0,
                                                None, Alu.mult)
                        nc.vector.tensor_tensor(ye, msel(0), sv[:, 0],
                                                Alu.add)
                        nc.vector.tensor_tensor(yo, sv[:, 0], sv[:, 1],
                                                Alu.subtract)
                        nc.vector.tensor_tensor(yo, yo, msel(3),
                                                Alu.subtract)
                        nc.vector.tensor_tensor(ye, ye, sv[:, 1], Alu.add)
                        # kervolution square, in place on the h2 slice
                        ph = h2ap(m, jp)
                        if mode2[0] == SLOW:
                            nc.scalar.activation(ph, ph, Sq, bias=1.0,
                                                 scale=1.0)
                            nc.vector.tensor_scalar(
                                ph, ph, scb[:, SC2 + m:SC2 + m + 1],
                                shb[:, SH2 + m:SH2 + m + 1],
                                Alu.mult, Alu.add)
                        else:
                            nc.scalar.activation(
                                ph, ph, Sq,
                                bias=scb[:, SC2 + M1 + m:SC2 + M1 + m + 1],
                                scale=scb[:, SC2 + m:SC2 + m + 1])
                            if mode2[0] == FAST_T:
                                nc.vector.tensor_scalar(
                                    ph, ph, shb[:, SH2 + m:SH2 + m + 1],
                                    None, Alu.add)

                # conv3: 1x1, C_MID -> C_IN, (2mp, 2mp+1) paired per j when
                # uniform; + residual, store per (m-pair, j)
                pair3 = mode3[1] is not None
                # interleave next-pass conv1 chunks where ring/drain waits
                # would otherwise stall PE: before group 1 (covers the
                # conv2-j1/c2 drain latency) and mid-phase
                if npairs == 2:
                    c1_at = {0: 0, 4: 1}
                else:
                    c1_at = {0: 0, 2: 1}
                gidx = 0
                if pi == len(plan) - 1 and npairs == 2:
                    # last pass j-outer: j0's whole drain/store chain
                    # completes under j1's matmuls, halving the tail
                    order = [(mp, j) for j in range(npairs)
                             for mp in range(M3 // 2)]
                else:
                    order = [(mp, j) for mp in range(M3 // 2)
                             for j in range(npairs)]
                for mp, j in order:
                    if True:
                        if gidx in c1_at and c1_at[gidx] < len(next_chunks):
                            next_chunks[c1_at[gidx]]()
                        gidx += 1
                        ps = pspool.tile([128, 2 * HB], F32, tag="psp")
                        for mi in range(2):
                            m = 2 * mp + mi
                            for k in range(K2):
                                nc.tensor.matmul(
                                    ps[:, mi * HB:mi * HB + N],
                                    w3ap(k, m), h2ap(k, j),
                                    start=(k == 0), stop=(k == K2 - 1))
                        zt = opool.tile([128, 2 * N], BF, tag="z")
                        last_pass = pi == len(plan) - 1
                        if (last_pass and mp == M3 // 2 - 1
                                and j == npairs - 1):
                            # final group: single-m drains + residuals (the
                            # chain after the very last matmul halves), but
                            # ONE paired store (two stores would serialize
                            # their HWDGE generations + DGE delays)
                            for mi in range(2):
                                pointwise(mode3, ps[:, mi * HB:mi * HB + N],
                                          zt[:, mi * N:(mi + 1) * N],
                                          SC3, SH3, 2 * mp + mi)
                                zvi = (zt[:, mi * N:(mi + 1) * N]
                                       .rearrange("c (n s) -> c n s", n=2))
                                xvi = (xt[j][:]
                                       .rearrange("p (k n s) -> p k n s",
                                                  k=K1, n=2)
                                       [:, 2 * mp + mi, :, :])
                                nc.vector.tensor_tensor(zvi, zvi, xvi,
                                                        Alu.add)
                            zv = zt[:].rearrange("c (m n s) -> c m n s",
                                                 m=2, n=2)
                            dst = out_cm[:, 2 * mp:2 * mp + 2,
                                         2 * (q0 + j):2 * (q0 + j) + 2, :]
                            nc.sync.dma_start(dst, zv)
                            continue
                        elif pair3:
                            src = (ps[:].rearrange("c (m b) -> c m b", m=2)
                                   [:, :, 0:N])
                            pointwise(mode3, src, zt[:], SC3, SH3, 2 * mp)
                        else:
                            for mi in range(2):
                                pointwise(mode3, ps[:, mi * HB:mi * HB + N],
                                          zt[:, mi * N:(mi + 1) * N],
                                          SC3, SH3, 2 * mp + mi)
                        zv = zt[:].rearrange("c (m n s) -> c m n s",
                                             m=2, n=2)
                        xv = (xt[j][:].rearrange("p (k n s) -> p k n s",
                                                 k=K1, n=2)
                              [:, 2 * mp:2 * mp + 2, :, :])
                        nc.vector.tensor_tensor(zv, zv, xv, Alu.add)
                        dst = out_cm[:, 2 * mp:2 * mp + 2,
                                     2 * (q0 + j):2 * (q0 + j) + 2, :]
                        nc.sync.dma_start(dst, zv)

            if reps is None:
                emit_passes()
            else:
                with tc.For_i(0, reps, 1):
                    emit_passes()

    nc.compile()
    return nc


# ---------------- host side ----------------

_CACHE = {}


def _get_runner(modes):
    if modes in _CACHE:
        return _CACHE[modes]
    import jax
    from jax.experimental.shard_map import shard_map
    from jax.sharding import Mesh, PartitionSpec
    from concourse.bass2jax import (_bass_exec_p, install_neuronx_cc_hook,
                                    partition_id_tensor)

    nc = _build(modes)
    install_neuronx_cc_hook()
    partition_name = nc.partition_id_tensor.name if nc.partition_id_tensor else None
    in_names, out_names, out_avals = [], [], []
    for alloc in nc.m.functions[0].allocations:
        if not isinstance(alloc, mybir.MemoryLocationSet):
            continue
        name = alloc.memorylocations[0].name
        if alloc.kind == "ExternalInput":
            if name != partition_name:
                in_names.append(name)
        elif alloc.kind == "ExternalOutput":
            out_names.append(name)
            out_avals.append(jax.core.ShapedArray(
                tuple(alloc.tensor_shape), mybir.dt.np(alloc.dtype)))
    n_params, n_outs = len(in_names), len(out_avals)
    all_in_names = list(in_names) + list(out_names)
    if partition_name is not None:
        all_in_names.append(partition_name)

    def _body(*args):
        operands = list(args)
        if partition_name is not None:
            operands.append(partition_id_tensor())
        outs = _bass_exec_p.bind(
            *operands,
            out_avals=tuple(out_avals),
            in_names=tuple(all_in_names),
            out_names=tuple(out_names),
            lowering_input_output_aliases=(),
            sim_require_finite=True,
            sim_require_nnan=True,
            nc=nc,
        )
        return tuple(outs)

    devices = jax.devices()[:8]
    mesh = Mesh(np.asarray(devices), ("core",))
    sharded = jax.jit(
        shard_map(_body, mesh=mesh,
                  in_specs=(PartitionSpec("core"),) * (n_params + n_outs),
                  out_specs=(PartitionSpec("core"),) * n_outs,
                  check_rep=False),
        donate_argnums=tuple(range(n_params, n_params + n_outs)),
        keep_unused=True,
    )
    sharding = jax.sharding.NamedSharding(mesh, PartitionSpec("core"))
    runner = dict(nc=nc, sharded=sharded, sharding=sharding, jax=jax,
                  in_names=in_names, out_names=out_names, out_avals=out_avals)
    _CACHE[modes] = runner
    return runner


def _vec_tile(v, m_tiles):
    """[C] -> [128, m_tiles] column-per-m-tile layout."""
    return np.ascontiguousarray(np.asarray(v).reshape(m_tiles, 128).T
                                .astype(np.float32))


def _bf16(a):
    import ml_dtypes
    return np.asarray(a, dtype=np.float32).astype(ml_dtypes.bfloat16)


def prepare(w1, w2, w3, g1, b1, m1, v1, g2, b2, m2, v2, g3, b3, m3, v3):
    """Host prep: returns (modes, shared_input_dict_without_x)."""
    s1 = g1 / np.sqrt(v1 + EPS)
    t1 = b1 - m1 * s1
    s2 = g2 / np.sqrt(v2 + EPS)
    t2 = b2 - m2 * s2
    s3 = g3 / np.sqrt(v3 + EPS)
    t3 = b3 - m3 * s3

    def mode_of(s, t):
        """Returns (mode, const): const = sqrt(s) as a float when s is
        exactly channel-uniform and shifts are zero (enables m-paired
        PSUM drains on ACT), else None."""
        if np.all(s > 0):
            if not np.any(t):
                r = np.sqrt(s)
                const = float(r[0]) if np.all(s == s[0]) else None
                return (FAST_T0, const)
            return (FAST_T, None)
        return (SLOW, None)

    modes = (mode_of(s1, t1), mode_of(s2, t2), mode_of(s3, t3))

    def sc_bi(lmode, s, m_tiles):
        mode = lmode[0]
        if mode == SLOW:
            return _vec_tile(s, m_tiles), np.ones((128, m_tiles), np.float32)
        r = np.sqrt(s)
        return _vec_tile(r, m_tiles), _vec_tile(r, m_tiles)

    sc1, bi1 = sc_bi(modes[0], s1, M1)
    sc2, bi2 = sc_bi(modes[1], s2, M1)
    sc3, bi3 = sc_bi(modes[2], s3, M3)
    scb = np.concatenate([sc1, bi1, sc2, bi2, sc3, bi3], axis=1)
    shb = np.concatenate([_vec_tile(t1, M1), _vec_tile(t2, M1),
                          _vec_tile(t3, M3)], axis=1)

    w1t = _bf16(np.ascontiguousarray(w1[:, :, 0, 0].T))          # [1024,256]
    # w2: [o, i, kh, kw] -> 1D-winograd F(2,3) along kw: U_j = G @ w[kw]
    G = np.array([[1, 0, 0], [.5, .5, .5], [.5, -.5, .5], [0, 0, 1]],
                 np.float64)
    U = np.einsum('jw,oihw->oihj', G, w2.astype(np.float64))  # [o,i,kh,j]
    w2t = _bf16(np.ascontiguousarray(
        U.transpose(1, 2, 3, 0)                   # [i, kh, j, o]
          .reshape(K2, 128, 3, 4, C_MID)          # [kt, i128, kh, j, o]
          .transpose(0, 2, 3, 1, 4)))             # [kt, kh, j, i128, o]
    w3t = _bf16(np.ascontiguousarray(w3[:, :, 0, 0].T))          # [256,1024]

    shared = dict(w1t=w1t, w2t=w2t, w3t=w3t, scb=scb, shb=shb)
    return modes, shared


def kernel(**inputs):
    inputs = {k: np.asarray(v) for k, v in inputs.items()}
    x = inputs.pop("x").astype(np.float32)
    modes, shared = prepare(**inputs)
    r = _get_runner(modes)
    jax = r["jax"]

    n_cores = 8
    # x: [128, 1024, 14, 14] -> [core(8) x k(8), 128, 16, 196] bf16 channel-major
    x_cm = _bf16(x.reshape(8, B, K1, 128, S)
                 .transpose(0, 2, 3, 1, 4)
                 .reshape(n_cores * K1, 128, B, S))
    dev_in = []
    for name in r["in_names"]:
        if name == "x":
            cat = x_cm
        else:
            a = shared[name]
            cat = np.concatenate([a] * n_cores, axis=0)
        dev_in.append(jax.device_put(cat, r["sharding"]))
    zero_outs = [
        jax.device_put(np.zeros((n_cores * av.shape[0], *av.shape[1:]), av.dtype),
                       r["sharding"])
        for av in r["out_avals"]
    ]
    outs = r["sharded"](*dev_in, *zero_outs)
    jax.block_until_ready(outs)
    out = np.asarray(outs[r["out_names"].index("out")])
    # [core x m(8), 128, 16, 196] bf16 -> [128, 1024, 14, 14] f32
    return np.ascontiguousarray(
        out.reshape(n_cores, M3, 128, B, S)
           .transpose(0, 3, 1, 2, 4)
           .reshape(128, C_IN, HW, HW)).astype(np.float32)


# revision 73
# speedup vs baseline: 1.1136x; 1.0122x over previous
"""TRN2 Bass kernel for nn_Block_6476810682806 (dense_cnn).

Bottleneck block: 1x1 kerv -> BN -> 3x3 kerv -> BN -> 1x1 kerv -> BN -> +residual,
where kerv(x) = (conv(x) + 1)^2 and BN is inference-mode (frozen stats).

Distribution: data-parallel over batch (128 -> 16 per core) across 8 cores,
weights replicated. Each core computes its shard fully independently.

Device strategy (per core):
  - everything that crosses DMA is bf16 (halves HBM traffic; norm_rel ~5.6e-3
    vs the 2e-2 gate). Host pre-transposes x/out to channel-major so bf16
    descriptor runs stay >=512B (smaller runs pay a 2x DMA latency penalty).
  - activations channel-major: [C partitions, batch*spatial free]
  - convs as PE matmuls in bf16 (1 cyc/row at any N), f32 PSUM accumulate
  - 3x3 conv: 9 shifted matmuls over a zero-padded per-image 16x16 SBUF plane
  - BN scale folded into the kervolution square on ACT:
        s*(y+1)^2 = (sqrt(s)*y + sqrt(s))^2  (requires s > 0)
    shifts (t = b - m*s) are zero for this problem's fills; generic paths
    emit an extra per-channel add / affine when they are not.
  - residual add on DVE, straight from the resident x tiles (all-bf16 SBUF
    operands hit the DVE 2x/4x fast modes)
  - pass plan 4+4+4+2+2 images: small final passes shrink the tail drain
    (ACT pointwise + DVE residual + store DMA after the last matmul)
"""

import numpy as np

import concourse.bacc as bacc
import concourse.mybir as mybir
import concourse.tile as tile

F32 = mybir.dt.float32
BF = mybir.dt.bfloat16
EPS = 1e-5

B = 16          # images per core
C_IN = 1024
C_MID = 256
HW = 14
S = HW * HW     # 196
N = 2 * S       # matmul moving size per image pair = 392
PAD = 16        # padded plane side
PS = PAD * PAD  # 256 padded plane size
K1 = C_IN // 128          # 8
K2 = C_MID // 128         # 2
M1 = C_MID // 128         # 2
M3 = C_IN // 128          # 8
MAXBP = 4                 # max images per pass

# layer modes
FAST_T0 = 0   # all s>0, all t==0: ACT-only pointwise
FAST_T = 1    # all s>0, some t!=0: ACT + per-channel add
SLOW = 2      # some s<=0: plain square on ACT + DVE affine

# packed scale/bias column offsets in scb [128, 24]
SC1, BI1, SC2, BI2, SC3, BI3 = 0, 2, 4, 6, 8, 16
# packed shift column offsets in shb [128, 12]
SH1, SH2, SH3 = 0, 2, 4


def _build(modes, reps=None):
    mode1, mode2, mode3 = modes
    nc = bacc.Bacc("TRN2", target_bir_lowering=False, debug=False)

    x_d = nc.dram_tensor("x", [K1, 128, B, S], BF, kind="ExternalInput").ap()
    w1_d = nc.dram_tensor("w1t", [C_IN, C_MID], BF, kind="ExternalInput").ap()
    w2_d = nc.dram_tensor("w2t", [K2, 3, 4, 128, C_MID], BF,
                          kind="ExternalInput").ap()
    w3_d = nc.dram_tensor("w3t", [C_MID, C_IN], BF, kind="ExternalInput").ap()
    scb_d = nc.dram_tensor("scb", [128, 24], F32, kind="ExternalInput").ap()
    shb_d = nc.dram_tensor("shb", [128, 12], F32, kind="ExternalInput").ap()
    out_d = nc.dram_tensor("out", [M3, 128, B, S], BF, kind="ExternalOutput").ap()

    x_cm = x_d.rearrange("k p n s -> p k n s")     # [128, 8, 16, 196]
    out_cm = out_d.rearrange("m p n s -> p m n s")  # [128, 8, 16, 196]

    Sq = mybir.ActivationFunctionType.Square
    Alu = mybir.AluOpType

    with tile.TileContext(nc) as tc:
        with (
            tc.tile_pool(name="wpool", bufs=1) as wpool,
            tc.tile_pool(name="xpool", bufs=4) as xpool,
            tc.tile_pool(name="h1pool", bufs=2) as h1pool,
            tc.tile_pool(name="h2pool", bufs=2) as h2pool,
            tc.tile_pool(name="tpool", bufs=2) as tpool,
            tc.tile_pool(name="vpool", bufs=2) as vpool,
            tc.tile_pool(name="opool", bufs=6) as opool,
            tc.tile_pool(name="psp", bufs=4, space="PSUM") as pspool,
        ):
            # every PSUM tile is 2 banks; accumulation groups go to the
            # bank-aligned halves [0:N] and [HB:HB+N], drained by ONE
            # strided ACT op (halves the per-op init overhead share)
            HB = 512
            def xcol(xh, k, j):
                # [128, 2, S] rhs slice for k-tile k, image pair j
                v = xh[j][:].rearrange("p (k n s) -> p k n s", k=K1, n=2)
                return v[:, k, :, :]

            # ---- startup: one serialized DMA stream (SP queue) in first-use
            # order: xj0, scale vec, w1, w2, xj1, w3 ----
            def load_xj(pair, j):
                # pair: global image-pair index 0..7; j: slot parity in pass
                t = xpool.tile([128, K1 * 2 * S], BF, tag=f"x{j}",
                               name=f"xt_q{pair}")
                nc.sync.dma_start(
                    t[:].rearrange("p (k n s) -> p k n s", k=K1, n=2),
                    x_cm[:, :, 2 * pair:2 * pair + 2, :])
                return t

            # first x pair split into k-halves so conv1's first matmuls can
            # start ~1us sooner (w1 slots between the halves)
            xj0 = xpool.tile([128, K1 * 2 * S], BF, tag="x0", name="xt_q0")
            xj0v = xj0[:].rearrange("p (k n s) -> p k n s", k=K1, n=2)
            nc.sync.dma_start(xj0v[:, 0:K1 // 2], x_cm[:, 0:K1 // 2, 0:2, :])
            w1view = w1_d.rearrange("(k p) o -> p k o", p=128)
            w1s = wpool.tile([128, K1 * C_MID], BF, tag="w1s")
            w1v = w1s[:].rearrange("p (k o) -> p k o", k=K1)
            nc.sync.dma_start(w1v[:, 0:K1 // 2], w1view[:, 0:K1 // 2])
            nc.sync.dma_start(xj0v[:, K1 // 2:], x_cm[:, K1 // 2:, 0:2, :])
            nc.sync.dma_start(w1v[:, K1 // 2:], w1view[:, K1 // 2:])
            scb = wpool.tile([128, 24], F32, tag="scb")
            nc.sync.dma_start(scb[:], scb_d)
            if any(mo[0] != FAST_T0 for mo in modes):
                shb = wpool.tile([128, 12], F32, tag="shb")
                nc.sync.dma_start(shb[:], shb_d)
            else:
                shb = None
            xj1 = load_xj(1, 1)
            xt0 = [xj0, xj1]
            w2view = w2_d.rearrange("k h j p o -> p (k h j) o")
            w2s = wpool.tile([128, 24 * C_MID], BF, tag="w2s")
            w2v = w2s[:].rearrange("p (kt o) -> p kt o", kt=24)
            nc.sync.dma_start(w2v[:], w2view[:])
            w3s = wpool.tile([128, K2 * C_IN], BF, tag="w3s")
            nc.sync.dma_start(
                w3s[:].rearrange("p (k o) -> p k o", k=K2),
                w3_d.rearrange("(k p) o -> p k o", p=128))

            def w1ap(k, m):
                return w1s[:, k * C_MID + m * 128: k * C_MID + (m + 1) * 128]

            def w2ap(kt, kh, j4, m):
                base = ((kt * 3 + kh) * 4 + j4) * C_MID + m * 128
                return w2s[:, base: base + 128]

            def w3ap(k, m):
                return w3s[:, k * C_IN + m * 128: k * C_IN + (m + 1) * 128]

            def pointwise(lmode, src_ap, out_ap, sc_off, sh_off, m):
                """out = s*(src+1)^2 + t, written to out_ap.

                lmode is (mode, const): const is sqrt(s) as a python float
                when s is channel-uniform (allows m-paired drains), else
                None (per-channel scb column; src must be single-m)."""
                mode, const = lmode
                if mode == SLOW:
                    nc.scalar.activation(out_ap, src_ap, Sq, bias=1.0, scale=1.0)
                    nc.vector.tensor_scalar(
                        out_ap, out_ap, scb[:, sc_off + m:sc_off + m + 1],
                        shb[:, sh_off + m:sh_off + m + 1], Alu.mult, Alu.add)
                else:
                    # for m-paired drains (const flag set) the scale is
                    # channel-uniform, so the first m's column is valid for
                    # the whole pair
                    nc.scalar.activation(
                        out_ap, src_ap, Sq,
                        bias=scb[:, sc_off + (M1 if sc_off < SC3 else M3) + m:
                                 sc_off + (M1 if sc_off < SC3 else M3) + m + 1],
                        scale=scb[:, sc_off + m:sc_off + m + 1])
                    if mode == FAST_T:
                        nc.vector.tensor_scalar(
                            out_ap, out_ap, shb[:, sh_off + m:sh_off + m + 1],
                            None, Alu.add)

            def pointwise_dve(src_ap, dst_ap, sc_off, m, nelem):
                """conv1 pointwise on DVE (FAST_T0 only): frees the ACT
                queue for conv3 drains at pass boundaries.
                t = sqrt(s)*y + sqrt(s); dst = t*t."""
                tq = tpool.tile([128, 2 * N], BF, tag="tq")
                tv = tq[:, 0:nelem]
                if nelem > N:
                    tv = tv.rearrange("c (j b) -> c j b", b=N)
                nc.vector.tensor_scalar(
                    tv, src_ap, scb[:, sc_off + m:sc_off + m + 1],
                    scb[:, sc_off + m:sc_off + m + 1], Alu.mult, Alu.add)
                tsq = (tq[:, 0:nelem]
                       .rearrange("c (n a b) -> c n a b", a=HW, b=HW))
                nc.vector.tensor_tensor(dst_ap, tsq, tsq, Alu.mult)

            # ---- PE warmup: dummy matmuls on scratch data keep the PE
            # clock ramping while the startup DMAs land; the early dummy
            # activation pulls the act-table load off the critical path ----
            wu = wpool.tile([128, 128], BF, tag="wu")
            nc.vector.memset(wu[:].bitcast(F32), 0.0)
            wusc = wpool.tile([128, 4], F32, tag="wusc")
            # act-table preload reads SBUF (reading the warmup PSUM tile
            # would WAR-serialize every warmup matmul behind the 1.3us
            # LoadActFuncSet)
            nc.scalar.activation(wusc[:], wu[:].bitcast(F32)[:, 0:4], Sq,
                                 bias=1.0, scale=1.0)
            wups = pspool.tile([128, 2 * HB], F32, tag="psp", name="wups")
            NWU = 120
            for i in range(NWU):
                nc.tensor.matmul(wups[:, 0:64], wu[:], wu[:, 0:64],
                                 start=(i == 0), stop=(i == NWU - 1))

            # ---- main passes: (first image pair index, n pairs) ----
            def alloc_h1(npairs):
                h1, vt = [], []
                for k in range(K2):
                    t = h1pool.tile([128, MAXBP * PS], BF, tag=f"h1_{k}",
                                    name=f"h1t{k}")
                    nc.gpsimd.memset(t[:, 0:2 * npairs * PS].bitcast(F32),
                                     0.0)
                    h1.append(t)
                    v = vpool.tile([128, MAXBP * 16 * 4 * 7], BF,
                                   tag=f"v_{k}", name=f"vt{k}")
                    vt.append(v)
                return h1, vt

            def emit_vtf(k, h1t, vtt, npairs):
                """1D-winograd input transform for one k-plane on GPSIMD:
                V0=d0-d2 V1=d1+d2 V2=d2-d1 V3=d1-d3 over width pairs."""
                nn = 2 * npairs
                hv = (h1t[:, 0:nn * PS]
                      .rearrange("c (n a b) -> c n a b", a=PAD, b=PAD))
                vv = (vtt[:, 0:nn * 16 * 4 * 7]
                      .rearrange("c (n r j t) -> c n r j t", n=nn, r=16, j=4))

                def dsel(c0, par):
                    # pad cols c0+2t+par for t=0..6 (stride-2 column pick)
                    return (hv[:, :, :, c0:c0 + 14]
                            .rearrange("c n r (t two) -> c n r t two", two=2)
                            [:, :, :, :, par])

                d0 = dsel(0, 0)
                d1 = dsel(0, 1)
                d2 = dsel(2, 0)
                d3 = dsel(2, 1)
                eng = nc.gpsimd
                eng.tensor_tensor(vv[:, :, :, 0, :], d0, d2, Alu.subtract)
                eng.tensor_tensor(vv[:, :, :, 1, :], d1, d2, Alu.add)
                eng.tensor_tensor(vv[:, :, :, 2, :], d2, d1, Alu.subtract)
                eng.tensor_tensor(vv[:, :, :, 3, :], d1, d3, Alu.subtract)

            def emit_vtf_dve_jmajor(h1, vt, npairs):
                """Pass-1 variant: transforms on DVE, freq-major across both
                k planes, so conv2's first freq GEMM unblocks after 2 ops."""
                nn = 2 * npairs
                for j4 in range(4):
                    for k in range(K2):
                        hv = (h1[k][:, 0:nn * PS]
                              .rearrange("c (n a b) -> c n a b",
                                         a=PAD, b=PAD))
                        vv = (vt[k][:, 0:nn * 16 * 4 * 7]
                              .rearrange("c (n r j t) -> c n r j t",
                                         n=nn, r=16, j=4))

                        def dsel(c0, par):
                            return (hv[:, :, :, c0:c0 + 14]
                                    .rearrange("c n r (t two) -> c n r t two",
                                               two=2)[:, :, :, :, par])

                        pairs = {0: (dsel(0, 0), dsel(2, 0), Alu.subtract),
                                 1: (dsel(0, 1), dsel(2, 0), Alu.add),
                                 2: (dsel(2, 0), dsel(0, 1), Alu.subtract),
                                 3: (dsel(0, 1), dsel(2, 1), Alu.subtract)}
                        a, b, op = pairs[j4]
                        nc.vector.tensor_tensor(vv[:, :, :, j4, :], a, b, op)

            def conv1_chunks(pi, npairs, xt, h1, vt):
                """Returns a list of emitter callables (2 chunks) for this
                pass's conv1; each chunk is one PSUM tile's worth."""
                def emit_pair(m):
                    # pair (j0,j1) per m: PSUM halves, one drain per m
                    ps = pspool.tile([128, 2 * HB], F32, tag="psp",
                                     name=f"c1ps{m}")
                    for j in range(2):
                        for k in range(K1):
                            nc.tensor.matmul(
                                ps[:, j * HB:j * HB + N],
                                w1ap(k, m), xcol(xt, k, j),
                                start=(k == 0), stop=(k == K1 - 1))
                    src = ps[:].rearrange("c (j b) -> c j b", j=2)[:, :, 0:N]
                    dst = (h1[m][:]
                           .rearrange("c (n a b) -> c n a b", a=PAD, b=PAD)
                           [:, 0:4, 1:1 + HW, 1:1 + HW])
                    pointwise(mode1, src, dst, SC1, SH1, m)

                def emit_single(j, m):
                    ps = pspool.tile([128, 2 * HB], F32, tag="psp",
                                     name=f"c1ps{j}_{m}")
                    for k in range(K1):
                        nc.tensor.matmul(
                            ps[:, 0:N], w1ap(k, m), xcol(xt, k, j),
                            start=(k == 0), stop=(k == K1 - 1))
                    dst = (h1[m][:]
                           .rearrange("c (n a b) -> c n a b", a=PAD, b=PAD)
                           [:, 2 * j:2 * j + 2, 1:1 + HW, 1:1 + HW])
                    pointwise(mode1, ps[:, 0:N], dst, SC1, SH1, m)

                def emit_j0_khalves():
                    # startup: both m groups in one tile, k-halves
                    # interleaved, so matmuls start on the first half-x DMA
                    ps = pspool.tile([128, 2 * HB], F32, tag="psp",
                                     name="c1ps_j0")
                    for khalf in range(2):
                        for m in range(M1):
                            for k in range(4 * khalf, 4 * khalf + 4):
                                nc.tensor.matmul(
                                    ps[:, m * HB:m * HB + N],
                                    w1ap(k, m), xcol(xt, k, 0),
                                    start=(k == 0), stop=(k == K1 - 1),
                                    skip_group_check=True)
                    for m in range(M1):
                        dst = (h1[m][:]
                               .rearrange("c (n a b) -> c n a b",
                                          a=PAD, b=PAD)
                               [:, 0:2, 1:1 + HW, 1:1 + HW])
                        pointwise(mode1, ps[:, m * HB:m * HB + N],
                                  dst, SC1, SH1, m)

                if npairs == 2 and pi == 0:
                    # j-outer so conv1(j0) never waits on the xj1 DMA (a
                    # long stall would also reset the PE p-state clock)
                    def chunk1():
                        for m in range(M1):
                            emit_single(1, m)
                        emit_vtf_dve_jmajor(h1, vt, npairs)
                    return [emit_j0_khalves, chunk1]
                if npairs == 2:
                    def mk(m):
                        def c():
                            emit_pair(m)
                            emit_vtf(m, h1[m], vt[m], npairs)
                        return c
                    return [mk(m) for m in range(M1)]

                def mk1(m):
                    def c():
                        emit_single(0, m)
                        emit_vtf(m, h1[m], vt[m], npairs)
                    return c
                return [mk1(m) for m in range(M1)]

            def emit_passes():
              plan = [(0, 2), (2, 2), (4, 2), (6, 2)]
              # prefetch: emit pass p+1's x loads at the START of pass p so
              # they sit ahead of pass p's store DMAs in SP queue order
              xt_next = xt0
              h1_next = None
              for pi, (q0, npairs) in enumerate(plan):
                xt = xt_next
                if pi + 1 < len(plan):
                    nq0, nnp = plan[pi + 1]
                    xt_next = [load_xj(nq0 + j, j) for j in range(nnp)]

                if pi == 0:
                    h1, vt = alloc_h1(npairs)
                    for c in conv1_chunks(pi, npairs, xt, h1, vt):
                        c()
                else:
                    h1, vt = h1_next  # conv1 emitted inside pass pi-1

                # h2 per k-plane (conv3's k0 matmuls then don't wait on the
                # k1 plane's drain chain)
                h2l = [h2pool.tile([128, MAXBP * S], BF, tag=f"h2_{k}",
                                   name=f"h2t{k}")
                       for k in range(K2)]

                def h2ap(k, j, nj=1):
                    return h2l[k][:, j * N:(j + nj) * N]

                # conv1 for pass pi+1 is emitted interleaved into this
                # pass's conv3 (software pipelining: PE fills ACT's drain
                # lag with conv1 matmuls whose pointwise runs on DVE)
                if pi + 1 < len(plan):
                    h1_next = alloc_h1(plan[pi + 1][1])
                    next_chunks = conv1_chunks(pi + 1, plan[pi + 1][1],
                                               xt_next, *h1_next)
                else:
                    next_chunks = []

                # conv2: 3x3 pad 1 via 1D-winograd F(2,3) along width:
                # per (pair, m): 4 freq GEMMs (N=196) accumulating over
                # (kh, kt), then the A^T output transform on DVE and the
                # kervolution square on ACT
                nn = 2 * npairs
                vv = [vt[k][:, 0:nn * 16 * 4 * 7]
                      .rearrange("c (n r j t) -> c n r j t", n=nn, r=16, j=4)
                      for k in range(K2)]
                for jp in range(npairs):
                    for m in range(M1):
                        ps = pspool.tile([128, 2 * HB], F32, tag="psp")
                        for j4 in range(4):
                            off = (j4 // 2) * HB + (j4 % 2) * S
                            first = True
                            for kh in range(3):
                                for kt in range(K2):
                                    rhs = vv[kt][:, 2 * jp:2 * jp + 2,
                                                 kh:kh + HW, j4, :]
                                    nc.tensor.matmul(
                                        ps[:, off:off + S],
                                        w2ap(kt, kh, j4, m), rhs,
                                        start=first,
                                        stop=(kh == 2 and kt == K2 - 1))
                                    first = False
                        # output transform: even = M0+M1+M2, odd = M1-M2-M3
                        yv = (h2ap(m, jp)
                              .rearrange("c (n a b) -> c n a b", a=HW, b=HW))

                        def ysel(par):
                            return (yv.rearrange(
                                "c n a (t two) -> c n a t two", two=2)
                                [:, :, :, :, par])

                        def msel(j4):
                            off = (j4 // 2) * HB + (j4 % 2) * S
                            return (ps[:, off:off + S]
                                    .rearrange("c (n a t) -> c n a t",
                                               n=2, a=HW))

                        ye, yo = ysel(0), ysel(1)
                        # stage M1 to SBUF via ACT Copy, M2 via DVE (a
                        # TensorTensor may read at most one PSUM operand;
                        # splitting the staging balances the two engines —
                        # copy+square share every act table, so no reload)
                        s12 = tpool.tile([128, 2 * S], BF, tag="tq",
                                         name="s12")
                        sv = s12[:].rearrange("c (g n a t) -> c g n a t",
                                              g=2, n=2, a=HW)
                        Cp = mybir.ActivationFunctionType.Copy
                        nc.scalar.activation(sv[:, 0], msel(1), Cp)
                        nc.vector.tensor_scalar(sv[:, 1], msel(2), 1.0,
                                                None, Alu.mult)
                        nc.vector.tensor_tensor(ye, msel(0), sv[:, 0],
                                                Alu.add)
                        nc.vector.tensor_tensor(yo, sv[:, 0], sv[:, 1],
                                                Alu.subtract)
                        nc.vector.tensor_tensor(yo, yo, msel(3),
                                                Alu.subtract)
                        nc.vector.tensor_tensor(ye, ye, sv[:, 1], Alu.add)
                        # kervolution square, in place on the h2 slice
                        ph = h2ap(m, jp)
                        if mode2[0] == SLOW:
                            nc.scalar.activation(ph, ph, Sq, bias=1.0,
                                                 scale=1.0)
                            nc.vector.tensor_scalar(
                                ph, ph, scb[:, SC2 + m:SC2 + m + 1],
                                shb[:, SH2 + m:SH2 + m + 1],
                                Alu.mult, Alu.add)
                        else:
                            nc.scalar.activation(
                                ph, ph, Sq,
                                bias=scb[:, SC2 + M1 + m:SC2 + M1 + m + 1],
                                scale=scb[:, SC2 + m:SC2 + m + 1])
                            if mode2[0] == FAST_T:
                                nc.vector.tensor_scalar(
                                    ph, ph, shb[:, SH2 + m:SH2 + m + 1],
                                    None, Alu.add)

                # conv3: 1x1, C_MID -> C_IN, (2mp, 2mp+1) paired per j when
                # uniform; + residual, store per (m-pair, j)
                pair3 = mode3[1] is not None
                # interleave next-pass conv1 chunks where ring/drain waits
                # would otherwise stall PE: before group 1 (covers the
                # conv2-j1/c2 drain latency) and mid-phase
                if npairs == 2:
                    c1_at = {0: 0, 4: 1}
                else:
                    c1_at = {0: 0, 2: 1}
                gidx = 0
                if pi == len(plan) - 1 and npairs == 2:
                    # last pass j-outer: j0's whole drain/store chain
                    # completes under j1's matmuls, halving the tail
                    order = [(mp, j) for j in range(npairs)
                             for mp in range(M3 // 2)]
                else:
                    order = [(mp, j) for mp in range(M3 // 2)
                             for j in range(npairs)]
                for mp, j in order:
                    if True:
                        if gidx in c1_at and c1_at[gidx] < len(next_chunks):
                            next_chunks[c1_at[gidx]]()
                        gidx += 1
                        ps = pspool.tile([128, 2 * HB], F32, tag="psp")
                        for mi in range(2):
                            m = 2 * mp + mi
                            for k in range(K2):
                                nc.tensor.matmul(
                                    ps[:, mi * HB:mi * HB + N],
                                    w3ap(k, m), h2ap(k, j),
                                    start=(k == 0), stop=(k == K2 - 1))
                        zt = opool.tile([128, 2 * N], BF, tag="z")
                        last_pass = pi == len(plan) - 1
                        if (last_pass and mp == M3 // 2 - 1
                                and j == npairs - 1):
                            # final group: single-m drains + residuals (the
                            # chain after the very last matmul halves), but
                            # ONE paired store (two stores would serialize
                            # their HWDGE generations + DGE delays)
                            for mi in range(2):
                                pointwise(mode3, ps[:, mi * HB:mi * HB + N],
                                          zt[:, mi * N:(mi + 1) * N],
                                          SC3, SH3, 2 * mp + mi)
                                zvi = (zt[:, mi * N:(mi + 1) * N]
                                       .rearrange("c (n s) -> c n s", n=2))
                                xvi = (xt[j][:]
                                       .rearrange("p (k n s) -> p k n s",
                                                  k=K1, n=2)
                                       [:, 2 * mp + mi, :, :])
                                nc.vector.tensor_tensor(zvi, zvi, xvi,
                                                        Alu.add)
                            zv = zt[:].rearrange("c (m n s) -> c m n s",
                                                 m=2, n=2)
                            dst = out_cm[:, 2 * mp:2 * mp + 2,
                                         2 * (q0 + j):2 * (q0 + j) + 2, :]
                            nc.sync.dma_start(dst, zv)
                            continue
                        elif pair3:
                            src = (ps[:].rearrange("c (m b) -> c m b", m=2)
                                   [:, :, 0:N])
                            pointwise(mode3, src, zt[:], SC3, SH3, 2 * mp)
                        else:
                            for mi in range(2):
                                pointwise(mode3, ps[:, mi * HB:mi * HB + N],
                                          zt[:, mi * N:(mi + 1) * N],
                                          SC3, SH3, 2 * mp + mi)
                        zv = zt[:].rearrange("c (m n s) -> c m n s",
                                             m=2, n=2)
                        xv = (xt[j][:].rearrange("p (k n s) -> p k n s",
                                                 k=K1, n=2)
                              [:, 2 * mp:2 * mp + 2, :, :])
                        nc.vector.tensor_tensor(zv, zv, xv, Alu.add)
                        dst = out_cm[:, 2 * mp:2 * mp + 2,
                                     2 * (q0 + j):2 * (q0 + j) + 2, :]
                        nc.sync.dma_start(dst, zv)

            if reps is None:
                emit_passes()
            else:
                with tc.For_i(0, reps, 1):
                    emit_passes()

    nc.compile()
    return nc


# ---------------- host side ----------------

_CACHE = {}


def _get_runner(modes):
    if modes in _CACHE:
        return _CACHE[modes]
    import jax
    from jax.experimental.shard_map import shard_map
    from jax.sharding import Mesh, PartitionSpec
    from concourse.bass2jax import (_bass_exec_p, install_neuronx_cc_hook,
                                    partition_id_tensor)

    nc = _build(modes)
    install_neuronx_cc_hook()
    partition_name = nc.partition_id_tensor.name if nc.partition_id_tensor else None
    in_names, out_names, out_avals = [], [], []
    for alloc in nc.m.functions[0].allocations:
        if not isinstance(alloc, mybir.MemoryLocationSet):
            continue
        name = alloc.memorylocations[0].name
        if alloc.kind == "ExternalInput":
            if name != partition_name:
                in_names.append(name)
        elif alloc.kind == "ExternalOutput":
            out_names.append(name)
            out_avals.append(jax.core.ShapedArray(
                tuple(alloc.tensor_shape), mybir.dt.np(alloc.dtype)))
    n_params, n_outs = len(in_names), len(out_avals)
    all_in_names = list(in_names) + list(out_names)
    if partition_name is not None:
        all_in_names.append(partition_name)

    def _body(*args):
        operands = list(args)
        if partition_name is not None:
            operands.append(partition_id_tensor())
        outs = _bass_exec_p.bind(
            *operands,
            out_avals=tuple(out_avals),
            in_names=tuple(all_in_names),
            out_names=tuple(out_names),
            lowering_input_output_aliases=(),
            sim_require_finite=True,
            sim_require_nnan=True,
            nc=nc,
        )
        return tuple(outs)

    devices = jax.devices()[:8]
    mesh = Mesh(np.asarray(devices), ("core",))
    sharded = jax.jit(
        shard_map(_body, mesh=mesh,
                  in_specs=(PartitionSpec("core"),) * (n_params + n_outs),
                  out_specs=(PartitionSpec("core"),) * n_outs,
                  check_rep=False),
        donate_argnums=tuple(range(n_params, n_params + n_outs)),
        keep_unused=True,
    )
    sharding = jax.sharding.NamedSharding(mesh, PartitionSpec("core"))
    runner = dict(nc=nc, sharded=sharded, sharding=sharding, jax=jax,
                  in_names=in_names, out_names=out_names, out_avals=out_avals)
    _CACHE[modes] = runner
    return runner


def _vec_tile(v, m_tiles):
    """[C] -> [128, m_tiles] column-per-m-tile layout."""
    return np.ascontiguousarray(np.asarray(v).reshape(m_tiles, 128).T
                                .astype(np.float32))


def _bf16(a):
    import ml_dtypes
    return np.asarray(a, dtype=np.float32).astype(ml_dtypes.bfloat16)


def prepare(w1, w2, w3, g1, b1, m1, v1, g2, b2, m2, v2, g3, b3, m3, v3):
    """Host prep: returns (modes, shared_input_dict_without_x)."""
    s1 = g1 / np.sqrt(v1 + EPS)
    t1 = b1 - m1 * s1
    s2 = g2 / np.sqrt(v2 + EPS)
    t2 = b2 - m2 * s2
    s3 = g3 / np.sqrt(v3 + EPS)
    t3 = b3 - m3 * s3

    def mode_of(s, t):
        """Returns (mode, const): const = sqrt(s) as a float when s is
        exactly channel-uniform and shifts are zero (enables m-paired
        PSUM drains on ACT), else None."""
        if np.all(s > 0):
            if not np.any(t):
                r = np.sqrt(s)
                const = float(r[0]) if np.all(s == s[0]) else None
                return (FAST_T0, const)
            return (FAST_T, None)
        return (SLOW, None)

    modes = (mode_of(s1, t1), mode_of(s2, t2), mode_of(s3, t3))

    def sc_bi(lmode, s, m_tiles):
        mode = lmode[0]
        if mode == SLOW:
            return _vec_tile(s, m_tiles), np.ones((128, m_tiles), np.float32)
        r = np.sqrt(s)
        return _vec_tile(r, m_tiles), _vec_tile(r, m_tiles)

    sc1, bi1 = sc_bi(modes[0], s1, M1)
    sc2, bi2 = sc_bi(modes[1], s2, M1)
    sc3, bi3 = sc_bi(modes[2], s3, M3)
    scb = np.concatenate([sc1, bi1, sc2, bi2, sc3, bi3], axis=1)
    shb = np.concatenate([_vec_tile(t1, M1), _vec_tile(t2, M1),
                          _vec_tile(t3, M3)], axis=1)

    w1t = _bf16(np.ascontiguousarray(w1[:, :, 0, 0].T))          # [1024,256]
    # w2: [o, i, kh, kw] -> 1D-winograd F(2,3) along kw: U_j = G @ w[kw]
    G = np.array([[1, 0, 0], [.5, .5, .5], [.5, -.5, .5], [0, 0, 1]],
                 np.float64)
    U = np.einsum('jw,oihw->oihj', G, w2.astype(np.float64))  # [o,i,kh,j]
    w2t = _bf16(np.ascontiguousarray(
        U.transpose(1, 2, 3, 0)                   # [i, kh, j, o]
          .reshape(K2, 128, 3, 4, C_MID)          # [kt, i128, kh, j, o]
          .transpose(0, 2, 3, 1, 4)))             # [kt, kh, j, i128, o]
    w3t = _bf16(np.ascontiguousarray(w3[:, :, 0, 0].T))          # [256,1024]

    shared = dict(w1t=w1t, w2t=w2t, w3t=w3t, scb=scb, shb=shb)
    return modes, shared


def kernel(**inputs):
    inputs = {k: np.asarray(v) for k, v in inputs.items()}
    x = inputs.pop("x").astype(np.float32)
    modes, shared = prepare(**inputs)
    r = _get_runner(modes)
    jax = r["jax"]

    n_cores = 8
    # x: [128, 1024, 14, 14] -> [core(8) x k(8), 128, 16, 196] bf16 channel-major
    x_cm = _bf16(x.reshape(8, B, K1, 128, S)
                 .transpose(0, 2, 3, 1, 4)
                 .reshape(n_cores * K1, 128, B, S))
    dev_in = []
    for name in r["in_names"]:
        if name == "x":
            cat = x_cm
        else:
            a = shared[name]
            cat = np.concatenate([a] * n_cores, axis=0)
        dev_in.append(jax.device_put(cat, r["sharding"]))
    zero_outs = [
        jax.device_put(np.zeros((n_cores * av.shape[0], *av.shape[1:]), av.dtype),
                       r["sharding"])
        for av in r["out_avals"]
    ]
    outs = r["sharded"](*dev_in, *zero_outs)
    jax.block_until_ready(outs)
    out = np.asarray(outs[r["out_names"].index("out")])
    # [core x m(8), 128, 16, 196] bf16 -> [128, 1024, 14, 14] f32
    return np.ascontiguousarray(
        out.reshape(n_cores, M3, 128, B, S)
           .transpose(0, 3, 1, 2, 4)
           .reshape(128, C_IN, HW, HW)).astype(np.float32)


# revision 75
# speedup vs baseline: 1.1161x; 1.0022x over previous
"""TRN2 Bass kernel for nn_Block_6476810682806 (dense_cnn).

Bottleneck block: 1x1 kerv -> BN -> 3x3 kerv -> BN -> 1x1 kerv -> BN -> +residual,
where kerv(x) = (conv(x) + 1)^2 and BN is inference-mode (frozen stats).

Distribution: data-parallel over batch (128 -> 16 per core) across 8 cores,
weights replicated. Each core computes its shard fully independently.

Device strategy (per core):
  - everything that crosses DMA is bf16 (halves HBM traffic; norm_rel ~5.6e-3
    vs the 2e-2 gate). Host pre-transposes x/out to channel-major so bf16
    descriptor runs stay >=512B (smaller runs pay a 2x DMA latency penalty).
  - activations channel-major: [C partitions, batch*spatial free]
  - convs as PE matmuls in bf16 (1 cyc/row at any N), f32 PSUM accumulate
  - 3x3 conv: 9 shifted matmuls over a zero-padded per-image 16x16 SBUF plane
  - BN scale folded into the kervolution square on ACT:
        s*(y+1)^2 = (sqrt(s)*y + sqrt(s))^2  (requires s > 0)
    shifts (t = b - m*s) are zero for this problem's fills; generic paths
    emit an extra per-channel add / affine when they are not.
  - residual add on DVE, straight from the resident x tiles (all-bf16 SBUF
    operands hit the DVE 2x/4x fast modes)
  - pass plan 4+4+4+2+2 images: small final passes shrink the tail drain
    (ACT pointwise + DVE residual + store DMA after the last matmul)
"""

import numpy as np

import concourse.bacc as bacc
import concourse.mybir as mybir
import concourse.tile as tile

F32 = mybir.dt.float32
BF = mybir.dt.bfloat16
EPS = 1e-5

B = 16          # images per core
C_IN = 1024
C_MID = 256
HW = 14
S = HW * HW     # 196
N = 2 * S       # matmul moving size per image pair = 392
PAD = 16        # padded plane side
PS = PAD * PAD  # 256 padded plane size
K1 = C_IN // 128          # 8
K2 = C_MID // 128         # 2
M1 = C_MID // 128         # 2
M3 = C_IN // 128          # 8
MAXBP = 4                 # max images per pass

# layer modes
FAST_T0 = 0   # all s>0, all t==0: ACT-only pointwise
FAST_T = 1    # all s>0, some t!=0: ACT + per-channel add
SLOW = 2      # some s<=0: plain square on ACT + DVE affine

# packed scale/bias column offsets in scb [128, 24]
SC1, BI1, SC2, BI2, SC3, BI3 = 0, 2, 4, 6, 8, 16
# packed shift column offsets in shb [128, 12]
SH1, SH2, SH3 = 0, 2, 4


def _build(modes, reps=None):
    mode1, mode2, mode3 = modes
    nc = bacc.Bacc("TRN2", target_bir_lowering=False, debug=False)

    x_d = nc.dram_tensor("x", [K1, 128, B, S], BF, kind="ExternalInput").ap()
    w1_d = nc.dram_tensor("w1t", [C_IN, C_MID], BF, kind="ExternalInput").ap()
    w2_d = nc.dram_tensor("w2t", [K2, 3, 4, 128, C_MID], BF,
                          kind="ExternalInput").ap()
    w3_d = nc.dram_tensor("w3t", [C_MID, C_IN], BF, kind="ExternalInput").ap()
    scb_d = nc.dram_tensor("scb", [128, 24], F32, kind="ExternalInput").ap()
    shb_d = nc.dram_tensor("shb", [128, 12], F32, kind="ExternalInput").ap()
    out_d = nc.dram_tensor("out", [M3, 128, B, S], BF, kind="ExternalOutput").ap()

    x_cm = x_d.rearrange("k p n s -> p k n s")     # [128, 8, 16, 196]
    out_cm = out_d.rearrange("m p n s -> p m n s")  # [128, 8, 16, 196]

    Sq = mybir.ActivationFunctionType.Square
    Alu = mybir.AluOpType

    with tile.TileContext(nc) as tc:
        with (
            tc.tile_pool(name="wpool", bufs=1) as wpool,
            tc.tile_pool(name="xpool", bufs=4) as xpool,
            tc.tile_pool(name="h1pool", bufs=2) as h1pool,
            tc.tile_pool(name="h2pool", bufs=3) as h2pool,
            tc.tile_pool(name="tpool", bufs=4) as tpool,
            tc.tile_pool(name="vpool", bufs=2) as vpool,
            tc.tile_pool(name="opool", bufs=6) as opool,
            tc.tile_pool(name="psp", bufs=4, space="PSUM") as pspool,
        ):
            # every PSUM tile is 2 banks; accumulation groups go to the
            # bank-aligned halves [0:N] and [HB:HB+N], drained by ONE
            # strided ACT op (halves the per-op init overhead share)
            HB = 512
            def xcol(xh, k, j):
                # [128, 2, S] rhs slice for k-tile k, image pair j
                v = xh[j][:].rearrange("p (k n s) -> p k n s", k=K1, n=2)
                return v[:, k, :, :]

            # ---- startup: one serialized DMA stream (SP queue) in first-use
            # order: xj0, scale vec, w1, w2, xj1, w3 ----
            def load_xj(pair, j):
                # pair: global image-pair index 0..7; j: slot parity in pass
                t = xpool.tile([128, K1 * 2 * S], BF, tag=f"x{j}",
                               name=f"xt_q{pair}")
                nc.sync.dma_start(
                    t[:].rearrange("p (k n s) -> p k n s", k=K1, n=2),
                    x_cm[:, :, 2 * pair:2 * pair + 2, :])
                return t

            # first x pair split into k-halves so conv1's first matmuls can
            # start ~1us sooner (w1 slots between the halves)
            xj0 = xpool.tile([128, K1 * 2 * S], BF, tag="x0", name="xt_q0")
            xj0v = xj0[:].rearrange("p (k n s) -> p k n s", k=K1, n=2)
            nc.sync.dma_start(xj0v[:, 0:K1 // 2], x_cm[:, 0:K1 // 2, 0:2, :])
            w1view = w1_d.rearrange("(k p) o -> p k o", p=128)
            w1s = wpool.tile([128, K1 * C_MID], BF, tag="w1s")
            w1v = w1s[:].rearrange("p (k o) -> p k o", k=K1)
            nc.sync.dma_start(w1v[:, 0:K1 // 2], w1view[:, 0:K1 // 2])
            nc.sync.dma_start(xj0v[:, K1 // 2:], x_cm[:, K1 // 2:, 0:2, :])
            nc.sync.dma_start(w1v[:, K1 // 2:], w1view[:, K1 // 2:])
            scb = wpool.tile([128, 24], F32, tag="scb")
            nc.sync.dma_start(scb[:], scb_d)
            if any(mo[0] != FAST_T0 for mo in modes):
                shb = wpool.tile([128, 12], F32, tag="shb")
                nc.sync.dma_start(shb[:], shb_d)
            else:
                shb = None
            xj1 = load_xj(1, 1)
            xt0 = [xj0, xj1]
            w2view = w2_d.rearrange("k h j p o -> p (k h j) o")
            w2s = wpool.tile([128, 24 * C_MID], BF, tag="w2s")
            w2v = w2s[:].rearrange("p (kt o) -> p kt o", kt=24)
            nc.sync.dma_start(w2v[:], w2view[:])
            w3s = wpool.tile([128, K2 * C_IN], BF, tag="w3s")
            nc.sync.dma_start(
                w3s[:].rearrange("p (k o) -> p k o", k=K2),
                w3_d.rearrange("(k p) o -> p k o", p=128))

            def w1ap(k, m):
                return w1s[:, k * C_MID + m * 128: k * C_MID + (m + 1) * 128]

            def w2ap(kt, kh, j4, m):
                base = ((kt * 3 + kh) * 4 + j4) * C_MID + m * 128
                return w2s[:, base: base + 128]

            def w3ap(k, m):
                return w3s[:, k * C_IN + m * 128: k * C_IN + (m + 1) * 128]

            def pointwise(lmode, src_ap, out_ap, sc_off, sh_off, m):
                """out = s*(src+1)^2 + t, written to out_ap.

                lmode is (mode, const): const is sqrt(s) as a python float
                when s is channel-uniform (allows m-paired drains), else
                None (per-channel scb column; src must be single-m)."""
                mode, const = lmode
                if mode == SLOW:
                    nc.scalar.activation(out_ap, src_ap, Sq, bias=1.0, scale=1.0)
                    nc.vector.tensor_scalar(
                        out_ap, out_ap, scb[:, sc_off + m:sc_off + m + 1],
                        shb[:, sh_off + m:sh_off + m + 1], Alu.mult, Alu.add)
                else:
                    # for m-paired drains (const flag set) the scale is
                    # channel-uniform, so the first m's column is valid for
                    # the whole pair
                    nc.scalar.activation(
                        out_ap, src_ap, Sq,
                        bias=scb[:, sc_off + (M1 if sc_off < SC3 else M3) + m:
                                 sc_off + (M1 if sc_off < SC3 else M3) + m + 1],
                        scale=scb[:, sc_off + m:sc_off + m + 1])
                    if mode == FAST_T:
                        nc.vector.tensor_scalar(
                            out_ap, out_ap, shb[:, sh_off + m:sh_off + m + 1],
                            None, Alu.add)

            def pointwise_dve(src_ap, dst_ap, sc_off, m, nelem):
                """conv1 pointwise on DVE (FAST_T0 only): frees the ACT
                queue for conv3 drains at pass boundaries.
                t = sqrt(s)*y + sqrt(s); dst = t*t."""
                tq = tpool.tile([128, 2 * N], BF, tag="tq")
                tv = tq[:, 0:nelem]
                if nelem > N:
                    tv = tv.rearrange("c (j b) -> c j b", b=N)
                nc.vector.tensor_scalar(
                    tv, src_ap, scb[:, sc_off + m:sc_off + m + 1],
                    scb[:, sc_off + m:sc_off + m + 1], Alu.mult, Alu.add)
                tsq = (tq[:, 0:nelem]
                       .rearrange("c (n a b) -> c n a b", a=HW, b=HW))
                nc.vector.tensor_tensor(dst_ap, tsq, tsq, Alu.mult)

            # ---- PE warmup: dummy matmuls on scratch data keep the PE
            # clock ramping while the startup DMAs land; the early dummy
            # activation pulls the act-table load off the critical path ----
            wu = wpool.tile([128, 128], BF, tag="wu")
            nc.vector.memset(wu[:].bitcast(F32), 0.0)
            wusc = wpool.tile([128, 4], F32, tag="wusc")
            # act-table preload reads SBUF (reading the warmup PSUM tile
            # would WAR-serialize every warmup matmul behind the 1.3us
            # LoadActFuncSet)
            nc.scalar.activation(wusc[:], wu[:].bitcast(F32)[:, 0:4], Sq,
                                 bias=1.0, scale=1.0)
            wups = pspool.tile([128, 2 * HB], F32, tag="psp", name="wups")
            NWU = 120
            for i in range(NWU):
                nc.tensor.matmul(wups[:, 0:64], wu[:], wu[:, 0:64],
                                 start=(i == 0), stop=(i == NWU - 1))

            # ---- main passes: (first image pair index, n pairs) ----
            def alloc_h1(npairs):
                h1, vt = [], []
                for k in range(K2):
                    t = h1pool.tile([128, MAXBP * PS], BF, tag=f"h1_{k}",
                                    name=f"h1t{k}")
                    nc.gpsimd.memset(t[:, 0:2 * npairs * PS].bitcast(F32),
                                     0.0)
                    h1.append(t)
                    v = vpool.tile([128, MAXBP * 16 * 4 * 7], BF,
                                   tag=f"v_{k}", name=f"vt{k}")
                    vt.append(v)
                return h1, vt

            def emit_vtf(k, h1t, vtt, npairs):
                """1D-winograd input transform for one k-plane on GPSIMD:
                V0=d0-d2 V1=d1+d2 V2=d2-d1 V3=d1-d3 over width pairs."""
                nn = 2 * npairs
                hv = (h1t[:, 0:nn * PS]
                      .rearrange("c (n a b) -> c n a b", a=PAD, b=PAD))
                vv = (vtt[:, 0:nn * 16 * 4 * 7]
                      .rearrange("c (n r j t) -> c n r j t", n=nn, r=16, j=4))

                def dsel(c0, par):
                    # pad cols c0+2t+par for t=0..6 (stride-2 column pick)
                    return (hv[:, :, :, c0:c0 + 14]
                            .rearrange("c n r (t two) -> c n r t two", two=2)
                            [:, :, :, :, par])

                d0 = dsel(0, 0)
                d1 = dsel(0, 1)
                d2 = dsel(2, 0)
                d3 = dsel(2, 1)
                eng = nc.gpsimd
                eng.tensor_tensor(vv[:, :, :, 0, :], d0, d2, Alu.subtract)
                eng.tensor_tensor(vv[:, :, :, 1, :], d1, d2, Alu.add)
                eng.tensor_tensor(vv[:, :, :, 2, :], d2, d1, Alu.subtract)
                eng.tensor_tensor(vv[:, :, :, 3, :], d1, d3, Alu.subtract)

            def emit_vtf_dve_jmajor(h1, vt, npairs):
                """Pass-1 variant: transforms on DVE, freq-major across both
                k planes, so conv2's first freq GEMM unblocks after 2 ops."""
                nn = 2 * npairs
                for j4 in range(4):
                    for k in range(K2):
                        hv = (h1[k][:, 0:nn * PS]
                              .rearrange("c (n a b) -> c n a b",
                                         a=PAD, b=PAD))
                        vv = (vt[k][:, 0:nn * 16 * 4 * 7]
                              .rearrange("c (n r j t) -> c n r j t",
                                         n=nn, r=16, j=4))

                        def dsel(c0, par):
                            return (hv[:, :, :, c0:c0 + 14]
                                    .rearrange("c n r (t two) -> c n r t two",
                                               two=2)[:, :, :, :, par])

                        pairs = {0: (dsel(0, 0), dsel(2, 0), Alu.subtract),
                                 1: (dsel(0, 1), dsel(2, 0), Alu.add),
                                 2: (dsel(2, 0), dsel(0, 1), Alu.subtract),
                                 3: (dsel(0, 1), dsel(2, 1), Alu.subtract)}
                        a, b, op = pairs[j4]
                        nc.vector.tensor_tensor(vv[:, :, :, j4, :], a, b, op)

            def conv1_chunks(pi, npairs, xt, h1, vt):
                """Returns a list of emitter callables (2 chunks) for this
                pass's conv1; each chunk is one PSUM tile's worth."""
                def emit_pair(m):
                    # pair (j0,j1) per m: PSUM halves, one drain per m
                    ps = pspool.tile([128, 2 * HB], F32, tag="psp",
                                     name=f"c1ps{m}")
                    for j in range(2):
                        for k in range(K1):
                            nc.tensor.matmul(
                                ps[:, j * HB:j * HB + N],
                                w1ap(k, m), xcol(xt, k, j),
                                start=(k == 0), stop=(k == K1 - 1))
                    src = ps[:].rearrange("c (j b) -> c j b", j=2)[:, :, 0:N]
                    dst = (h1[m][:]
                           .rearrange("c (n a b) -> c n a b", a=PAD, b=PAD)
                           [:, 0:4, 1:1 + HW, 1:1 + HW])
                    pointwise(mode1, src, dst, SC1, SH1, m)

                def emit_single(j, m):
                    ps = pspool.tile([128, 2 * HB], F32, tag="psp",
                                     name=f"c1ps{j}_{m}")
                    for k in range(K1):
                        nc.tensor.matmul(
                            ps[:, 0:N], w1ap(k, m), xcol(xt, k, j),
                            start=(k == 0), stop=(k == K1 - 1))
                    dst = (h1[m][:]
                           .rearrange("c (n a b) -> c n a b", a=PAD, b=PAD)
                           [:, 2 * j:2 * j + 2, 1:1 + HW, 1:1 + HW])
                    pointwise(mode1, ps[:, 0:N], dst, SC1, SH1, m)

                def emit_j0_khalves():
                    # startup: both m groups in one tile, k-halves
                    # interleaved, so matmuls start on the first half-x DMA
                    ps = pspool.tile([128, 2 * HB], F32, tag="psp",
                                     name="c1ps_j0")
                    for khalf in range(2):
                        for m in range(M1):
                            for k in range(4 * khalf, 4 * khalf + 4):
                                nc.tensor.matmul(
                                    ps[:, m * HB:m * HB + N],
                                    w1ap(k, m), xcol(xt, k, 0),
                                    start=(k == 0), stop=(k == K1 - 1),
                                    skip_group_check=True)
                    for m in range(M1):
                        dst = (h1[m][:]
                               .rearrange("c (n a b) -> c n a b",
                                          a=PAD, b=PAD)
                               [:, 0:2, 1:1 + HW, 1:1 + HW])
                        pointwise(mode1, ps[:, m * HB:m * HB + N],
                                  dst, SC1, SH1, m)

                if npairs == 2 and pi == 0:
                    # j-outer so conv1(j0) never waits on the xj1 DMA (a
                    # long stall would also reset the PE p-state clock)
                    def chunk1():
                        for m in range(M1):
                            emit_single(1, m)
                        emit_vtf_dve_jmajor(h1, vt, npairs)
                    return [emit_j0_khalves, chunk1]
                if npairs == 2:
                    def mk(m):
                        def c():
                            emit_pair(m)
                            emit_vtf(m, h1[m], vt[m], npairs)
                        return c
                    return [mk(m) for m in range(M1)]

                def mk1(m):
                    def c():
                        emit_single(0, m)
                        emit_vtf(m, h1[m], vt[m], npairs)
                    return c
                return [mk1(m) for m in range(M1)]

            def emit_passes():
              plan = [(0, 2), (2, 2), (4, 2), (6, 2)]
              # prefetch: emit pass p+1's x loads at the START of pass p so
              # they sit ahead of pass p's store DMAs in SP queue order
              xt_next = xt0
              h1_next = None
              for pi, (q0, npairs) in enumerate(plan):
                xt = xt_next
                if pi + 1 < len(plan):
                    nq0, nnp = plan[pi + 1]
                    xt_next = [load_xj(nq0 + j, j) for j in range(nnp)]

                if pi == 0:
                    h1, vt = alloc_h1(npairs)
                    for c in conv1_chunks(pi, npairs, xt, h1, vt):
                        c()
                else:
                    h1, vt = h1_next  # conv1 emitted inside pass pi-1

                # h2 per k-plane (conv3's k0 matmuls then don't wait on the
                # k1 plane's drain chain)
                h2l = [h2pool.tile([128, MAXBP * S], BF, tag=f"h2_{k}",
                                   name=f"h2t{k}")
                       for k in range(K2)]

                def h2ap(k, j, nj=1):
                    return h2l[k][:, j * N:(j + nj) * N]

                # conv1 for pass pi+1 is emitted interleaved into this
                # pass's conv3 (software pipelining: PE fills ACT's drain
                # lag with conv1 matmuls whose pointwise runs on DVE)
                if pi + 1 < len(plan):
                    h1_next = alloc_h1(plan[pi + 1][1])
                    next_chunks = conv1_chunks(pi + 1, plan[pi + 1][1],
                                               xt_next, *h1_next)
                else:
                    next_chunks = []

                # conv2: 3x3 pad 1 via 1D-winograd F(2,3) along width:
                # per (pair, m): 4 freq GEMMs (N=196) accumulating over
                # (kh, kt), then the A^T output transform on DVE and the
                # kervolution square on ACT
                nn = 2 * npairs
                vv = [vt[k][:, 0:nn * 16 * 4 * 7]
                      .rearrange("c (n r j t) -> c n r j t", n=nn, r=16, j=4)
                      for k in range(K2)]
                for jp in range(npairs):
                    for m in range(M1):
                        ps = pspool.tile([128, 2 * HB], F32, tag="psp")
                        for j4 in range(4):
                            off = (j4 // 2) * HB + (j4 % 2) * S
                            first = True
                            for kh in range(3):
                                for kt in range(K2):
                                    rhs = vv[kt][:, 2 * jp:2 * jp + 2,
                                                 kh:kh + HW, j4, :]
                                    nc.tensor.matmul(
                                        ps[:, off:off + S],
                                        w2ap(kt, kh, j4, m), rhs,
                                        start=first,
                                        stop=(kh == 2 and kt == K2 - 1))
                                    first = False
                        # output transform: even = M0+M1+M2, odd = M1-M2-M3
                        yv = (h2ap(m, jp)
                              .rearrange("c (n a b) -> c n a b", a=HW, b=HW))

                        def ysel(par):
                            return (yv.rearrange(
                                "c n a (t two) -> c n a t two", two=2)
                                [:, :, :, :, par])

                        def msel(j4):
                            off = (j4 // 2) * HB + (j4 % 2) * S
                            return (ps[:, off:off + S]
                                    .rearrange("c (n a t) -> c n a t",
                                               n=2, a=HW))

                        ye, yo = ysel(0), ysel(1)
                        # stage M1 to SBUF via ACT Copy, M2 via DVE (a
                        # TensorTensor may read at most one PSUM operand;
                        # splitting the staging balances the two engines —
                        # copy+square share every act table, so no reload)
                        s12 = tpool.tile([128, 2 * S], BF, tag="tq",
                                         name="s12")
                        sv = s12[:].rearrange("c (g n a t) -> c g n a t",
                                              g=2, n=2, a=HW)
                        Cp = mybir.ActivationFunctionType.Copy
                        nc.scalar.activation(sv[:, 0], msel(1), Cp)
                        nc.vector.tensor_scalar(sv[:, 1], msel(2), 1.0,
                                                None, Alu.mult)
                        nc.vector.tensor_tensor(ye, msel(0), sv[:, 0],
                                                Alu.add)
                        nc.vector.tensor_tensor(yo, sv[:, 0], sv[:, 1],
                                                Alu.subtract)
                        nc.vector.tensor_tensor(yo, yo, msel(3),
                                                Alu.subtract)
                        nc.vector.tensor_tensor(ye, ye, sv[:, 1], Alu.add)
                        # kervolution square, in place on the h2 slice
                        ph = h2ap(m, jp)
                        if mode2[0] == SLOW:
                            nc.scalar.activation(ph, ph, Sq, bias=1.0,
                                                 scale=1.0)
                            nc.vector.tensor_scalar(
                                ph, ph, scb[:, SC2 + m:SC2 + m + 1],
                                shb[:, SH2 + m:SH2 + m + 1],
                                Alu.mult, Alu.add)
                        else:
                            nc.scalar.activation(
                                ph, ph, Sq,
                                bias=scb[:, SC2 + M1 + m:SC2 + M1 + m + 1],
                                scale=scb[:, SC2 + m:SC2 + m + 1])
                            if mode2[0] == FAST_T:
                                nc.vector.tensor_scalar(
                                    ph, ph, shb[:, SH2 + m:SH2 + m + 1],
                                    None, Alu.add)

                # conv3: 1x1, C_MID -> C_IN, (2mp, 2mp+1) paired per j when
                # uniform; + residual, store per (m-pair, j)
                pair3 = mode3[1] is not None
                # interleave next-pass conv1 chunks where ring/drain waits
                # would otherwise stall PE: before group 1 (covers the
                # conv2-j1/c2 drain latency) and mid-phase
                if npairs == 2:
                    c1_at = {0: 0, 4: 1}
                else:
                    c1_at = {0: 0, 2: 1}
                gidx = 0
                if pi == len(plan) - 1 and npairs == 2:
                    # last pass j-outer: j0's whole drain/store chain
                    # completes under j1's matmuls, halving the tail
                    order = [(mp, j) for j in range(npairs)
                             for mp in range(M3 // 2)]
                else:
                    order = [(mp, j) for mp in range(M3 // 2)
                             for j in range(npairs)]
                for mp, j in order:
                    if True:
                        if gidx in c1_at and c1_at[gidx] < len(next_chunks):
                            next_chunks[c1_at[gidx]]()
                        gidx += 1
                        ps = pspool.tile([128, 2 * HB], F32, tag="psp")
                        for mi in range(2):
                            m = 2 * mp + mi
                            for k in range(K2):
                                nc.tensor.matmul(
                                    ps[:, mi * HB:mi * HB + N],
                                    w3ap(k, m), h2ap(k, j),
                                    start=(k == 0), stop=(k == K2 - 1))
                        zt = opool.tile([128, 2 * N], BF, tag="z")
                        last_pass = pi == len(plan) - 1
                        if (last_pass and pair3 and mode3[0] == FAST_T0
                                and j == npairs - 1 and mp == 1):
                            # tail relief: one of the final drains on DVE so
                            # ACT's serial end-of-kernel train is shorter
                            tq3 = tpool.tile([128, 2 * N], BF, tag="tq")
                            tv3 = tq3[:].rearrange("c (m b) -> c m b", b=N)
                            src = (ps[:].rearrange("c (m b) -> c m b", m=2)
                                   [:, :, 0:N])
                            nc.vector.tensor_scalar(
                                tv3, src,
                                scb[:, SC3 + 2 * mp:SC3 + 2 * mp + 1],
                                scb[:, SC3 + 2 * mp:SC3 + 2 * mp + 1],
                                Alu.mult, Alu.add)
                            nc.vector.tensor_tensor(zt[:], tq3[:], tq3[:],
                                                    Alu.mult)
                        elif (last_pass and mp == M3 // 2 - 1
                                and j == npairs - 1):
                            # final group: single-m drains + residuals (the
                            # chain after the very last matmul halves), but
                            # ONE paired store (two stores would serialize
                            # their HWDGE generations + DGE delays)
                            for mi in range(2):
                                pointwise(mode3, ps[:, mi * HB:mi * HB + N],
                                          zt[:, mi * N:(mi + 1) * N],
                                          SC3, SH3, 2 * mp + mi)
                                zvi = (zt[:, mi * N:(mi + 1) * N]
                                       .rearrange("c (n s) -> c n s", n=2))
                                xvi = (xt[j][:]
                                       .rearrange("p (k n s) -> p k n s",
                                                  k=K1, n=2)
                                       [:, 2 * mp + mi, :, :])
                                nc.vector.tensor_tensor(zvi, zvi, xvi,
                                                        Alu.add)
                            zv = zt[:].rearrange("c (m n s) -> c m n s",
                                                 m=2, n=2)
                            dst = out_cm[:, 2 * mp:2 * mp + 2,
                                         2 * (q0 + j):2 * (q0 + j) + 2, :]
                            nc.sync.dma_start(dst, zv)
                            continue
                        elif pair3:
                            src = (ps[:].rearrange("c (m b) -> c m b", m=2)
                                   [:, :, 0:N])
                            pointwise(mode3, src, zt[:], SC3, SH3, 2 * mp)
                        else:
                            for mi in range(2):
                                pointwise(mode3, ps[:, mi * HB:mi * HB + N],
                                          zt[:, mi * N:(mi + 1) * N],
                                          SC3, SH3, 2 * mp + mi)
                        zv = zt[:].rearrange("c (m n s) -> c m n s",
                                             m=2, n=2)
                        xv = (xt[j][:].rearrange("p (k n s) -> p k n s",
                                                 k=K1, n=2)
                              [:, 2 * mp:2 * mp + 2, :, :])
                        nc.vector.tensor_tensor(zv, zv, xv, Alu.add)
                        dst = out_cm[:, 2 * mp:2 * mp + 2,
                                     2 * (q0 + j):2 * (q0 + j) + 2, :]
                        nc.sync.dma_start(dst, zv)

            if reps is None:
                emit_passes()
            else:
                with tc.For_i(0, reps, 1):
                    emit_passes()

    nc.compile()
    return nc


# ---------------- host side ----------------

_CACHE = {}


def _get_runner(modes):
    if modes in _CACHE:
        return _CACHE[modes]
    import jax
    from jax.experimental.shard_map import shard_map
    from jax.sharding import Mesh, PartitionSpec
    from concourse.bass2jax import (_bass_exec_p, install_neuronx_cc_hook,
                                    partition_id_tensor)

    nc = _build(modes)
    install_neuronx_cc_hook()
    partition_name = nc.partition_id_tensor.name if nc.partition_id_tensor else None
    in_names, out_names, out_avals = [], [], []
    for alloc in nc.m.functions[0].allocations:
        if not isinstance(alloc, mybir.MemoryLocationSet):
            continue
        name = alloc.memorylocations[0].name
        if alloc.kind == "ExternalInput":
            if name != partition_name:
                in_names.append(name)
        elif alloc.kind == "ExternalOutput":
            out_names.append(name)
            out_avals.append(jax.core.ShapedArray(
                tuple(alloc.tensor_shape), mybir.dt.np(alloc.dtype)))
    n_params, n_outs = len(in_names), len(out_avals)
    all_in_names = list(in_names) + list(out_names)
    if partition_name is not None:
        all_in_names.append(partition_name)

    def _body(*args):
        operands = list(args)
        if partition_name is not None:
            operands.append(partition_id_tensor())
        outs = _bass_exec_p.bind(
            *operands,
            out_avals=tuple(out_avals),
            in_names=tuple(all_in_names),
            out_names=tuple(out_names),
            lowering_input_output_aliases=(),
            sim_require_finite=True,
            sim_require_nnan=True,
            nc=nc,
        )
        return tuple(outs)

    devices = jax.devices()[:8]
    mesh = Mesh(np.asarray(devices), ("core",))
    sharded = jax.jit(
        shard_map(_body, mesh=mesh,
                  in_specs=(PartitionSpec("core"),) * (n_params + n_outs),
                  out_specs=(PartitionSpec("core"),) * n_outs,
                  check_rep=False),
        donate_argnums=tuple(range(n_params, n_params + n_outs)),
        keep_unused=True,
    )
    sharding = jax.sharding.NamedSharding(mesh, PartitionSpec("core"))
    runner = dict(nc=nc, sharded=sharded, sharding=sharding, jax=jax,
                  in_names=in_names, out_names=out_names, out_avals=out_avals)
    _CACHE[modes] = runner
    return runner


def _vec_tile(v, m_tiles):
    """[C] -> [128, m_tiles] column-per-m-tile layout."""
    return np.ascontiguousarray(np.asarray(v).reshape(m_tiles, 128).T
                                .astype(np.float32))


def _bf16(a):
    import ml_dtypes
    return np.asarray(a, dtype=np.float32).astype(ml_dtypes.bfloat16)


def prepare(w1, w2, w3, g1, b1, m1, v1, g2, b2, m2, v2, g3, b3, m3, v3):
    """Host prep: returns (modes, shared_input_dict_without_x)."""
    s1 = g1 / np.sqrt(v1 + EPS)
    t1 = b1 - m1 * s1
    s2 = g2 / np.sqrt(v2 + EPS)
    t2 = b2 - m2 * s2
    s3 = g3 / np.sqrt(v3 + EPS)
    t3 = b3 - m3 * s3

    def mode_of(s, t):
        """Returns (mode, const): const = sqrt(s) as a float when s is
        exactly channel-uniform and shifts are zero (enables m-paired
        PSUM drains on ACT), else None."""
        if np.all(s > 0):
            if not np.any(t):
                r = np.sqrt(s)
                const = float(r[0]) if np.all(s == s[0]) else None
                return (FAST_T0, const)
            return (FAST_T, None)
        return (SLOW, None)

    modes = (mode_of(s1, t1), mode_of(s2, t2), mode_of(s3, t3))

    def sc_bi(lmode, s, m_tiles):
        mode = lmode[0]
        if mode == SLOW:
            return _vec_tile(s, m_tiles), np.ones((128, m_tiles), np.float32)
        r = np.sqrt(s)
        return _vec_tile(r, m_tiles), _vec_tile(r, m_tiles)

    sc1, bi1 = sc_bi(modes[0], s1, M1)
    sc2, bi2 = sc_bi(modes[1], s2, M1)
    sc3, bi3 = sc_bi(modes[2], s3, M3)
    scb = np.concatenate([sc1, bi1, sc2, bi2, sc3, bi3], axis=1)
    shb = np.concatenate([_vec_tile(t1, M1), _vec_tile(t2, M1),
                          _vec_tile(t3, M3)], axis=1)

    w1t = _bf16(np.ascontiguousarray(w1[:, :, 0, 0].T))          # [1024,256]
    # w2: [o, i, kh, kw] -> 1D-winograd F(2,3) along kw: U_j = G @ w[kw]
    G = np.array([[1, 0, 0], [.5, .5, .5], [.5, -.5, .5], [0, 0, 1]],
                 np.float64)
    U = np.einsum('jw,oihw->oihj', G, w2.astype(np.float64))  # [o,i,kh,j]
    w2t = _bf16(np.ascontiguousarray(
        U.transpose(1, 2, 3, 0)                   # [i, kh, j, o]
          .reshape(K2, 128, 3, 4, C_MID)          # [kt, i128, kh, j, o]
          .transpose(0, 2, 3, 1, 4)))             # [kt, kh, j, i128, o]
    w3t = _bf16(np.ascontiguousarray(w3[:, :, 0, 0].T))          # [256,1024]

    shared = dict(w1t=w1t, w2t=w2t, w3t=w3t, scb=scb, shb=shb)
    return modes, shared


def kernel(**inputs):
    inputs = {k: np.asarray(v) for k, v in inputs.items()}
    x = inputs.pop("x").astype(np.float32)
    modes, shared = prepare(**inputs)
    r = _get_runner(modes)
    jax = r["jax"]

    n_cores = 8
    # x: [128, 1024, 14, 14] -> [core(8) x k(8), 128, 16, 196] bf16 channel-major
    x_cm = _bf16(x.reshape(8, B, K1, 128, S)
                 .transpose(0, 2, 3, 1, 4)
                 .reshape(n_cores * K1, 128, B, S))
    dev_in = []
    for name in r["in_names"]:
        if name == "x":
            cat = x_cm
        else:
            a = shared[name]
            cat = np.concatenate([a] * n_cores, axis=0)
        dev_in.append(jax.device_put(cat, r["sharding"]))
    zero_outs = [
        jax.device_put(np.zeros((n_cores * av.shape[0], *av.shape[1:]), av.dtype),
                       r["sharding"])
        for av in r["out_avals"]
    ]
    outs = r["sharded"](*dev_in, *zero_outs)
    jax.block_until_ready(outs)
    out = np.asarray(outs[r["out_names"].index("out")])
    # [core x m(8), 128, 16, 196] bf16 -> [128, 1024, 14, 14] f32
    return np.ascontiguousarray(
        out.reshape(n_cores, M3, 128, B, S)
           .transpose(0, 3, 1, 2, 4)
           .reshape(128, C_IN, HW, HW)).astype(np.float32)


# revision 82
# speedup vs baseline: 1.1435x; 1.0246x over previous
"""TRN2 Bass kernel for nn_Block_6476810682806 (dense_cnn).

Bottleneck block: 1x1 kerv -> BN -> 3x3 kerv -> BN -> 1x1 kerv -> BN -> +residual,
where kerv(x) = (conv(x) + 1)^2 and BN is inference-mode (frozen stats).

Distribution: data-parallel over batch (128 -> 16 per core) across 8 cores,
weights replicated. Each core computes its shard fully independently.

Device strategy (per core):
  - everything that crosses DMA is bf16 (halves HBM traffic; norm_rel ~5.6e-3
    vs the 2e-2 gate). Host pre-transposes x/out to channel-major so bf16
    descriptor runs stay >=512B (smaller runs pay a 2x DMA latency penalty).
  - activations channel-major: [C partitions, batch*spatial free]
  - convs as PE matmuls in bf16 (1 cyc/row at any N), f32 PSUM accumulate
  - 3x3 conv: 9 shifted matmuls over a zero-padded per-image 16x16 SBUF plane
  - BN scale folded into the kervolution square on ACT:
        s*(y+1)^2 = (sqrt(s)*y + sqrt(s))^2  (requires s > 0)
    shifts (t = b - m*s) are zero for this problem's fills; generic paths
    emit an extra per-channel add / affine when they are not.
  - residual add on DVE, straight from the resident x tiles (all-bf16 SBUF
    operands hit the DVE 2x/4x fast modes)
  - pass plan 4x4 images; the last pass drains conv3 j-outer and splits
    its final drains (ACT/DVE) to shorten the end-of-kernel chain
"""

import numpy as np

import concourse.bacc as bacc
import concourse.mybir as mybir
import concourse.tile as tile

F32 = mybir.dt.float32
BF = mybir.dt.bfloat16
EPS = 1e-5

B = 16          # images per core
C_IN = 1024
C_MID = 256
HW = 14
S = HW * HW     # 196
N = 2 * S       # matmul moving size per image pair = 392
PAD = 16        # padded plane side
PS = PAD * PAD  # 256 padded plane size
K1 = C_IN // 128          # 8
K2 = C_MID // 128         # 2
M1 = C_MID // 128         # 2
M3 = C_IN // 128          # 8
MAXBP = 4                 # max images per pass

# layer modes
FAST_T0 = 0   # all s>0, all t==0: ACT-only pointwise
FAST_T = 1    # all s>0, some t!=0: ACT + per-channel add
SLOW = 2      # some s<=0: plain square on ACT + DVE affine

# packed scale/bias column offsets in scb [128, 24]
SC1, BI1, SC2, BI2, SC3, BI3 = 0, 2, 4, 6, 8, 16
# packed shift column offsets in shb [128, 12]
SH1, SH2, SH3 = 0, 2, 4


def _build(modes, reps=None):
    mode1, mode2, mode3 = modes
    nc = bacc.Bacc("TRN2", target_bir_lowering=False, debug=False)

    x_d = nc.dram_tensor("x", [K1, 128, B, S], BF, kind="ExternalInput").ap()
    w1_d = nc.dram_tensor("w1t", [C_IN, C_MID], BF, kind="ExternalInput").ap()
    w2_d = nc.dram_tensor("w2t", [4, K2, 3, 128, C_MID], BF,
                          kind="ExternalInput").ap()
    w3_d = nc.dram_tensor("w3t", [C_MID, C_IN], BF, kind="ExternalInput").ap()
    scb_d = nc.dram_tensor("scb", [128, 24], F32, kind="ExternalInput").ap()
    shb_d = nc.dram_tensor("shb", [128, 12], F32, kind="ExternalInput").ap()
    out_d = nc.dram_tensor("out", [M3, 128, B, S], BF, kind="ExternalOutput").ap()

    x_cm = x_d.rearrange("k p n s -> p k n s")     # [128, 8, 16, 196]
    out_cm = out_d.rearrange("m p n s -> p m n s")  # [128, 8, 16, 196]

    Sq = mybir.ActivationFunctionType.Square
    Alu = mybir.AluOpType

    with tile.TileContext(nc) as tc:
        with (
            tc.tile_pool(name="wpool", bufs=1) as wpool,
            tc.tile_pool(name="xpool", bufs=4) as xpool,
            tc.tile_pool(name="h1pool", bufs=2) as h1pool,
            tc.tile_pool(name="h2pool", bufs=3) as h2pool,
            tc.tile_pool(name="tpool", bufs=4) as tpool,
            tc.tile_pool(name="vpool", bufs=2) as vpool,
            tc.tile_pool(name="opool", bufs=6) as opool,
            tc.tile_pool(name="psp", bufs=4, space="PSUM") as pspool,
        ):
            # every PSUM tile is 2 banks; accumulation groups go to the
            # bank-aligned halves [0:N] and [HB:HB+N], drained by ONE
            # strided ACT op (halves the per-op init overhead share)
            HB = 512
            def xcol(xh, k, j):
                # [128, 2, S] rhs slice for k-tile k, image pair j
                v = xh[j][:].rearrange("p (k n s) -> p k n s", k=K1, n=2)
                return v[:, k, :, :]

            # ---- startup: one serialized DMA stream (SP queue) in first-use
            # order: xj0, scale vec, w1, w2, xj1, w3 ----
            def load_xj(pair, j):
                # pair: global image-pair index 0..7; j: slot parity in pass
                t = xpool.tile([128, K1 * 2 * S], BF, tag=f"x{j}",
                               name=f"xt_q{pair}")
                nc.sync.dma_start(
                    t[:].rearrange("p (k n s) -> p k n s", k=K1, n=2),
                    x_cm[:, :, 2 * pair:2 * pair + 2, :])
                return t

            # first x pair split into k-halves so conv1's first matmuls can
            # start ~1us sooner (w1 slots between the halves)
            xj0 = xpool.tile([128, K1 * 2 * S], BF, tag="x0", name="xt_q0")
            xj0v = xj0[:].rearrange("p (k n s) -> p k n s", k=K1, n=2)
            nc.sync.dma_start(xj0v[:, 0:K1 // 2], x_cm[:, 0:K1 // 2, 0:2, :])
            w1view = w1_d.rearrange("(k p) o -> p k o", p=128)
            w1s = wpool.tile([128, K1 * C_MID], BF, tag="w1s")
            w1v = w1s[:].rearrange("p (k o) -> p k o", k=K1)
            nc.sync.dma_start(w1v[:, 0:K1 // 2], w1view[:, 0:K1 // 2])
            nc.sync.dma_start(xj0v[:, K1 // 2:], x_cm[:, K1 // 2:, 0:2, :])
            nc.sync.dma_start(w1v[:, K1 // 2:], w1view[:, K1 // 2:])
            scb = wpool.tile([128, 24], F32, tag="scb")
            nc.sync.dma_start(scb[:], scb_d)
            if any(mo[0] != FAST_T0 for mo in modes):
                shb = wpool.tile([128, 12], F32, tag="shb")
                nc.sync.dma_start(shb[:], shb_d)
            else:
                shb = None
            xj1 = load_xj(1, 1)
            xt0 = [xj0, xj1]
            # w2 is freq-major and DMA'd in per-freq quarters: pass-1 conv2's
            # j4-th freq GEMM starts as soon as its quarter lands
            w2view = w2_d.rearrange("j k h p o -> p (j k h) o")
            w2s = wpool.tile([128, 24 * C_MID], BF, tag="w2s")
            w2v = w2s[:].rearrange("p (kt o) -> p kt o", kt=24)
            for q4 in range(4):
                nc.sync.dma_start(w2v[:, 6 * q4:6 * (q4 + 1)],
                                  w2view[:, 6 * q4:6 * (q4 + 1)])
            w3s = wpool.tile([128, K2 * C_IN], BF, tag="w3s")
            nc.sync.dma_start(
                w3s[:].rearrange("p (k o) -> p k o", k=K2),
                w3_d.rearrange("(k p) o -> p k o", p=128))

            def w1ap(k, m):
                return w1s[:, k * C_MID + m * 128: k * C_MID + (m + 1) * 128]

            def w2ap(kt, kh, j4, m):
                base = ((j4 * K2 + kt) * 3 + kh) * C_MID + m * 128
                return w2s[:, base: base + 128]

            def w3ap(k, m):
                return w3s[:, k * C_IN + m * 128: k * C_IN + (m + 1) * 128]

            def pointwise(lmode, src_ap, out_ap, sc_off, sh_off, m):
                """out = s*(src+1)^2 + t, written to out_ap.

                lmode is (mode, const): const is sqrt(s) as a python float
                when s is channel-uniform (allows m-paired drains), else
                None (per-channel scb column; src must be single-m)."""
                mode, const = lmode
                if mode == SLOW:
                    nc.scalar.activation(out_ap, src_ap, Sq, bias=1.0, scale=1.0)
                    nc.vector.tensor_scalar(
                        out_ap, out_ap, scb[:, sc_off + m:sc_off + m + 1],
                        shb[:, sh_off + m:sh_off + m + 1], Alu.mult, Alu.add)
                else:
                    # for m-paired drains (const flag set) the scale is
                    # channel-uniform, so the first m's column is valid for
                    # the whole pair
                    nc.scalar.activation(
                        out_ap, src_ap, Sq,
                        bias=scb[:, sc_off + (M1 if sc_off < SC3 else M3) + m:
                                 sc_off + (M1 if sc_off < SC3 else M3) + m + 1],
                        scale=scb[:, sc_off + m:sc_off + m + 1])
                    if mode == FAST_T:
                        nc.vector.tensor_scalar(
                            out_ap, out_ap, shb[:, sh_off + m:sh_off + m + 1],
                            None, Alu.add)

            def pointwise_dve(src_ap, dst_ap, sc_off, m, nelem):
                """conv1 pointwise on DVE (FAST_T0 only): frees the ACT
                queue for conv3 drains at pass boundaries.
                t = sqrt(s)*y + sqrt(s); dst = t*t."""
                tq = tpool.tile([128, 2 * N], BF, tag="tq")
                tv = tq[:, 0:nelem]
                if nelem > N:
                    tv = tv.rearrange("c (j b) -> c j b", b=N)
                nc.vector.tensor_scalar(
                    tv, src_ap, scb[:, sc_off + m:sc_off + m + 1],
                    scb[:, sc_off + m:sc_off + m + 1], Alu.mult, Alu.add)
                tsq = (tq[:, 0:nelem]
                       .rearrange("c (n a b) -> c n a b", a=HW, b=HW))
                nc.vector.tensor_tensor(dst_ap, tsq, tsq, Alu.mult)

            # ---- PE warmup: dummy matmuls on scratch data keep the PE
            # clock ramping while the startup DMAs land; the early dummy
            # activation pulls the act-table load off the critical path ----
            wu = wpool.tile([128, 128], BF, tag="wu")
            nc.vector.memset(wu[:].bitcast(F32), 0.0)
            wusc = wpool.tile([128, 4], F32, tag="wusc")
            # act-table preload reads SBUF (reading the warmup PSUM tile
            # would WAR-serialize every warmup matmul behind the 1.3us
            # LoadActFuncSet)
            nc.scalar.activation(wusc[:], wu[:].bitcast(F32)[:, 0:4], Sq,
                                 bias=1.0, scale=1.0)
            wups = pspool.tile([128, 2 * HB], F32, tag="psp", name="wups")
            NWU = 120
            for i in range(NWU):
                nc.tensor.matmul(wups[:, 0:64], wu[:], wu[:, 0:64],
                                 start=(i == 0), stop=(i == NWU - 1))

            # ---- main passes: (first image pair index, n pairs) ----
            def alloc_h1(npairs):
                h1, vt = [], []
                for k in range(K2):
                    t = h1pool.tile([128, MAXBP * PS], BF, tag=f"h1_{k}",
                                    name=f"h1t{k}")
                    nc.gpsimd.memset(t[:, 0:2 * npairs * PS].bitcast(F32),
                                     0.0)
                    h1.append(t)
                    v = vpool.tile([128, MAXBP * 16 * 4 * 7], BF,
                                   tag=f"v_{k}", name=f"vt{k}")
                    vt.append(v)
                return h1, vt

            def emit_vtf(k, h1t, vtt, npairs):
                """1D-winograd input transform for one k-plane on GPSIMD:
                V0=d0-d2 V1=d1+d2 V2=d2-d1 V3=d1-d3 over width pairs."""
                nn = 2 * npairs
                hv = (h1t[:, 0:nn * PS]
                      .rearrange("c (n a b) -> c n a b", a=PAD, b=PAD))
                vv = (vtt[:, 0:nn * 16 * 4 * 7]
                      .rearrange("c (n r j t) -> c n r j t", n=nn, r=16, j=4))

                def dsel(c0, par):
                    # pad cols c0+2t+par for t=0..6 (stride-2 column pick)
                    return (hv[:, :, :, c0:c0 + 14]
                            .rearrange("c n r (t two) -> c n r t two", two=2)
                            [:, :, :, :, par])

                d0 = dsel(0, 0)
                d1 = dsel(0, 1)
                d2 = dsel(2, 0)
                d3 = dsel(2, 1)
                eng = nc.gpsimd
                eng.tensor_tensor(vv[:, :, :, 0, :], d0, d2, Alu.subtract)
                eng.tensor_tensor(vv[:, :, :, 1, :], d1, d2, Alu.add)
                eng.tensor_tensor(vv[:, :, :, 2, :], d2, d1, Alu.subtract)
                eng.tensor_tensor(vv[:, :, :, 3, :], d1, d3, Alu.subtract)

            def emit_vtf_dve_jmajor(h1, vt, npairs, jh):
                """Pass-1 variant: transforms on DVE, freq-major across both
                k planes and split per image-pair jh, so conv2's first freq
                GEMM unblocks right after that pair's conv1 drains."""
                nn = 2 * npairs
                for j4 in range(4):
                    for k in range(K2):
                        hv = (h1[k][:, 0:nn * PS]
                              .rearrange("c (n a b) -> c n a b",
                                         a=PAD, b=PAD)
                              [:, 2 * jh:2 * jh + 2])
                        vv = (vt[k][:, 0:nn * 16 * 4 * 7]
                              .rearrange("c (n r j t) -> c n r j t",
                                         n=nn, r=16, j=4)
                              [:, 2 * jh:2 * jh + 2])

                        def dsel(c0, par):
                            return (hv[:, :, :, c0:c0 + 14]
                                    .rearrange("c n r (t two) -> c n r t two",
                                               two=2)[:, :, :, :, par])

                        pairs = {0: (dsel(0, 0), dsel(2, 0), Alu.subtract),
                                 1: (dsel(0, 1), dsel(2, 0), Alu.add),
                                 2: (dsel(2, 0), dsel(0, 1), Alu.subtract),
                                 3: (dsel(0, 1), dsel(2, 1), Alu.subtract)}
                        a, b, op = pairs[j4]
                        nc.vector.tensor_tensor(vv[:, :, :, j4, :], a, b, op)

            def conv1_chunks(pi, npairs, xt, h1, vt):
                """Returns a list of emitter callables (2 chunks) for this
                pass's conv1; each chunk is one PSUM tile's worth."""
                def emit_pair(m):
                    # pair (j0,j1) per m: PSUM halves, one drain per m
                    ps = pspool.tile([128, 2 * HB], F32, tag="psp",
                                     name=f"c1ps{m}")
                    for j in range(2):
                        for k in range(K1):
                            nc.tensor.matmul(
                                ps[:, j * HB:j * HB + N],
                                w1ap(k, m), xcol(xt, k, j),
                                start=(k == 0), stop=(k == K1 - 1))
                    src = ps[:].rearrange("c (j b) -> c j b", j=2)[:, :, 0:N]
                    dst = (h1[m][:]
                           .rearrange("c (n a b) -> c n a b", a=PAD, b=PAD)
                           [:, 0:4, 1:1 + HW, 1:1 + HW])
                    pointwise(mode1, src, dst, SC1, SH1, m)

                def emit_single(j, m):
                    ps = pspool.tile([128, 2 * HB], F32, tag="psp",
                                     name=f"c1ps{j}_{m}")
                    for k in range(K1):
                        nc.tensor.matmul(
                            ps[:, 0:N], w1ap(k, m), xcol(xt, k, j),
                            start=(k == 0), stop=(k == K1 - 1))
                    dst = (h1[m][:]
                           .rearrange("c (n a b) -> c n a b", a=PAD, b=PAD)
                           [:, 2 * j:2 * j + 2, 1:1 + HW, 1:1 + HW])
                    pointwise(mode1, ps[:, 0:N], dst, SC1, SH1, m)

                def emit_j0_khalves():
                    # startup: both m groups in one tile, k-halves
                    # interleaved, so matmuls start on the first half-x DMA
                    ps = pspool.tile([128, 2 * HB], F32, tag="psp",
                                     name="c1ps_j0")
                    for khalf in range(2):
                        for m in range(M1):
                            for k in range(4 * khalf, 4 * khalf + 4):
                                nc.tensor.matmul(
                                    ps[:, m * HB:m * HB + N],
                                    w1ap(k, m), xcol(xt, k, 0),
                                    start=(k == 0), stop=(k == K1 - 1),
                                    skip_group_check=True)
                    for m in range(M1):
                        dst = (h1[m][:]
                               .rearrange("c (n a b) -> c n a b",
                                          a=PAD, b=PAD)
                               [:, 0:2, 1:1 + HW, 1:1 + HW])
                        pointwise(mode1, ps[:, m * HB:m * HB + N],
                                  dst, SC1, SH1, m)

                if npairs == 2 and pi == 0:
                    # j-outer so conv1(j0) never waits on the xj1 DMA (a
                    # long stall would also reset the PE p-state clock)
                    def chunk0():
                        emit_j0_khalves()
                        emit_vtf_dve_jmajor(h1, vt, npairs, 0)

                    def chunk1():
                        for m in range(M1):
                            emit_single(1, m)
                        emit_vtf_dve_jmajor(h1, vt, npairs, 1)
                    return [chunk0, chunk1]
                if npairs == 2:
                    def mk(m):
                        def c():
                            emit_pair(m)
                            emit_vtf(m, h1[m], vt[m], npairs)
                        return c
                    return [mk(m) for m in range(M1)]

                def mk1(m):
                    def c():
                        emit_single(0, m)
                        emit_vtf(m, h1[m], vt[m], npairs)
                    return c
                return [mk1(m) for m in range(M1)]

            def emit_passes():
              plan = [(0, 2), (2, 2), (4, 2), (6, 2)]
              # prefetch: emit pass p+1's x loads at the START of pass p so
              # they sit ahead of pass p's store DMAs in SP queue order
              xt_next = xt0
              h1_next = None
              for pi, (q0, npairs) in enumerate(plan):
                xt = xt_next
                if pi + 1 < len(plan):
                    nq0, nnp = plan[pi + 1]
                    xt_next = [load_xj(nq0 + j, j) for j in range(nnp)]

                if pi == 0:
                    h1, vt = alloc_h1(npairs)
                    for c in conv1_chunks(pi, npairs, xt, h1, vt):
                        c()
                else:
                    h1, vt = h1_next  # conv1 emitted inside pass pi-1

                # h2 per k-plane (conv3's k0 matmuls then don't wait on the
                # k1 plane's drain chain)
                h2l = [h2pool.tile([128, MAXBP * S], BF, tag=f"h2_{k}",
                                   name=f"h2t{k}")
                       for k in range(K2)]

                def h2ap(k, j, nj=1):
                    return h2l[k][:, j * N:(j + nj) * N]

                # conv1 for pass pi+1 is emitted interleaved into this
                # pass's conv3 (software pipelining: PE fills ACT's drain
                # lag with conv1 matmuls whose pointwise runs on DVE)
                if pi + 1 < len(plan):
                    h1_next = alloc_h1(plan[pi + 1][1])
                    next_chunks = conv1_chunks(pi + 1, plan[pi + 1][1],
                                               xt_next, *h1_next)
                else:
                    next_chunks = []

                # conv2: 3x3 pad 1 via 1D-winograd F(2,3) along width:
                # per (pair, m): 4 freq GEMMs (N=196) accumulating over
                # (kh, kt), then the A^T output transform on DVE and the
                # kervolution square on ACT
                nn = 2 * npairs
                vv = [vt[k][:, 0:nn * 16 * 4 * 7]
                      .rearrange("c (n r j t) -> c n r j t", n=nn, r=16, j=4)
                      for k in range(K2)]
                for jp in range(npairs):
                    for m in range(M1):
                        ps = pspool.tile([128, 2 * HB], F32, tag="psp")
                        for j4 in range(4):
                            off = (j4 // 2) * HB + (j4 % 2) * S
                            first = True
                            for kh in range(3):
                                for kt in range(K2):
                                    rhs = vv[kt][:, 2 * jp:2 * jp + 2,
                                                 kh:kh + HW, j4, :]
                                    nc.tensor.matmul(
                                        ps[:, off:off + S],
                                        w2ap(kt, kh, j4, m), rhs,
                                        start=first,
                                        stop=(kh == 2 and kt == K2 - 1))
                                    first = False
                        # output transform: even = M0+M1+M2, odd = M1-M2-M3
                        yv = (h2ap(m, jp)
                              .rearrange("c (n a b) -> c n a b", a=HW, b=HW))

                        def ysel(par):
                            return (yv.rearrange(
                                "c n a (t two) -> c n a t two", two=2)
                                [:, :, :, :, par])

                        def msel(j4):
                            off = (j4 // 2) * HB + (j4 % 2) * S
                            return (ps[:, off:off + S]
                                    .rearrange("c (n a t) -> c n a t",
                                               n=2, a=HW))

                        ye, yo = ysel(0), ysel(1)
                        # stage M1 to SBUF via ACT Copy, M2 via DVE (a
                        # TensorTensor may read at most one PSUM operand;
                        # splitting the staging balances the two engines —
                        # copy+square share every act table, so no reload)
                        s12 = tpool.tile([128, 2 * S], BF, tag="tq",
                                         name="s12")
                        sv = s12[:].rearrange("c (g n a t) -> c g n a t",
                                              g=2, n=2, a=HW)
                        Cp = mybir.ActivationFunctionType.Copy
                        nc.scalar.activation(sv[:, 0], msel(1), Cp)
                        nc.vector.tensor_scalar(sv[:, 1], msel(2), 1.0,
                                                None, Alu.mult)
                        nc.vector.tensor_tensor(ye, msel(0), sv[:, 0],
                                                Alu.add)
                        nc.vector.tensor_tensor(yo, sv[:, 0], sv[:, 1],
                                                Alu.subtract)
                        nc.vector.tensor_tensor(yo, yo, msel(3),
                                                Alu.subtract)
                        nc.vector.tensor_tensor(ye, ye, sv[:, 1], Alu.add)
                        # kervolution square, in place on the h2 slice
                        ph = h2ap(m, jp)
                        if mode2[0] == SLOW:
                            nc.scalar.activation(ph, ph, Sq, bias=1.0,
                                                 scale=1.0)
                            nc.vector.tensor_scalar(
                                ph, ph, scb[:, SC2 + m:SC2 + m + 1],
                                shb[:, SH2 + m:SH2 + m + 1],
                                Alu.mult, Alu.add)
                        else:
                            nc.scalar.activation(
                                ph, ph, Sq,
                                bias=scb[:, SC2 + M1 + m:SC2 + M1 + m + 1],
                                scale=scb[:, SC2 + m:SC2 + m + 1])
                            if mode2[0] == FAST_T:
                                nc.vector.tensor_scalar(
                                    ph, ph, shb[:, SH2 + m:SH2 + m + 1],
                                    None, Alu.add)

                # conv3: 1x1, C_MID -> C_IN, (2mp, 2mp+1) paired per j when
                # uniform; + residual, store per (m-pair, j)
                pair3 = mode3[1] is not None
                # interleave next-pass conv1 chunks where ring/drain waits
                # would otherwise stall PE: before group 1 (covers the
                # conv2-j1/c2 drain latency) and mid-phase
                if npairs == 2:
                    c1_at = {0: 0, 4: 1}
                else:
                    c1_at = {0: 0, 2: 1}
                gidx = 0
                if pi == len(plan) - 1 and npairs == 2:
                    # last pass j-outer: j0's whole drain/store chain
                    # completes under j1's matmuls, halving the tail
                    order = [(mp, j) for j in range(npairs)
                             for mp in range(M3 // 2)]
                else:
                    order = [(mp, j) for mp in range(M3 // 2)
                             for j in range(npairs)]
                for mp, j in order:
                    if True:
                        if gidx in c1_at and c1_at[gidx] < len(next_chunks):
                            next_chunks[c1_at[gidx]]()
                        gidx += 1
                        ps = pspool.tile([128, 2 * HB], F32, tag="psp")
                        for mi in range(2):
                            m = 2 * mp + mi
                            for k in range(K2):
                                nc.tensor.matmul(
                                    ps[:, mi * HB:mi * HB + N],
                                    w3ap(k, m), h2ap(k, j),
                                    start=(k == 0), stop=(k == K2 - 1))
                        zt = opool.tile([128, 2 * N], BF, tag="z")
                        last_pass = pi == len(plan) - 1
                        if (last_pass and pair3 and mode3[0] == FAST_T0
                                and j == npairs - 1 and mp == 1):
                            # tail relief: one of the final drains on DVE so
                            # ACT's serial end-of-kernel train is shorter
                            tq3 = tpool.tile([128, 2 * N], BF, tag="tq")
                            tv3 = tq3[:].rearrange("c (m b) -> c m b", b=N)
                            src = (ps[:].rearrange("c (m b) -> c m b", m=2)
                                   [:, :, 0:N])
                            nc.vector.tensor_scalar(
                                tv3, src,
                                scb[:, SC3 + 2 * mp:SC3 + 2 * mp + 1],
                                scb[:, SC3 + 2 * mp:SC3 + 2 * mp + 1],
                                Alu.mult, Alu.add)
                            nc.vector.tensor_tensor(zt[:], tq3[:], tq3[:],
                                                    Alu.mult)
                        elif (last_pass and mp == M3 // 2 - 1
                                and j == npairs - 1):
                            # final group: single-m drains + residuals (the
                            # chain after the very last matmul halves), but
                            # ONE paired store (two stores would serialize
                            # their HWDGE generations + DGE delays)
                            for mi in range(2):
                                pointwise(mode3, ps[:, mi * HB:mi * HB + N],
                                          zt[:, mi * N:(mi + 1) * N],
                                          SC3, SH3, 2 * mp + mi)
                                zvi = (zt[:, mi * N:(mi + 1) * N]
                                       .rearrange("c (n s) -> c n s", n=2))
                                xvi = (xt[j][:]
                                       .rearrange("p (k n s) -> p k n s",
                                                  k=K1, n=2)
                                       [:, 2 * mp + mi, :, :])
                                nc.vector.tensor_tensor(zvi, zvi, xvi,
                                                        Alu.add)
                            zv = zt[:].rearrange("c (m n s) -> c m n s",
                                                 m=2, n=2)
                            dst = out_cm[:, 2 * mp:2 * mp + 2,
                                         2 * (q0 + j):2 * (q0 + j) + 2, :]
                            nc.sync.dma_start(dst, zv)
                            continue
                        elif pair3:
                            src = (ps[:].rearrange("c (m b) -> c m b", m=2)
                                   [:, :, 0:N])
                            pointwise(mode3, src, zt[:], SC3, SH3, 2 * mp)
                        else:
                            for mi in range(2):
                                pointwise(mode3, ps[:, mi * HB:mi * HB + N],
                                          zt[:, mi * N:(mi + 1) * N],
                                          SC3, SH3, 2 * mp + mi)
                        zv = zt[:].rearrange("c (m n s) -> c m n s",
                                             m=2, n=2)
                        xv = (xt[j][:].rearrange("p (k n s) -> p k n s",
                                                 k=K1, n=2)
                              [:, 2 * mp:2 * mp + 2, :, :])
                        nc.vector.tensor_tensor(zv, zv, xv, Alu.add)
                        dst = out_cm[:, 2 * mp:2 * mp + 2,
                                     2 * (q0 + j):2 * (q0 + j) + 2, :]
                        nc.sync.dma_start(dst, zv)

            if reps is None:
                emit_passes()
            else:
                with tc.For_i(0, reps, 1):
                    emit_passes()

    nc.compile()
    return nc


# ---------------- host side ----------------

_CACHE = {}


def _get_runner(modes):
    if modes in _CACHE:
        return _CACHE[modes]
    import jax
    from jax.experimental.shard_map import shard_map
    from jax.sharding import Mesh, PartitionSpec
    from concourse.bass2jax import (_bass_exec_p, install_neuronx_cc_hook,
                                    partition_id_tensor)

    nc = _build(modes)
    install_neuronx_cc_hook()
    partition_name = nc.partition_id_tensor.name if nc.partition_id_tensor else None
    in_names, out_names, out_avals = [], [], []
    for alloc in nc.m.functions[0].allocations:
        if not isinstance(alloc, mybir.MemoryLocationSet):
            continue
        name = alloc.memorylocations[0].name
        if alloc.kind == "ExternalInput":
            if name != partition_name:
                in_names.append(name)
        elif alloc.kind == "ExternalOutput":
            out_names.append(name)
            out_avals.append(jax.core.ShapedArray(
                tuple(alloc.tensor_shape), mybir.dt.np(alloc.dtype)))
    n_params, n_outs = len(in_names), len(out_avals)
    all_in_names = list(in_names) + list(out_names)
    if partition_name is not None:
        all_in_names.append(partition_name)

    def _body(*args):
        operands = list(args)
        if partition_name is not None:
            operands.append(partition_id_tensor())
        outs = _bass_exec_p.bind(
            *operands,
            out_avals=tuple(out_avals),
            in_names=tuple(all_in_names),
            out_names=tuple(out_names),
            lowering_input_output_aliases=(),
            sim_require_finite=True,
            sim_require_nnan=True,
            nc=nc,
        )
        return tuple(outs)

    devices = jax.devices()[:8]
    mesh = Mesh(np.asarray(devices), ("core",))
    sharded = jax.jit(
        shard_map(_body, mesh=mesh,
                  in_specs=(PartitionSpec("core"),) * (n_params + n_outs),
                  out_specs=(PartitionSpec("core"),) * n_outs,
                  check_rep=False),
        donate_argnums=tuple(range(n_params, n_params + n_outs)),
        keep_unused=True,
    )
    sharding = jax.sharding.NamedSharding(mesh, PartitionSpec("core"))
    runner = dict(nc=nc, sharded=sharded, sharding=sharding, jax=jax,
                  in_names=in_names, out_names=out_names, out_avals=out_avals)
    _CACHE[modes] = runner
    return runner


def _vec_tile(v, m_tiles):
    """[C] -> [128, m_tiles] column-per-m-tile layout."""
    return np.ascontiguousarray(np.asarray(v).reshape(m_tiles, 128).T
                                .astype(np.float32))


def _bf16(a):
    import ml_dtypes
    return np.asarray(a, dtype=np.float32).astype(ml_dtypes.bfloat16)


def prepare(w1, w2, w3, g1, b1, m1, v1, g2, b2, m2, v2, g3, b3, m3, v3):
    """Host prep: returns (modes, shared_input_dict_without_x)."""
    s1 = g1 / np.sqrt(v1 + EPS)
    t1 = b1 - m1 * s1
    s2 = g2 / np.sqrt(v2 + EPS)
    t2 = b2 - m2 * s2
    s3 = g3 / np.sqrt(v3 + EPS)
    t3 = b3 - m3 * s3

    def mode_of(s, t):
        """Returns (mode, const): const = sqrt(s) as a float when s is
        exactly channel-uniform and shifts are zero (enables m-paired
        PSUM drains on ACT), else None."""
        if np.all(s > 0):
            if not np.any(t):
                r = np.sqrt(s)
                const = float(r[0]) if np.all(s == s[0]) else None
                return (FAST_T0, const)
            return (FAST_T, None)
        return (SLOW, None)

    modes = (mode_of(s1, t1), mode_of(s2, t2), mode_of(s3, t3))

    def sc_bi(lmode, s, m_tiles):
        mode = lmode[0]
        if mode == SLOW:
            return _vec_tile(s, m_tiles), np.ones((128, m_tiles), np.float32)
        r = np.sqrt(s)
        return _vec_tile(r, m_tiles), _vec_tile(r, m_tiles)

    sc1, bi1 = sc_bi(modes[0], s1, M1)
    sc2, bi2 = sc_bi(modes[1], s2, M1)
    sc3, bi3 = sc_bi(modes[2], s3, M3)
    scb = np.concatenate([sc1, bi1, sc2, bi2, sc3, bi3], axis=1)
    shb = np.concatenate([_vec_tile(t1, M1), _vec_tile(t2, M1),
                          _vec_tile(t3, M3)], axis=1)

    w1t = _bf16(np.ascontiguousarray(w1[:, :, 0, 0].T))          # [1024,256]
    # w2: [o, i, kh, kw] -> 1D-winograd F(2,3) along kw: U_j = G @ w[kw]
    G = np.array([[1, 0, 0], [.5, .5, .5], [.5, -.5, .5], [0, 0, 1]],
                 np.float64)
    U = np.einsum('jw,oihw->oihj', G, w2.astype(np.float64))  # [o,i,kh,j]
    w2t = _bf16(np.ascontiguousarray(
        U.transpose(1, 2, 3, 0)                   # [i, kh, j, o]
          .reshape(K2, 128, 3, 4, C_MID)          # [kt, i128, kh, j, o]
          .transpose(3, 0, 2, 1, 4)))             # [j, kt, kh, i128, o]
    w3t = _bf16(np.ascontiguousarray(w3[:, :, 0, 0].T))          # [256,1024]

    shared = dict(w1t=w1t, w2t=w2t, w3t=w3t, scb=scb, shb=shb)
    return modes, shared


def kernel(**inputs):
    inputs = {k: np.asarray(v) for k, v in inputs.items()}
    x = inputs.pop("x").astype(np.float32)
    modes, shared = prepare(**inputs)
    r = _get_runner(modes)
    jax = r["jax"]

    n_cores = 8
    # x: [128, 1024, 14, 14] -> [core(8) x k(8), 128, 16, 196] bf16 channel-major
    x_cm = _bf16(x.reshape(8, B, K1, 128, S)
                 .transpose(0, 2, 3, 1, 4)
                 .reshape(n_cores * K1, 128, B, S))
    dev_in = []
    for name in r["in_names"]:
        if name == "x":
            cat = x_cm
        else:
            a = shared[name]
            cat = np.concatenate([a] * n_cores, axis=0)
        dev_in.append(jax.device_put(cat, r["sharding"]))
    zero_outs = [
        jax.device_put(np.zeros((n_cores * av.shape[0], *av.shape[1:]), av.dtype),
                       r["sharding"])
        for av in r["out_avals"]
    ]
    outs = r["sharded"](*dev_in, *zero_outs)
    jax.block_until_ready(outs)
    out = np.asarray(outs[r["out_names"].index("out")])
    # [core x m(8), 128, 16, 196] bf16 -> [128, 1024, 14, 14] f32
    return np.ascontiguousarray(
        out.reshape(n_cores, M3, 128, B, S)
           .transpose(0, 3, 1, 2, 4)
           .reshape(128, C_IN, HW, HW)).astype(np.float32)


# revision 83
# speedup vs baseline: 1.1823x; 1.0339x over previous
"""TRN2 Bass kernel for nn_Block_6476810682806 (dense_cnn).

Bottleneck block: 1x1 kerv -> BN -> 3x3 kerv -> BN -> 1x1 kerv -> BN -> +residual,
where kerv(x) = (conv(x) + 1)^2 and BN is inference-mode (frozen stats).

Distribution: data-parallel over batch (128 -> 16 per core) across 8 cores,
weights replicated. Each core computes its shard fully independently.

Device strategy (per core):
  - everything that crosses DMA is bf16 (halves HBM traffic; norm_rel ~5.6e-3
    vs the 2e-2 gate). Host pre-transposes x/out to channel-major so bf16
    descriptor runs stay >=512B (smaller runs pay a 2x DMA latency penalty).
  - activations channel-major: [C partitions, batch*spatial free]
  - convs as PE matmuls in bf16 (1 cyc/row at any N), f32 PSUM accumulate
  - 3x3 conv: 9 shifted matmuls over a zero-padded per-image 16x16 SBUF plane
  - BN scale folded into the kervolution square on ACT:
        s*(y+1)^2 = (sqrt(s)*y + sqrt(s))^2  (requires s > 0)
    shifts (t = b - m*s) are zero for this problem's fills; generic paths
    emit an extra per-channel add / affine when they are not.
  - residual add on DVE, straight from the resident x tiles (all-bf16 SBUF
    operands hit the DVE 2x/4x fast modes)
  - pass plan 4x4 images; the last pass drains conv3 j-outer and splits
    its final drains (ACT/DVE) to shorten the end-of-kernel chain
"""

import numpy as np

import concourse.bacc as bacc
import concourse.mybir as mybir
import concourse.tile as tile

F32 = mybir.dt.float32
BF = mybir.dt.bfloat16
EPS = 1e-5

B = 16          # images per core
C_IN = 1024
C_MID = 256
HW = 14
S = HW * HW     # 196
N = 2 * S       # matmul moving size per image pair = 392
PAD = 16        # padded plane side
PS = PAD * PAD  # 256 padded plane size
K1 = C_IN // 128          # 8
K2 = C_MID // 128         # 2
M1 = C_MID // 128         # 2
M3 = C_IN // 128          # 8
MAXBP = 4                 # max images per pass

# layer modes
FAST_T0 = 0   # all s>0, all t==0: ACT-only pointwise
FAST_T = 1    # all s>0, some t!=0: ACT + per-channel add
SLOW = 2      # some s<=0: plain square on ACT + DVE affine

# packed scale/bias column offsets in scb [128, 24]
SC1, BI1, SC2, BI2, SC3, BI3 = 0, 2, 4, 6, 8, 16
# packed shift column offsets in shb [128, 12]
SH1, SH2, SH3 = 0, 2, 4


def _build(modes, reps=None):
    mode1, mode2, mode3 = modes
    nc = bacc.Bacc("TRN2", target_bir_lowering=False, debug=False)

    x_d = nc.dram_tensor("x", [K1, 128, B, S], BF, kind="ExternalInput").ap()
    w1_d = nc.dram_tensor("w1t", [C_IN, C_MID], BF, kind="ExternalInput").ap()
    w2_d = nc.dram_tensor("w2t", [4, K2, 3, 128, C_MID], BF,
                          kind="ExternalInput").ap()
    w3_d = nc.dram_tensor("w3t", [C_MID, C_IN], BF, kind="ExternalInput").ap()
    scb_d = nc.dram_tensor("scb", [128, 24], F32, kind="ExternalInput").ap()
    shb_d = nc.dram_tensor("shb", [128, 12], F32, kind="ExternalInput").ap()
    out_d = nc.dram_tensor("out", [M3, 128, B, S], BF, kind="ExternalOutput").ap()

    x_cm = x_d.rearrange("k p n s -> p k n s")     # [128, 8, 16, 196]
    out_cm = out_d.rearrange("m p n s -> p m n s")  # [128, 8, 16, 196]

    Sq = mybir.ActivationFunctionType.Square
    Alu = mybir.AluOpType

    with tile.TileContext(nc) as tc:
        with (
            tc.tile_pool(name="wpool", bufs=1) as wpool,
            tc.tile_pool(name="xpool", bufs=4) as xpool,
            tc.tile_pool(name="h1pool", bufs=2) as h1pool,
            tc.tile_pool(name="h2pool", bufs=3) as h2pool,
            tc.tile_pool(name="tpool", bufs=4) as tpool,
            tc.tile_pool(name="vpool", bufs=2) as vpool,
            tc.tile_pool(name="opool", bufs=6) as opool,
            tc.tile_pool(name="psp", bufs=4, space="PSUM") as pspool,
        ):
            # every PSUM tile is 2 banks; accumulation groups go to the
            # bank-aligned halves [0:N] and [HB:HB+N], drained by ONE
            # strided ACT op (halves the per-op init overhead share)
            HB = 512
            def xcol(xh, k, j):
                # [128, 2, S] rhs slice for k-tile k, image pair j
                v = xh[j][:].rearrange("p (k n s) -> p k n s", k=K1, n=2)
                return v[:, k, :, :]

            # ---- startup: one serialized DMA stream (SP queue) in first-use
            # order: xj0, scale vec, w1, w2, xj1, w3 ----
            def load_xj(pair, j):
                # pair: global image-pair index 0..7; j: slot parity in pass
                t = xpool.tile([128, K1 * 2 * S], BF, tag=f"x{j}",
                               name=f"xt_q{pair}")
                nc.sync.dma_start(
                    t[:].rearrange("p (k n s) -> p k n s", k=K1, n=2),
                    x_cm[:, :, 2 * pair:2 * pair + 2, :])
                return t

            # first x pair split into k-halves so conv1's first matmuls can
            # start ~1us sooner (w1 slots between the halves)
            xj0 = xpool.tile([128, K1 * 2 * S], BF, tag="x0", name="xt_q0")
            xj0v = xj0[:].rearrange("p (k n s) -> p k n s", k=K1, n=2)
            nc.sync.dma_start(xj0v[:, 0:K1 // 2], x_cm[:, 0:K1 // 2, 0:2, :])
            w1view = w1_d.rearrange("(k p) o -> p k o", p=128)
            w1s = wpool.tile([128, K1 * C_MID], BF, tag="w1s")
            w1v = w1s[:].rearrange("p (k o) -> p k o", k=K1)
            nc.sync.dma_start(w1v[:, 0:K1 // 2], w1view[:, 0:K1 // 2])
            nc.sync.dma_start(xj0v[:, K1 // 2:], x_cm[:, K1 // 2:, 0:2, :])
            nc.sync.dma_start(w1v[:, K1 // 2:], w1view[:, K1 // 2:])
            scb = wpool.tile([128, 24], F32, tag="scb")
            nc.sync.dma_start(scb[:], scb_d)
            if any(mo[0] != FAST_T0 for mo in modes):
                shb = wpool.tile([128, 12], F32, tag="shb")
                nc.sync.dma_start(shb[:], shb_d)
            else:
                shb = None
            xj1 = load_xj(1, 1)
            xt0 = [xj0, xj1]
            # w2 is freq-major and DMA'd in per-freq quarters: pass-1 conv2's
            # j4-th freq GEMM starts as soon as its quarter lands
            w2view = w2_d.rearrange("j k h p o -> p (j k h) o")
            w2s = wpool.tile([128, 24 * C_MID], BF, tag="w2s")
            w2v = w2s[:].rearrange("p (kt o) -> p kt o", kt=24)
            for q4 in range(4):
                nc.sync.dma_start(w2v[:, 6 * q4:6 * (q4 + 1)],
                                  w2view[:, 6 * q4:6 * (q4 + 1)])
            w3s = wpool.tile([128, K2 * C_IN], BF, tag="w3s")
            nc.sync.dma_start(
                w3s[:].rearrange("p (k o) -> p k o", k=K2),
                w3_d.rearrange("(k p) o -> p k o", p=128))

            def w1ap(k, m):
                return w1s[:, k * C_MID + m * 128: k * C_MID + (m + 1) * 128]

            def w2ap(kt, kh, j4, m):
                base = ((j4 * K2 + kt) * 3 + kh) * C_MID + m * 128
                return w2s[:, base: base + 128]

            def w3ap(k, m):
                return w3s[:, k * C_IN + m * 128: k * C_IN + (m + 1) * 128]

            def pointwise(lmode, src_ap, out_ap, sc_off, sh_off, m):
                """out = s*(src+1)^2 + t, written to out_ap.

                lmode is (mode, const): const is sqrt(s) as a python float
                when s is channel-uniform (allows m-paired drains), else
                None (per-channel scb column; src must be single-m)."""
                mode, const = lmode
                if mode == SLOW:
                    nc.scalar.activation(out_ap, src_ap, Sq, bias=1.0, scale=1.0)
                    nc.vector.tensor_scalar(
                        out_ap, out_ap, scb[:, sc_off + m:sc_off + m + 1],
                        shb[:, sh_off + m:sh_off + m + 1], Alu.mult, Alu.add)
                else:
                    # for m-paired drains (const flag set) the scale is
                    # channel-uniform, so the first m's column is valid for
                    # the whole pair
                    nc.scalar.activation(
                        out_ap, src_ap, Sq,
                        bias=scb[:, sc_off + (M1 if sc_off < SC3 else M3) + m:
                                 sc_off + (M1 if sc_off < SC3 else M3) + m + 1],
                        scale=scb[:, sc_off + m:sc_off + m + 1])
                    if mode == FAST_T:
                        nc.vector.tensor_scalar(
                            out_ap, out_ap, shb[:, sh_off + m:sh_off + m + 1],
                            None, Alu.add)

            def pointwise_dve(src_ap, dst_ap, sc_off, m, nelem):
                """conv1 pointwise on DVE (FAST_T0 only): frees the ACT
                queue for conv3 drains at pass boundaries.
                t = sqrt(s)*y + sqrt(s); dst = t*t."""
                tq = tpool.tile([128, 2 * N], BF, tag="tq")
                tv = tq[:, 0:nelem]
                if nelem > N:
                    tv = tv.rearrange("c (j b) -> c j b", b=N)
                nc.vector.tensor_scalar(
                    tv, src_ap, scb[:, sc_off + m:sc_off + m + 1],
                    scb[:, sc_off + m:sc_off + m + 1], Alu.mult, Alu.add)
                tsq = (tq[:, 0:nelem]
                       .rearrange("c (n a b) -> c n a b", a=HW, b=HW))
                nc.vector.tensor_tensor(dst_ap, tsq, tsq, Alu.mult)

            # ---- PE warmup: dummy matmuls on scratch data keep the PE
            # clock ramping while the startup DMAs land; the early dummy
            # activation pulls the act-table load off the critical path ----
            wu = wpool.tile([128, 128], BF, tag="wu")
            nc.vector.memset(wu[:].bitcast(F32), 0.0)
            wusc = wpool.tile([128, 4], F32, tag="wusc")
            # act-table preload reads SBUF (reading the warmup PSUM tile
            # would WAR-serialize every warmup matmul behind the 1.3us
            # LoadActFuncSet)
            nc.scalar.activation(wusc[:], wu[:].bitcast(F32)[:, 0:4], Sq,
                                 bias=1.0, scale=1.0)
            wups = pspool.tile([128, 2 * HB], F32, tag="psp", name="wups")
            NWU = 120
            for i in range(NWU):
                nc.tensor.matmul(wups[:, 0:64], wu[:], wu[:, 0:64],
                                 start=(i == 0), stop=(i == NWU - 1))

            # ---- main passes: (first image pair index, n pairs) ----
            def alloc_h1(npairs):
                h1, vt = [], []
                for k in range(K2):
                    t = h1pool.tile([128, MAXBP * PS], BF, tag=f"h1_{k}",
                                    name=f"h1t{k}")
                    nc.gpsimd.memset(t[:, 0:2 * npairs * PS].bitcast(F32),
                                     0.0)
                    h1.append(t)
                    v = vpool.tile([128, MAXBP * 16 * 4 * 7], BF,
                                   tag=f"v_{k}", name=f"vt{k}")
                    vt.append(v)
                return h1, vt

            def emit_vtf(k, h1t, vtt, npairs):
                """1D-winograd input transform for one k-plane on GPSIMD:
                V0=d0-d2 V1=d1+d2 V2=d2-d1 V3=d1-d3 over width pairs."""
                nn = 2 * npairs
                hv = (h1t[:, 0:nn * PS]
                      .rearrange("c (n a b) -> c n a b", a=PAD, b=PAD))
                vv = (vtt[:, 0:nn * 16 * 4 * 7]
                      .rearrange("c (n r j t) -> c n r j t", n=nn, r=16, j=4))

                def dsel(c0, par):
                    # pad cols c0+2t+par for t=0..6 (stride-2 column pick)
                    return (hv[:, :, :, c0:c0 + 14]
                            .rearrange("c n r (t two) -> c n r t two", two=2)
                            [:, :, :, :, par])

                d0 = dsel(0, 0)
                d1 = dsel(0, 1)
                d2 = dsel(2, 0)
                d3 = dsel(2, 1)
                # GPSIMD runs the transform serially (~0.9us/op); conv2's
                # freq GEMMs catch up to the SECOND k-tile's last two freq
                # planes, so those go to DVE instead
                eng2 = nc.vector if k == K2 - 1 else nc.gpsimd
                nc.gpsimd.tensor_tensor(vv[:, :, :, 0, :], d0, d2,
                                        Alu.subtract)
                nc.gpsimd.tensor_tensor(vv[:, :, :, 1, :], d1, d2, Alu.add)
                eng2.tensor_tensor(vv[:, :, :, 2, :], d2, d1, Alu.subtract)
                eng2.tensor_tensor(vv[:, :, :, 3, :], d1, d3, Alu.subtract)

            def emit_vtf_dve_jmajor(h1, vt, npairs, jh):
                """Pass-1 variant: transforms on DVE, freq-major across both
                k planes and split per image-pair jh, so conv2's first freq
                GEMM unblocks right after that pair's conv1 drains."""
                nn = 2 * npairs
                for j4 in range(4):
                    for k in range(K2):
                        hv = (h1[k][:, 0:nn * PS]
                              .rearrange("c (n a b) -> c n a b",
                                         a=PAD, b=PAD)
                              [:, 2 * jh:2 * jh + 2])
                        vv = (vt[k][:, 0:nn * 16 * 4 * 7]
                              .rearrange("c (n r j t) -> c n r j t",
                                         n=nn, r=16, j=4)
                              [:, 2 * jh:2 * jh + 2])

                        def dsel(c0, par):
                            return (hv[:, :, :, c0:c0 + 14]
                                    .rearrange("c n r (t two) -> c n r t two",
                                               two=2)[:, :, :, :, par])

                        pairs = {0: (dsel(0, 0), dsel(2, 0), Alu.subtract),
                                 1: (dsel(0, 1), dsel(2, 0), Alu.add),
                                 2: (dsel(2, 0), dsel(0, 1), Alu.subtract),
                                 3: (dsel(0, 1), dsel(2, 1), Alu.subtract)}
                        a, b, op = pairs[j4]
                        nc.vector.tensor_tensor(vv[:, :, :, j4, :], a, b, op)

            def conv1_chunks(pi, npairs, xt, h1, vt):
                """Returns a list of emitter callables (2 chunks) for this
                pass's conv1; each chunk is one PSUM tile's worth."""
                def emit_pair(m):
                    # pair (j0,j1) per m: PSUM halves, one drain per m
                    ps = pspool.tile([128, 2 * HB], F32, tag="psp",
                                     name=f"c1ps{m}")
                    for j in range(2):
                        for k in range(K1):
                            nc.tensor.matmul(
                                ps[:, j * HB:j * HB + N],
                                w1ap(k, m), xcol(xt, k, j),
                                start=(k == 0), stop=(k == K1 - 1))
                    src = ps[:].rearrange("c (j b) -> c j b", j=2)[:, :, 0:N]
                    dst = (h1[m][:]
                           .rearrange("c (n a b) -> c n a b", a=PAD, b=PAD)
                           [:, 0:4, 1:1 + HW, 1:1 + HW])
                    pointwise(mode1, src, dst, SC1, SH1, m)

                def emit_single(j, m):
                    ps = pspool.tile([128, 2 * HB], F32, tag="psp",
                                     name=f"c1ps{j}_{m}")
                    for k in range(K1):
                        nc.tensor.matmul(
                            ps[:, 0:N], w1ap(k, m), xcol(xt, k, j),
                            start=(k == 0), stop=(k == K1 - 1))
                    dst = (h1[m][:]
                           .rearrange("c (n a b) -> c n a b", a=PAD, b=PAD)
                           [:, 2 * j:2 * j + 2, 1:1 + HW, 1:1 + HW])
                    pointwise(mode1, ps[:, 0:N], dst, SC1, SH1, m)

                def emit_j0_khalves():
                    # startup: both m groups in one tile, k-halves
                    # interleaved, so matmuls start on the first half-x DMA
                    ps = pspool.tile([128, 2 * HB], F32, tag="psp",
                                     name="c1ps_j0")
                    for khalf in range(2):
                        for m in range(M1):
                            for k in range(4 * khalf, 4 * khalf + 4):
                                nc.tensor.matmul(
                                    ps[:, m * HB:m * HB + N],
                                    w1ap(k, m), xcol(xt, k, 0),
                                    start=(k == 0), stop=(k == K1 - 1),
                                    skip_group_check=True)
                    for m in range(M1):
                        dst = (h1[m][:]
                               .rearrange("c (n a b) -> c n a b",
                                          a=PAD, b=PAD)
                               [:, 0:2, 1:1 + HW, 1:1 + HW])
                        pointwise(mode1, ps[:, m * HB:m * HB + N],
                                  dst, SC1, SH1, m)

                if npairs == 2 and pi == 0:
                    # j-outer so conv1(j0) never waits on the xj1 DMA (a
                    # long stall would also reset the PE p-state clock)
                    def chunk0():
                        emit_j0_khalves()
                        emit_vtf_dve_jmajor(h1, vt, npairs, 0)

                    def chunk1():
                        for m in range(M1):
                            emit_single(1, m)
                        emit_vtf_dve_jmajor(h1, vt, npairs, 1)
                    return [chunk0, chunk1]
                if npairs == 2:
                    def mk(m):
                        def c():
                            emit_pair(m)
                            emit_vtf(m, h1[m], vt[m], npairs)
                        return c
                    return [mk(m) for m in range(M1)]

                def mk1(m):
                    def c():
                        emit_single(0, m)
                        emit_vtf(m, h1[m], vt[m], npairs)
                    return c
                return [mk1(m) for m in range(M1)]

            def emit_passes():
              plan = [(0, 2), (2, 2), (4, 2), (6, 2)]
              # prefetch: emit pass p+1's x loads at the START of pass p so
              # they sit ahead of pass p's store DMAs in SP queue order
              xt_next = xt0
              h1_next = None
              for pi, (q0, npairs) in enumerate(plan):
                xt = xt_next
                if pi + 1 < len(plan):
                    nq0, nnp = plan[pi + 1]
                    xt_next = [load_xj(nq0 + j, j) for j in range(nnp)]

                if pi == 0:
                    h1, vt = alloc_h1(npairs)
                    for c in conv1_chunks(pi, npairs, xt, h1, vt):
                        c()
                else:
                    h1, vt = h1_next  # conv1 emitted inside pass pi-1

                # h2 per k-plane (conv3's k0 matmuls then don't wait on the
                # k1 plane's drain chain)
                h2l = [h2pool.tile([128, MAXBP * S], BF, tag=f"h2_{k}",
                                   name=f"h2t{k}")
                       for k in range(K2)]

                def h2ap(k, j, nj=1):
                    return h2l[k][:, j * N:(j + nj) * N]

                # conv1 for pass pi+1 is emitted interleaved into this
                # pass's conv3 (software pipelining: PE fills ACT's drain
                # lag with conv1 matmuls whose pointwise runs on DVE)
                if pi + 1 < len(plan):
                    h1_next = alloc_h1(plan[pi + 1][1])
                    next_chunks = conv1_chunks(pi + 1, plan[pi + 1][1],
                                               xt_next, *h1_next)
                else:
                    next_chunks = []

                # conv2: 3x3 pad 1 via 1D-winograd F(2,3) along width:
                # per (pair, m): 4 freq GEMMs (N=196) accumulating over
                # (kh, kt), then the A^T output transform on DVE and the
                # kervolution square on ACT
                nn = 2 * npairs
                vv = [vt[k][:, 0:nn * 16 * 4 * 7]
                      .rearrange("c (n r j t) -> c n r j t", n=nn, r=16, j=4)
                      for k in range(K2)]
                for jp in range(npairs):
                    for m in range(M1):
                        ps = pspool.tile([128, 2 * HB], F32, tag="psp")
                        for j4 in range(4):
                            off = (j4 // 2) * HB + (j4 % 2) * S
                            first = True
                            for kh in range(3):
                                for kt in range(K2):
                                    rhs = vv[kt][:, 2 * jp:2 * jp + 2,
                                                 kh:kh + HW, j4, :]
                                    nc.tensor.matmul(
                                        ps[:, off:off + S],
                                        w2ap(kt, kh, j4, m), rhs,
                                        start=first,
                                        stop=(kh == 2 and kt == K2 - 1))
                                    first = False
                        # output transform: even = M0+M1+M2, odd = M1-M2-M3
                        yv = (h2ap(m, jp)
                              .rearrange("c (n a b) -> c n a b", a=HW, b=HW))

                        def ysel(par):
                            return (yv.rearrange(
                                "c n a (t two) -> c n a t two", two=2)
                                [:, :, :, :, par])

                        def msel(j4):
                            off = (j4 // 2) * HB + (j4 % 2) * S
                            return (ps[:, off:off + S]
                                    .rearrange("c (n a t) -> c n a t",
                                               n=2, a=HW))

                        ye, yo = ysel(0), ysel(1)
                        # stage M1 to SBUF via ACT Copy, M2 via DVE (a
                        # TensorTensor may read at most one PSUM operand;
                        # splitting the staging balances the two engines —
                        # copy+square share every act table, so no reload)
                        s12 = tpool.tile([128, 2 * S], BF, tag="tq",
                                         name="s12")
                        sv = s12[:].rearrange("c (g n a t) -> c g n a t",
                                              g=2, n=2, a=HW)
                        Cp = mybir.ActivationFunctionType.Copy
                        nc.scalar.activation(sv[:, 0], msel(1), Cp)
                        nc.vector.tensor_scalar(sv[:, 1], msel(2), 1.0,
                                                None, Alu.mult)
                        nc.vector.tensor_tensor(ye, msel(0), sv[:, 0],
                                                Alu.add)
                        nc.vector.tensor_tensor(yo, sv[:, 0], sv[:, 1],
                                                Alu.subtract)
                        nc.vector.tensor_tensor(yo, yo, msel(3),
                                                Alu.subtract)
                        nc.vector.tensor_tensor(ye, ye, sv[:, 1], Alu.add)
                        # kervolution square, in place on the h2 slice
                        ph = h2ap(m, jp)
                        if mode2[0] == SLOW:
                            nc.scalar.activation(ph, ph, Sq, bias=1.0,
                                                 scale=1.0)
                            nc.vector.tensor_scalar(
                                ph, ph, scb[:, SC2 + m:SC2 + m + 1],
                                shb[:, SH2 + m:SH2 + m + 1],
                                Alu.mult, Alu.add)
                        else:
                            nc.scalar.activation(
                                ph, ph, Sq,
                                bias=scb[:, SC2 + M1 + m:SC2 + M1 + m + 1],
                                scale=scb[:, SC2 + m:SC2 + m + 1])
                            if mode2[0] == FAST_T:
                                nc.vector.tensor_scalar(
                                    ph, ph, shb[:, SH2 + m:SH2 + m + 1],
                                    None, Alu.add)

                # conv3: 1x1, C_MID -> C_IN, (2mp, 2mp+1) paired per j when
                # uniform; + residual, store per (m-pair, j)
                pair3 = mode3[1] is not None
                # interleave next-pass conv1 chunks where ring/drain waits
                # would otherwise stall PE: before group 1 (covers the
                # conv2-j1/c2 drain latency) and mid-phase
                if npairs == 2:
                    c1_at = {0: 0, 4: 1}
                else:
                    c1_at = {0: 0, 2: 1}
                gidx = 0
                if pi == len(plan) - 1 and npairs == 2:
                    # last pass j-outer: j0's whole drain/store chain
                    # completes under j1's matmuls, halving the tail
                    order = [(mp, j) for j in range(npairs)
                             for mp in range(M3 // 2)]
                else:
                    order = [(mp, j) for mp in range(M3 // 2)
                             for j in range(npairs)]
                for mp, j in order:
                    if True:
                        if gidx in c1_at and c1_at[gidx] < len(next_chunks):
                            next_chunks[c1_at[gidx]]()
                        gidx += 1
                        ps = pspool.tile([128, 2 * HB], F32, tag="psp")
                        for mi in range(2):
                            m = 2 * mp + mi
                            for k in range(K2):
                                nc.tensor.matmul(
                                    ps[:, mi * HB:mi * HB + N],
                                    w3ap(k, m), h2ap(k, j),
                                    start=(k == 0), stop=(k == K2 - 1))
                        zt = opool.tile([128, 2 * N], BF, tag="z")
                        last_pass = pi == len(plan) - 1
                        if (last_pass and pair3 and mode3[0] == FAST_T0
                                and j == npairs - 1 and mp == 1):
                            # tail relief: one of the final drains on DVE so
                            # ACT's serial end-of-kernel train is shorter
                            tq3 = tpool.tile([128, 2 * N], BF, tag="tq")
                            tv3 = tq3[:].rearrange("c (m b) -> c m b", b=N)
                            src = (ps[:].rearrange("c (m b) -> c m b", m=2)
                                   [:, :, 0:N])
                            nc.vector.tensor_scalar(
                                tv3, src,
                                scb[:, SC3 + 2 * mp:SC3 + 2 * mp + 1],
                                scb[:, SC3 + 2 * mp:SC3 + 2 * mp + 1],
                                Alu.mult, Alu.add)
                            nc.vector.tensor_tensor(zt[:], tq3[:], tq3[:],
                                                    Alu.mult)
                        elif (last_pass and mp == M3 // 2 - 1
                                and j == npairs - 1):
                            # final group: single-m drains + residuals (the
                            # chain after the very last matmul halves), but
                            # ONE paired store (two stores would serialize
                            # their HWDGE generations + DGE delays)
                            for mi in range(2):
                                pointwise(mode3, ps[:, mi * HB:mi * HB + N],
                                          zt[:, mi * N:(mi + 1) * N],
                                          SC3, SH3, 2 * mp + mi)
                                zvi = (zt[:, mi * N:(mi + 1) * N]
                                       .rearrange("c (n s) -> c n s", n=2))
                                xvi = (xt[j][:]
                                       .rearrange("p (k n s) -> p k n s",
                                                  k=K1, n=2)
                                       [:, 2 * mp + mi, :, :])
                                nc.vector.tensor_tensor(zvi, zvi, xvi,
                                                        Alu.add)
                            zv = zt[:].rearrange("c (m n s) -> c m n s",
                                                 m=2, n=2)
                            dst = out_cm[:, 2 * mp:2 * mp + 2,
                                         2 * (q0 + j):2 * (q0 + j) + 2, :]
                            nc.sync.dma_start(dst, zv)
                            continue
                        elif pair3:
                            src = (ps[:].rearrange("c (m b) -> c m b", m=2)
                                   [:, :, 0:N])
                            pointwise(mode3, src, zt[:], SC3, SH3, 2 * mp)
                        else:
                            for mi in range(2):
                                pointwise(mode3, ps[:, mi * HB:mi * HB + N],
                                          zt[:, mi * N:(mi + 1) * N],
                                          SC3, SH3, 2 * mp + mi)
                        zv = zt[:].rearrange("c (m n s) -> c m n s",
                                             m=2, n=2)
                        xv = (xt[j][:].rearrange("p (k n s) -> p k n s",
                                                 k=K1, n=2)
                              [:, 2 * mp:2 * mp + 2, :, :])
                        nc.vector.tensor_tensor(zv, zv, xv, Alu.add)
                        dst = out_cm[:, 2 * mp:2 * mp + 2,
                                     2 * (q0 + j):2 * (q0 + j) + 2, :]
                        nc.sync.dma_start(dst, zv)

            if reps is None:
                emit_passes()
            else:
                with tc.For_i(0, reps, 1):
                    emit_passes()

    nc.compile()
    return nc


# ---------------- host side ----------------

_CACHE = {}


def _get_runner(modes):
    if modes in _CACHE:
        return _CACHE[modes]
    import jax
    from jax.experimental.shard_map import shard_map
    from jax.sharding import Mesh, PartitionSpec
    from concourse.bass2jax import (_bass_exec_p, install_neuronx_cc_hook,
                                    partition_id_tensor)

    nc = _build(modes)
    install_neuronx_cc_hook()
    partition_name = nc.partition_id_tensor.name if nc.partition_id_tensor else None
    in_names, out_names, out_avals = [], [], []
    for alloc in nc.m.functions[0].allocations:
        if not isinstance(alloc, mybir.MemoryLocationSet):
            continue
        name = alloc.memorylocations[0].name
        if alloc.kind == "ExternalInput":
            if name != partition_name:
                in_names.append(name)
        elif alloc.kind == "ExternalOutput":
            out_names.append(name)
            out_avals.append(jax.core.ShapedArray(
                tuple(alloc.tensor_shape), mybir.dt.np(alloc.dtype)))
    n_params, n_outs = len(in_names), len(out_avals)
    all_in_names = list(in_names) + list(out_names)
    if partition_name is not None:
        all_in_names.append(partition_name)

    def _body(*args):
        operands = list(args)
        if partition_name is not None:
            operands.append(partition_id_tensor())
        outs = _bass_exec_p.bind(
            *operands,
            out_avals=tuple(out_avals),
            in_names=tuple(all_in_names),
            out_names=tuple(out_names),
            lowering_input_output_aliases=(),
            sim_require_finite=True,
            sim_require_nnan=True,
            nc=nc,
        )
        return tuple(outs)

    devices = jax.devices()[:8]
    mesh = Mesh(np.asarray(devices), ("core",))
    sharded = jax.jit(
        shard_map(_body, mesh=mesh,
                  in_specs=(PartitionSpec("core"),) * (n_params + n_outs),
                  out_specs=(PartitionSpec("core"),) * n_outs,
                  check_rep=False),
        donate_argnums=tuple(range(n_params, n_params + n_outs)),
        keep_unused=True,
    )
    sharding = jax.sharding.NamedSharding(mesh, PartitionSpec("core"))
    runner = dict(nc=nc, sharded=sharded, sharding=sharding, jax=jax,
                  in_names=in_names, out_names=out_names, out_avals=out_avals)
    _CACHE[modes] = runner
    return runner


def _vec_tile(v, m_tiles):
    """[C] -> [128, m_tiles] column-per-m-tile layout."""
    return np.ascontiguousarray(np.asarray(v).reshape(m_tiles, 128).T
                                .astype(np.float32))


def _bf16(a):
    import ml_dtypes
    return np.asarray(a, dtype=np.float32).astype(ml_dtypes.bfloat16)


def prepare(w1, w2, w3, g1, b1, m1, v1, g2, b2, m2, v2, g3, b3, m3, v3):
    """Host prep: returns (modes, shared_input_dict_without_x)."""
    s1 = g1 / np.sqrt(v1 + EPS)
    t1 = b1 - m1 * s1
    s2 = g2 / np.sqrt(v2 + EPS)
    t2 = b2 - m2 * s2
    s3 = g3 / np.sqrt(v3 + EPS)
    t3 = b3 - m3 * s3

    def mode_of(s, t):
        """Returns (mode, const): const = sqrt(s) as a float when s is
        exactly channel-uniform and shifts are zero (enables m-paired
        PSUM drains on ACT), else None."""
        if np.all(s > 0):
            if not np.any(t):
                r = np.sqrt(s)
                const = float(r[0]) if np.all(s == s[0]) else None
                return (FAST_T0, const)
            return (FAST_T, None)
        return (SLOW, None)

    modes = (mode_of(s1, t1), mode_of(s2, t2), mode_of(s3, t3))

    def sc_bi(lmode, s, m_tiles):
        mode = lmode[0]
        if mode == SLOW:
            return _vec_tile(s, m_tiles), np.ones((128, m_tiles), np.float32)
        r = np.sqrt(s)
        return _vec_tile(r, m_tiles), _vec_tile(r, m_tiles)

    sc1, bi1 = sc_bi(modes[0], s1, M1)
    sc2, bi2 = sc_bi(modes[1], s2, M1)
    sc3, bi3 = sc_bi(modes[2], s3, M3)
    scb = np.concatenate([sc1, bi1, sc2, bi2, sc3, bi3], axis=1)
    shb = np.concatenate([_vec_tile(t1, M1), _vec_tile(t2, M1),
                          _vec_tile(t3, M3)], axis=1)

    w1t = _bf16(np.ascontiguousarray(w1[:, :, 0, 0].T))          # [1024,256]
    # w2: [o, i, kh, kw] -> 1D-winograd F(2,3) along kw: U_j = G @ w[kw]
    G = np.array([[1, 0, 0], [.5, .5, .5], [.5, -.5, .5], [0, 0, 1]],
                 np.float64)
    U = np.einsum('jw,oihw->oihj', G, w2.astype(np.float64))  # [o,i,kh,j]
    w2t = _bf16(np.ascontiguousarray(
        U.transpose(1, 2, 3, 0)                   # [i, kh, j, o]
          .reshape(K2, 128, 3, 4, C_MID)          # [kt, i128, kh, j, o]
          .transpose(3, 0, 2, 1, 4)))             # [j, kt, kh, i128, o]
    w3t = _bf16(np.ascontiguousarray(w3[:, :, 0, 0].T))          # [256,1024]

    shared = dict(w1t=w1t, w2t=w2t, w3t=w3t, scb=scb, shb=shb)
    return modes, shared


def kernel(**inputs):
    inputs = {k: np.asarray(v) for k, v in inputs.items()}
    x = inputs.pop("x").astype(np.float32)
    modes, shared = prepare(**inputs)
    r = _get_runner(modes)
    jax = r["jax"]

    n_cores = 8
    # x: [128, 1024, 14, 14] -> [core(8) x k(8), 128, 16, 196] bf16 channel-major
    x_cm = _bf16(x.reshape(8, B, K1, 128, S)
                 .transpose(0, 2, 3, 1, 4)
                 .reshape(n_cores * K1, 128, B, S))
    dev_in = []
    for name in r["in_names"]:
        if name == "x":
            cat = x_cm
        else:
            a = shared[name]
            cat = np.concatenate([a] * n_cores, axis=0)
        dev_in.append(jax.device_put(cat, r["sharding"]))
    zero_outs = [
        jax.device_put(np.zeros((n_cores * av.shape[0], *av.shape[1:]), av.dtype),
                       r["sharding"])
        for av in r["out_avals"]
    ]
    outs = r["sharded"](*dev_in, *zero_outs)
    jax.block_until_ready(outs)
    out = np.asarray(outs[r["out_names"].index("out")])
    # [core x m(8), 128, 16, 196] bf16 -> [128, 1024, 14, 14] f32
    return np.ascontiguousarray(
        out.reshape(n_cores, M3, 128, B, S)
           .transpose(0, 3, 1, 2, 4)
           .reshape(128, C_IN, HW, HW)).astype(np.float32)
